# revision 5
# baseline (speedup 1.0000x reference)
"""Trainium2 Bass kernel for nn_InvDirectImageAlign (inverse-compositional image alignment).

Per core (8 cores): 2 batch elements. 5 launches of ONE compiled NEFF (one per
Gauss-Newton iteration); host does only the tiny O(B) 6x6 solve + se3_exp
between launches. Device does warp, bilinear grid_sample (GPSIMD ap_gather
from fp16 pair-dup band tables), Jacobian combos, and the JtWJ/Rhs reduction.

Chunking: (batch, 16-row y-band, 224-col x-half) = 80 chunks/core; the 8
GPSIMD partition-groups each own one chunk per superstep; 10 supersteps.
Two pixel layouts, bridged only by PE transposes of gathered data:
  mod-128:    pixel j of chunk(g,s) at partition j%128, free col (g, j//128)
  wrapped-16: pixel j at partition 16g + j%16, free col j//16   (ap_gather's
              index layout; idx math is recomputed here, ~30 cheap ops)
"""
import numpy as np

B, C, H, W = 16, 3, 320, 448
HW = H * W
N_ITERS = 5
LAMBDA = 0.01
HUBER_DELTA = 0.1
EPS = 1e-6

BH = 16            # band rows per chunk
CW = 224           # band cols per chunk
N = BH * CW        # 3584 px per chunk
A = N // 128       # 28
M = N // 16        # 224
NS = 10            # supersteps
TR = 67            # table rows (16 + 25 + 26)
TC = 266           # table cols (224 + 20 + 21 + 1)
NELEM = TR * TC    # 17822 pairs
YPAD = 25
XPAD = 20


def skew3(w):
    x, y, z = w[..., 0], w[..., 1], w[..., 2]
    O = np.zeros_like(x)
    return np.stack([np.stack([O, -z, y], -1),
                     np.stack([z, O, -x], -1),
                     np.stack([-y, x, O], -1)], -2)


def se3_exp(xi):
    xi = np.asarray(xi, np.float64)
    v, w = xi[:, :3], xi[:, 3:]
    th2 = np.sum(w * w, -1)[:, None, None]
    th2c = np.maximum(th2, 1e-16)
    th = np.sqrt(th2c)
    small = th2 < 1e-10
    Aa = np.where(small, 1.0 - th2 / 6.0, np.sin(th) / th)
    Bc = np.where(small, 0.5 - th2 / 24.0, (1.0 - np.cos(th)) / th2c)
    Cc = np.where(small, 1.0 / 6.0 - th2 / 120.0, (1.0 - Aa) / th2c)
    K = skew3(w)
    K2 = K @ K
    I = np.eye(3)
    R = I + Aa * K + Bc * K2
    V = I + Bc * K + Cc * K2
    t = np.einsum('bij,bj->bi', V, v)
    T = np.zeros((xi.shape[0], 4, 4))
    T[:, :3, :3] = R
    T[:, :3, 3] = t
    T[:, 3, 3] = 1.0
    return T.astype(np.float32)


def feature_gradient(img):
    p = np.pad(img, ((0, 0), (0, 0), (0, 0), (1, 1)), mode='edge')
    dx = 0.5 * (p[..., 2:] - p[..., :-2])
    p = np.pad(img, ((0, 0), (0, 0), (1, 1), (0, 0)), mode='edge')
    dy = 0.5 * (p[..., 2:, :] - p[..., :-2, :])
    return dx.astype(np.float32), dy.astype(np.float32)


def chunk_of(g, s):
    b = g // 4
    local = (g % 4) * 10 + s
    return b, local // 2, local % 2


def bases_of(yb, xh):
    r0, c0 = yb * BH, xh * CW
    rbase = int(np.clip(r0 - YPAD, 0, H - TR))
    cbase = int(np.clip(c0 - XPAD, 0, W - (TC - 1)))
    return rbase, cbase


def mod128_cols(plane_bhw):
    """[2,(K,)H,W] -> [128, NS, 8, A, K]."""
    x = np.asarray(plane_bhw)
    K = 1 if x.ndim == 3 else x.shape[1]
    x = x.reshape(2, K, H, W)
    out = np.zeros((128, NS, 8, A, K), x.dtype)
    for g in range(8):
        for s in range(NS):
            b, yb, xh = chunk_of(g, s)
            r0, c0 = yb * BH, xh * CW
            blk = x[b, :, r0:r0 + BH, c0:c0 + CW].reshape(K, N)
            out[:, s, g, :, :] = blk.reshape(K, A, 128).transpose(2, 1, 0)
    return out


def wrap16_cols(plane_bhw):
    """[2,H,W] -> [128, NS, M]."""
    x = np.asarray(plane_bhw).reshape(2, H, W)
    out = np.zeros((128, NS, M), x.dtype)
    for g in range(8):
        for s in range(NS):
            b, yb, xh = chunk_of(g, s)
            r0, c0 = yb * BH, xh * CW
            blk = x[b, r0:r0 + BH, c0:c0 + CW].reshape(N)
            out[16 * g:16 * g + 16, s, :] = blk.reshape(M, 16).T
    return out


def host_precompute(pose_twist2, I0_2, I1_2, invD0_2, invD1_2, intr2):
    T0 = se3_exp(pose_twist2)
    fx = intr2[:, 0][:, None, None]; fy = intr2[:, 1][:, None, None]
    cx = intr2[:, 2][:, None, None]; cy = intr2[:, 3][:, None, None]
    uu = np.arange(W, dtype=np.float32)[None, None, :]
    vv = np.arange(H, dtype=np.float32)[None, :, None]
    iD = np.maximum(invD1_2[:, 0], EPS).astype(np.float32)
    z1 = (1.0 / iD).astype(np.float32)
    x1 = ((uu - cx) / fx * z1).astype(np.float32)
    y1 = ((vv - cy) / fy * z1).astype(np.float32)
    R0, t0 = T0[:, :3, :3], T0[:, :3, 3]
    X0 = np.einsum('bij,bhwj->bhwi', R0, np.stack([x1, y1, z1], -1)) + t0[:, None, None, :]
    X0 = X0.astype(np.float32)
    z0 = X0[..., 2]
    z0s = np.where(np.abs(z0) > EPS, z0, EPS).astype(np.float32)
    iz = (1.0 / z0s).astype(np.float32)
    xh_, yh_ = X0[..., 0], X0[..., 1]
    O = np.zeros_like(z0)
    Jp = np.stack([np.stack([fx * iz + O, O, -fx * xh_ * iz * iz], -1),
                   np.stack([O, fy * iz + O, -fy * yh_ * iz * iz], -1)], -2).astype(np.float32)
    I3 = np.broadcast_to(np.eye(3, dtype=np.float32), X0.shape[:3] + (3, 3))
    Jt = np.concatenate([I3, -skew3(X0)], -1).astype(np.float32)
    Jw = np.einsum('bhwij,bhwjk->bhwik', Jp, Jt).astype(np.float32)
    A6 = (-Jw[..., 0, :]).astype(np.float32)
    B6 = (-Jw[..., 1, :]).astype(np.float32)
    T6 = Jt[..., 2, :].astype(np.float32)

    dI0x, dI0y = feature_gradient(I0_2)
    dD0x, dD0y = feature_gradient(invD0_2)
    planes12 = np.concatenate([dI0x, dI0y, dD0x, dD0y, I0_2, invD0_2], axis=1).astype(np.float32)
    flat = planes12.reshape(2, 12, HW)
    pd = np.zeros((2, 12, HW + 1, 2), np.float16)
    pd[:, :, 1:, 0] = flat.astype(np.float16)
    pd[:, :, :HW, 1] = flat.astype(np.float16)

    inp = {}
    inp["pd"] = np.ascontiguousarray(pd.reshape(2, 12, (HW + 1) * 2))
    ABT = np.concatenate([np.moveaxis(A6, -1, 1), np.moveaxis(B6, -1, 1),
                          np.moveaxis(T6, -1, 1)], axis=1)  # [2, 18, H, W] k = j*6+x
    inp["abtm"] = np.ascontiguousarray(
        mod128_cols(ABT).astype(np.float16).reshape(128, NS * 8 * A * 18))
    X1 = np.stack([x1, y1, z1], 1)
    inp["x1m"] = np.ascontiguousarray(
        mod128_cols(X1).astype(np.float32).reshape(128, NS * 8 * A * 3))
    inp["x1w"] = np.ascontiguousarray(
        np.stack([wrap16_cols(X1[:, k]) for k in range(3)], -1)
        .astype(np.float32).reshape(128, NS * M * 3))
    inp["i1m"] = np.ascontiguousarray(
        mod128_cols(I1_2).astype(np.float16).reshape(128, NS * 8 * A * 3))

    bw = np.zeros((128, NS, 4), np.float32)
    for g in range(8):
        for s in range(NS):
            b, yb, xh = chunk_of(g, s)
            rbase, cbase = bases_of(yb, xh)
            bw[16 * g:16 * g + 16, s, 0] = rbase
            bw[16 * g:16 * g + 16, s, 1] = cbase - 1          # xf min
            bw[16 * g:16 * g + 16, s, 2] = cbase + (TC - 2)   # xf max
            bw[16 * g:16 * g + 16, s, 3] = 1 - cbase          # kx offset
    inp["bw"] = np.ascontiguousarray(bw.reshape(128, NS * 4))
    inp["idn"] = np.eye(128, dtype=np.float16)
    return inp, dict(T0=T0)


def host_iter_params(T2, intr2):
    R = T2[:, :3, :3].astype(np.float32); t = T2[:, :3, 3].astype(np.float32)
    q = np.zeros((2, 16), np.float32)
    q[:, :9] = R.reshape(2, 9)
    q[:, 9:12] = t
    q[:, 12:16] = intr2
    rtm = np.zeros((128, 16, 8), np.float32)
    rtw = np.zeros((128, 16), np.float32)
    for g in range(8):
        b = g // 4
        rtm[:, :, g] = q[b][None, :]
        rtw[16 * g:16 * g + 16, :] = q[b][None, :]
    return {"rtm": np.ascontiguousarray(rtm.reshape(128, 16 * 8)),
            "rtw": rtw}


_NC_CACHE = {}
PROFILE = False
LAST_EXEC_NS = []
LAST_TRACES = []
LAST_WALL = []


def build_nc(debug=False):
    import concourse.bacc as bacc
    import concourse.bass as bass
    import concourse.tile as tile
    from concourse import mybir

    fp32 = mybir.dt.float32
    fp16 = mybir.dt.float16
    i16 = mybir.dt.int16
    AL = mybir.AluOpType
    ACT = mybir.ActivationFunctionType
    AX = mybir.AxisListType

    nc = bacc.Bacc("TRN2", target_bir_lowering=False, debug=False, num_devices=8)

    pd_in = nc.dram_tensor("pd", [2, 12, (HW + 1) * 2], fp16, kind="ExternalInput")
    abtm_in = nc.dram_tensor("abtm", [128, NS * 8 * A * 18], fp16, kind="ExternalInput")
    x1m_in = nc.dram_tensor("x1m", [128, NS * 8 * A * 3], fp32, kind="ExternalInput")
    x1w_in = nc.dram_tensor("x1w", [128, NS * M * 3], fp32, kind="ExternalInput")
    i1m_in = nc.dram_tensor("i1m", [128, NS * 8 * A * 3], fp16, kind="ExternalInput")
    bw_in = nc.dram_tensor("bw", [128, NS * 4], fp32, kind="ExternalInput")
    idn_in = nc.dram_tensor("idn", [128, 128], fp16, kind="ExternalInput")
    rtm_in = nc.dram_tensor("rtm", [128, 16 * 8], fp32, kind="ExternalInput")
    rtw_in = nc.dram_tensor("rtw", [128, 16], fp32, kind="ExternalInput")
    out_ext = nc.dram_tensor("sums", [128, 64], fp32, kind="ExternalOutput")
    if debug:
        dbg_ext = nc.dram_tensor("dbg", [128, A * 128 + 5 * 8 * A], fp32, kind="ExternalOutput")
        dbw_ext = nc.dram_tensor("dbw", [128, 2 * M], fp32, kind="ExternalOutput")

    with tile.TileContext(nc) as tc:
        with tc.tile_pool(name="cst", bufs=1) as cpool, \
             tc.tile_pool(name="tblp", bufs=1) as tpool, \
             tc.tile_pool(name="strm", bufs=2) as sp, \
             tc.tile_pool(name="scr", bufs=1) as sc, \
             tc.tile_pool(name="gath", bufs=1) as gp, \
             tc.tile_pool(name="ps", bufs=2, space="PSUM") as pp, \
             tc.tile_pool(name="accp", bufs=1) as accp:

            rtm = cpool.tile([128, 16 * 8], fp32, tag="rtm")
            rtw = cpool.tile([128, 16], fp32, tag="rtw")
            bwc = cpool.tile([128, NS * 4], fp32, tag="bw")
            idn = cpool.tile([128, 128], fp16, tag="idn")
            acc = accp.tile([128, 64], fp32, tag="acc")
            nc.sync.dma_start(out=rtm[:, :], in_=rtm_in.ap())
            nc.sync.dma_start(out=rtw[:, :], in_=rtw_in.ap())
            nc.sync.dma_start(out=bwc[:, :], in_=bw_in.ap())
            nc.sync.dma_start(out=idn[:, :], in_=idn_in.ap())
            nc.vector.memset(acc[:, :], 0.0)

            def rq(qi):   # mod-128 per-group broadcast: dims (g x8, a x A step0)
                sl = rtm[:, qi * 8:(qi + 1) * 8]
                return bass.AP(sl.tensor, sl.offset, [list(sl.ap[0]), [1, 8], [0, A]])

            def rqw(qi):  # wrapped per-partition scalar bcast over M
                sl = rtw[:, qi:qi + 1]
                return bass.AP(sl.tensor, sl.offset, [list(sl.ap[0]), [0, M]])

            def bwq(s, j):
                sl = bwc[:, s * 4 + j:s * 4 + j + 1]
                return bass.AP(sl.tensor, sl.offset, [list(sl.ap[0]), [0, M]])

            TT = nc.vector.tensor_tensor
            TS = lambda out, in0, s1, op: nc.vector.tensor_scalar(out, in0, s1, None, op)
            TS2 = lambda out, in0, s1, s2, op0, op1: nc.vector.tensor_scalar(out, in0, s1, s2, op0, op1)

            for s in range(NS):
                tbl = tpool.tile([128, NELEM * 2], fp16, tag="tbl")
                for g in range(8):
                    b, yb, xh = chunk_of(g, s)
                    rbase, cbase = bases_of(yb, xh)
                    start = (rbase * W + cbase) * 2
                    src0 = pd_in.ap()
                    # pd [2, 12, (HW+1)*2]: offset = b*12*(HW+1)*2 + plane*(HW+1)*2 + start
                    src = bass.AP(src0.tensor,
                                  src0.offset + b * 12 * (HW + 1) * 2 + start,
                                  [[(HW + 1) * 2, 12], [W * 2, TR], [1, TC * 2]])
                    dsl = tbl[16 * g:16 * g + 12, :]
                    dst = bass.AP(dsl.tensor, dsl.offset,
                                  [[dsl.ap[0][0], 12], [TC * 2, TR], [1, TC * 2]])
                    nc.sync.dma_start(out=dst, in_=src)

                x1w = sp.tile([128, M * 3], fp32, tag="x1w")
                nc.sync.dma_start(out=x1w[:, :], in_=x1w_in.ap()[:, s * M * 3:(s + 1) * M * 3])
                x1m = sp.tile([128, 8 * A * 3], fp32, tag="x1m")
                nc.sync.dma_start(out=x1m[:, :], in_=x1m_in.ap()[:, s * 8 * A * 3:(s + 1) * 8 * A * 3])
                abt = sp.tile([128, 8 * A * 18], fp16, tag="abt")
                nc.sync.dma_start(out=abt[:, :], in_=abtm_in.ap()[:, s * 8 * A * 18:(s + 1) * 8 * A * 18])
                i1 = sp.tile([128, 8 * A * 3], fp16, tag="i1")
                nc.sync.dma_start(out=i1[:, :], in_=i1m_in.ap()[:, s * 8 * A * 3:(s + 1) * 8 * A * 3])

                # ---------- wrapped-16 idx pipeline ----------
                def xw(k):
                    sl = x1w[:, :]
                    return bass.AP(sl.tensor, sl.offset + k, [list(sl.ap[0]), [3, M]])

                def tw(name):
                    return sc.tile([128, M], fp32, name="w_" + name + f"_{s}", tag="w_" + name)

                def matvec(dst, aps, qis, t1):
                    TT(dst[:, :], aps[0], qis[0], op=AL.mult)
                    TT(t1[:, :], aps[1], qis[1], op=AL.mult)
                    TT(dst[:, :], dst[:, :], t1[:, :], op=AL.add)
                    TT(t1[:, :], aps[2], qis[2], op=AL.mult)
                    TT(dst[:, :], dst[:, :], t1[:, :], op=AL.add)
                    TT(dst[:, :], dst[:, :], qis[3], op=AL.add)

                t1w = tw("t1"); t2w = tw("t2")
                X0zw = tw("X0z")
                matvec(X0zw, [xw(0), xw(1), xw(2)], [rqw(6), rqw(7), rqw(8), rqw(11)], t1w)
                X0xw = tw("X0x")
                matvec(X0xw, [xw(0), xw(1), xw(2)], [rqw(0), rqw(1), rqw(2), rqw(9)], t1w)
                X0yw = tw("X0y")
                matvec(X0yw, [xw(0), xw(1), xw(2)], [rqw(3), rqw(4), rqw(5), rqw(10)], t1w)

                def safe_recip2(dst, z, t1, t2):
                    nc.scalar.activation(t1[:, :], z[:, :], ACT.Abs)
                    TS(t2[:, :], t1[:, :], EPS, AL.is_gt)              # m
                    TT(t1[:, :], z[:, :], t2[:, :], op=AL.mult)        # z*m
                    TS2(t2[:, :], t2[:, :], 1.0, -EPS, AL.subtract, AL.mult)  # EPS*(1-m)
                    TT(t1[:, :], t1[:, :], t2[:, :], op=AL.add)        # zs
                    nc.vector.reciprocal_approx_fast(dst[:, :], t1[:, :])

                izw = tw("iz")
                safe_recip2(izw, X0zw, t1w, t2w)
                u0w = tw("u0"); v0w = tw("v0")
                TT(u0w[:, :], X0xw[:, :], izw[:, :], op=AL.mult)
                TT(u0w[:, :], u0w[:, :], rqw(12), op=AL.mult)
                TT(u0w[:, :], u0w[:, :], rqw(14), op=AL.add)
                TT(v0w[:, :], X0yw[:, :], izw[:, :], op=AL.mult)
                TT(v0w[:, :], v0w[:, :], rqw(13), op=AL.mult)
                TT(v0w[:, :], v0w[:, :], rqw(15), op=AL.add)
                TS2(u0w[:, :], u0w[:, :], -0.5 * (W - 1), 1.5 * (W - 1), AL.max, AL.min)
                TS2(v0w[:, :], v0w[:, :], -0.5 * (H - 1), 1.5 * (H - 1), AL.max, AL.min)
                x0fw = tw("x0f"); y0fw = tw("y0f")
                fi32w = sc.tile([128, M], mybir.dt.int32, name=f"fi32w_{s}", tag="fi32w")
                TS(t1w[:, :], u0w[:, :], 0.5, AL.subtract)
                nc.vector.tensor_copy(fi32w[:, :], t1w[:, :])
                nc.vector.tensor_copy(x0fw[:, :], fi32w[:, :])
                TS(t1w[:, :], v0w[:, :], 0.5, AL.subtract)
                nc.vector.tensor_copy(fi32w[:, :], t1w[:, :])
                nc.vector.tensor_copy(y0fw[:, :], fi32w[:, :])
                xfw = tw("xf")
                TT(xfw[:, :], x0fw[:, :], bwq(s, 1), op=AL.max)
                TT(xfw[:, :], xfw[:, :], bwq(s, 2), op=AL.min)
                TT(t2w[:, :], xfw[:, :], bwq(s, 3), op=AL.add)         # kx = xf + 1 - cbase
                ktw = tw("kt"); kbw = tw("kb")
                TT(ktw[:, :], y0fw[:, :], bwq(s, 0), op=AL.subtract)
                TS2(ktw[:, :], ktw[:, :], 0.0, float(TR - 1), AL.max, AL.min)
                TS(ktw[:, :], ktw[:, :], float(TC), AL.mult)
                TT(ktw[:, :], ktw[:, :], t2w[:, :], op=AL.add)
                TT(kbw[:, :], y0fw[:, :], bwq(s, 0), op=AL.subtract)
                TS2(kbw[:, :], kbw[:, :], 1.0, 0.0, AL.add, AL.max)
                TS(kbw[:, :], kbw[:, :], float(TR - 1), AL.min)
                TS(kbw[:, :], kbw[:, :], float(TC), AL.mult)
                TT(kbw[:, :], kbw[:, :], t2w[:, :], op=AL.add)
                kt16 = sc.tile([128, M], i16, tag="kt16")
                kb16 = sc.tile([128, M], i16, tag="kb16")
                nc.vector.tensor_copy(kt16[:, :], ktw[:, :])
                nc.vector.tensor_copy(kb16[:, :], kbw[:, :])

                gt = gp.tile([128, N * 2], fp16, tag="gt")
                gb = gp.tile([128, N * 2], fp16, tag="gb")
                nc.gpsimd.ap_gather(gt[:, :], tbl[:, :], kt16[:, :],
                                    channels=128, num_elems=NELEM, d=2, num_idxs=N)
                nc.gpsimd.ap_gather(gb[:, :], tbl[:, :], kb16[:, :],
                                    channels=128, num_elems=NELEM, d=2, num_idxs=N)

                # ---------- mod-128 warp pipeline ----------
                def xm(k):
                    sl = x1m[:, :]
                    return bass.AP(sl.tensor, sl.offset + k, [list(sl.ap[0]), [3, 8 * A]])

                def tm(name):
                    return sc.tile([128, 8 * A], fp32, name="m_" + name + f"_{s}", tag="m_" + name)

                m1 = tm("m1"); m2 = tm("m2")
                X0z = tm("X0z")
                matvec(X0z, [xm(0), xm(1), xm(2)], [rq(6), rq(7), rq(8), rq(11)], m1)
                X0x = tm("X0x")
                matvec(X0x, [xm(0), xm(1), xm(2)], [rq(0), rq(1), rq(2), rq(9)], m1)
                X0y = tm("X0y")
                matvec(X0y, [xm(0), xm(1), xm(2)], [rq(3), rq(4), rq(5), rq(10)], m1)
                iz = tm("iz")
                safe_recip2(iz, X0z, m1, m2)
                u0 = tm("u0"); v0 = tm("v0")
                TT(u0[:, :], X0x[:, :], iz[:, :], op=AL.mult)
                TT(u0[:, :], u0[:, :], rq(12), op=AL.mult)
                TT(u0[:, :], u0[:, :], rq(14), op=AL.add)
                TT(v0[:, :], X0y[:, :], iz[:, :], op=AL.mult)
                TT(v0[:, :], v0[:, :], rq(13), op=AL.mult)
                TT(v0[:, :], v0[:, :], rq(15), op=AL.add)
                vmask = tm("vmask")
                TS(vmask[:, :], X0z[:, :], EPS, AL.is_gt)
                TS(m1[:, :], u0[:, :], 0.0, AL.is_gt)
                TT(vmask[:, :], vmask[:, :], m1[:, :], op=AL.mult)
                TS(m1[:, :], u0[:, :], float(W - 1), AL.is_lt)
                TT(vmask[:, :], vmask[:, :], m1[:, :], op=AL.mult)
                TS(m1[:, :], v0[:, :], 0.0, AL.is_gt)
                TT(vmask[:, :], vmask[:, :], m1[:, :], op=AL.mult)
                TS(m1[:, :], v0[:, :], float(H - 1), AL.is_lt)
                TT(vmask[:, :], vmask[:, :], m1[:, :], op=AL.mult)
                TS2(u0[:, :], u0[:, :], -0.5 * (W - 1), 1.5 * (W - 1), AL.max, AL.min)
                TS2(v0[:, :], v0[:, :], -0.5 * (H - 1), 1.5 * (H - 1), AL.max, AL.min)
                wx = tm("wx"); wy = tm("wy"); x0f = tm("x0f"); y0f = tm("y0f")
                fi32m = sc.tile([128, 8 * A], mybir.dt.int32, name=f"fi32m_{s}", tag="fi32m")
                TS(m1[:, :], u0[:, :], 0.5, AL.subtract)
                nc.vector.tensor_copy(fi32m[:, :], m1[:, :])
                nc.vector.tensor_copy(x0f[:, :], fi32m[:, :])
                TT(wx[:, :], u0[:, :], x0f[:, :], op=AL.subtract)
                TS(m1[:, :], v0[:, :], 0.5, AL.subtract)
                nc.vector.tensor_copy(fi32m[:, :], m1[:, :])
                nc.vector.tensor_copy(y0f[:, :], fi32m[:, :])
                TT(wy[:, :], v0[:, :], y0f[:, :], op=AL.subtract)
                mx0 = tm("mx0"); mx1 = tm("mx1"); my0 = tm("my0"); my1 = tm("my1")
                TS(mx0[:, :], x0f[:, :], -0.5, AL.is_gt)
                TS(m1[:, :], x0f[:, :], float(W - 1) + 0.5, AL.is_lt)
                TT(mx0[:, :], mx0[:, :], m1[:, :], op=AL.mult)
                TS(mx1[:, :], x0f[:, :], -1.5, AL.is_gt)
                TS(m1[:, :], x0f[:, :], float(W - 2) + 0.5, AL.is_lt)
                TT(mx1[:, :], mx1[:, :], m1[:, :], op=AL.mult)
                TS(my0[:, :], y0f[:, :], -0.5, AL.is_gt)
                TS(m1[:, :], y0f[:, :], float(H - 1) + 0.5, AL.is_lt)
                TT(my0[:, :], my0[:, :], m1[:, :], op=AL.mult)
                TS(my1[:, :], y0f[:, :], -1.5, AL.is_gt)
                TS(m1[:, :], y0f[:, :], float(H - 2) + 0.5, AL.is_lt)
                TT(my1[:, :], my1[:, :], m1[:, :], op=AL.mult)
                W00 = tm("W00"); W01 = tm("W01"); W10 = tm("W10"); W11 = tm("W11")
                TS2(m1[:, :], wx[:, :], 1.0, -1.0, AL.subtract, AL.mult)  # 1-wx
                TS2(m2[:, :], wy[:, :], 1.0, -1.0, AL.subtract, AL.mult)  # 1-wy
                TT(W00[:, :], m1[:, :], m2[:, :], op=AL.mult)
                TT(W00[:, :], W00[:, :], mx0[:, :], op=AL.mult)
                TT(W00[:, :], W00[:, :], my0[:, :], op=AL.mult)
                TT(W01[:, :], wx[:, :], m2[:, :], op=AL.mult)
                TT(W01[:, :], W01[:, :], mx1[:, :], op=AL.mult)
                TT(W01[:, :], W01[:, :], my0[:, :], op=AL.mult)
                TT(W10[:, :], m1[:, :], wy[:, :], op=AL.mult)
                TT(W10[:, :], W10[:, :], mx0[:, :], op=AL.mult)
                TT(W10[:, :], W10[:, :], my1[:, :], op=AL.mult)
                TT(W11[:, :], wx[:, :], wy[:, :], op=AL.mult)
                TT(W11[:, :], W11[:, :], mx1[:, :], op=AL.mult)
                TT(W11[:, :], W11[:, :], my1[:, :], op=AL.mult)

                # ---------- PE transpose + combine ----------
                samp = sc.tile([128, A * 128], fp32, tag="samp")
                ctmp = sc.tile([128, 512], fp32, tag="ctmp")
                for a4 in range(A // 4):
                    pts = {}
                    for ci, (gsrc, e) in enumerate(((gt, 0), (gt, 1), (gb, 0), (gb, 1))):
                        pt = pp.tile([128, 512], fp16, tag=f"pt{ci}")
                        pts[ci] = pt
                        for aa in range(4):
                            a = a4 * 4 + aa
                            src = bass.AP(gsrc.tensor, gsrc.offset + (a * 128 * 2 + e),
                                          [list(gsrc.ap[0]), [2, 128]])
                            nc.tensor.transpose(pt[:, aa * 128:(aa + 1) * 128], src, idn[:, :])
                    # combine 4 a-blocks at once; free dims (aa x4, g x8, q x16)
                    for ci, wt_ in ((0, W00), (1, W01), (2, W10), (3, W11)):
                        pt = pts[ci]
                        pap = bass.AP(pt.tensor, pt.offset, [list(pt.ap[0]), [128, 4], [16, 8], [1, 16]])
                        woff = wt_.offset + a4 * 4
                        wap = bass.AP(wt_.tensor, woff, [list(wt_.ap[0]), [1, 4], [A, 8], [0, 16]])
                        dst_off = samp.offset + a4 * 4 * 128
                        dap = bass.AP(samp.tensor, dst_off, [list(samp.ap[0]), [128, 4], [16, 8], [1, 16]])
                        if ci == 0:
                            TT(dap, pap, wap, op=AL.mult)
                        else:
                            tap = bass.AP(ctmp.tensor, ctmp.offset, [list(ctmp.ap[0]), [128, 4], [16, 8], [1, 16]])
                            TT(tap, pap, wap, op=AL.mult)
                            TT(dap, dap, tap, op=AL.add)

                # ---------- residuals, huber, S-scalars ----------
                def sq(q):
                    sl = samp[:, :]
                    return bass.AP(sl.tensor, sl.offset + q, [list(sl.ap[0]), [16, 8], [128, A]])

                def i1q(c):
                    sl = i1[:, :]
                    return bass.AP(sl.tensor, sl.offset + c, [list(sl.ap[0]), [3 * A, 8], [3, A]])

                SAA = tm("SAA"); SAB = tm("SAB"); SBB = tm("SBB")
                SAT = tm("SAT"); SBT = tm("SBT")
                rhoA = tm("rhoA"); rhoB = tm("rhoB"); rhoT = tm("rhoT")
                wz = tm("wz"); rz = tm("rz")
                r1 = tm("r1"); w1 = tm("w1"); one_m = tm("one_m")
                TS2(one_m[:, :], vmask[:, :], 1.0, -1e-6, AL.subtract, AL.mult)  # (1-vm)*1e-6... (vm-1)*-1e-6
                first = True
                for c in range(3):
                    TT(r1[:, :], i1q(c), sq(8 + c), op=AL.subtract)
                    TT(r1[:, :], r1[:, :], vmask[:, :], op=AL.mult)
                    TT(r1[:, :], r1[:, :], one_m[:, :], op=AL.add)
                    nc.scalar.activation(w1[:, :], r1[:, :], ACT.Abs)
                    TS(w1[:, :], w1[:, :], HUBER_DELTA, AL.max)
                    nc.vector.reciprocal_approx_fast(w1[:, :], w1[:, :])
                    TS(w1[:, :], w1[:, :], HUBER_DELTA, AL.mult)
                    TT(m1[:, :], sq(0 + c), w1[:, :], op=AL.mult)
                    TT(m2[:, :], m1[:, :], sq(0 + c), op=AL.mult)
                    if first:
                        nc.vector.tensor_copy(SAA[:, :], m2[:, :])
                    else:
                        TT(SAA[:, :], SAA[:, :], m2[:, :], op=AL.add)
                    TT(m2[:, :], m1[:, :], sq(3 + c), op=AL.mult)
                    if first:
                        nc.vector.tensor_copy(SAB[:, :], m2[:, :])
                    else:
                        TT(SAB[:, :], SAB[:, :], m2[:, :], op=AL.add)
                    TT(m2[:, :], m1[:, :], r1[:, :], op=AL.mult)
                    if first:
                        nc.vector.tensor_copy(rhoA[:, :], m2[:, :])
                    else:
                        TT(rhoA[:, :], rhoA[:, :], m2[:, :], op=AL.add)
                    TT(m1[:, :], sq(3 + c), w1[:, :], op=AL.mult)
                    TT(m2[:, :], m1[:, :], sq(3 + c), op=AL.mult)
                    if first:
                        nc.vector.tensor_copy(SBB[:, :], m2[:, :])
                    else:
                        TT(SBB[:, :], SBB[:, :], m2[:, :], op=AL.add)
                    TT(m2[:, :], m1[:, :], r1[:, :], op=AL.mult)
                    if first:
                        nc.vector.tensor_copy(rhoB[:, :], m2[:, :])
                    else:
                        TT(rhoB[:, :], rhoB[:, :], m2[:, :], op=AL.add)
                    first = False
                TT(rz[:, :], iz[:, :], sq(11), op=AL.subtract)
                TT(rz[:, :], rz[:, :], vmask[:, :], op=AL.mult)
                TT(rz[:, :], rz[:, :], one_m[:, :], op=AL.add)
                nc.scalar.activation(w1[:, :], rz[:, :], ACT.Abs)
                TS2(w1[:, :], w1[:, :], LAMBDA, HUBER_DELTA, AL.mult, AL.max)
                nc.vector.reciprocal_approx_fast(w1[:, :], w1[:, :])
                TS(wz[:, :], w1[:, :], HUBER_DELTA * LAMBDA * LAMBDA, AL.mult)
                TT(m1[:, :], sq(6), wz[:, :], op=AL.mult)
                TT(m2[:, :], m1[:, :], sq(6), op=AL.mult)
                TT(SAA[:, :], SAA[:, :], m2[:, :], op=AL.add)
                TT(m2[:, :], m1[:, :], sq(7), op=AL.mult)
                TT(SAB[:, :], SAB[:, :], m2[:, :], op=AL.add)
                TT(m2[:, :], m1[:, :], rz[:, :], op=AL.mult)
                TT(rhoA[:, :], rhoA[:, :], m2[:, :], op=AL.add)
                TT(m1[:, :], sq(7), wz[:, :], op=AL.mult)
                TT(m2[:, :], m1[:, :], sq(7), op=AL.mult)
                TT(SBB[:, :], SBB[:, :], m2[:, :], op=AL.add)
                TT(m2[:, :], m1[:, :], rz[:, :], op=AL.mult)
                TT(rhoB[:, :], rhoB[:, :], m2[:, :], op=AL.add)
                TT(SAT[:, :], wz[:, :], sq(6), op=AL.mult)
                TT(SBT[:, :], wz[:, :], sq(7), op=AL.mult)
                TT(rhoT[:, :], wz[:, :], rz[:, :], op=AL.mult)

                # ---------- quadratic accumulation ----------
                def abtq(k):
                    sl = abt[:, :]
                    return bass.AP(sl.tensor, sl.offset + k, [list(sl.ap[0]), [18 * A, 8], [18, A]])

                CA = [tm(f"CA{y}") for y in range(6)]
                CB = [tm(f"CB{y}") for y in range(6)]
                CT = [tm(f"CT{y}") for y in range(6)]
                for y in range(6):
                    TT(CA[y][:, :], SAA[:, :], abtq(y), op=AL.mult)
                    TT(m1[:, :], SAB[:, :], abtq(6 + y), op=AL.mult)
                    TT(CA[y][:, :], CA[y][:, :], m1[:, :], op=AL.add)
                    TT(m1[:, :], SAT[:, :], abtq(12 + y), op=AL.mult)
                    TT(CA[y][:, :], CA[y][:, :], m1[:, :], op=AL.add)
                    TT(CB[y][:, :], SAB[:, :], abtq(y), op=AL.mult)
                    TT(m1[:, :], SBB[:, :], abtq(6 + y), op=AL.mult)
                    TT(CB[y][:, :], CB[y][:, :], m1[:, :], op=AL.add)
                    TT(m1[:, :], SBT[:, :], abtq(12 + y), op=AL.mult)
                    TT(CB[y][:, :], CB[y][:, :], m1[:, :], op=AL.add)
                    TT(CT[y][:, :], SAT[:, :], abtq(y), op=AL.mult)
                    TT(m1[:, :], SBT[:, :], abtq(6 + y), op=AL.mult)
                    TT(CT[y][:, :], CT[y][:, :], m1[:, :], op=AL.add)
                    TT(m1[:, :], wz[:, :], abtq(12 + y), op=AL.mult)
                    TT(CT[y][:, :], CT[y][:, :], m1[:, :], op=AL.add)

                ent = tm("ent")
                red = sc.tile([128, 1], fp32, tag="red")
                eidx = 0
                for x_ in range(6):
                    for y_ in range(x_, 6):
                        TT(ent[:, :], abtq(x_), CA[y_][:, :], op=AL.mult)
                        TT(m1[:, :], abtq(6 + x_), CB[y_][:, :], op=AL.mult)
                        TT(ent[:, :], ent[:, :], m1[:, :], op=AL.add)
                        TT(m1[:, :], abtq(12 + x_), CT[y_][:, :], op=AL.mult)
                        TT(ent[:, :], ent[:, :], m1[:, :], op=AL.add)
                        for bb in range(2):
                            nc.vector.reduce_sum(red[:, :], ent[:, bb * 4 * A:(bb + 1) * 4 * A], axis=AX.X)
                            TT(acc[:, bb * 32 + eidx:bb * 32 + eidx + 1],
                               acc[:, bb * 32 + eidx:bb * 32 + eidx + 1], red[:, :], op=AL.add)
                        eidx += 1
                for x_ in range(6):
                    TT(ent[:, :], rhoA[:, :], abtq(x_), op=AL.mult)
                    TT(m1[:, :], rhoB[:, :], abtq(6 + x_), op=AL.mult)
                    TT(ent[:, :], ent[:, :], m1[:, :], op=AL.add)
                    TT(m1[:, :], rhoT[:, :], abtq(12 + x_), op=AL.mult)
                    TT(ent[:, :], ent[:, :], m1[:, :], op=AL.add)
                    for bb in range(2):
                        nc.vector.reduce_sum(red[:, :], ent[:, bb * 4 * A:(bb + 1) * 4 * A], axis=AX.X)
                        TT(acc[:, bb * 32 + 21 + x_:bb * 32 + 22 + x_],
                           acc[:, bb * 32 + 21 + x_:bb * 32 + 22 + x_], red[:, :], op=AL.add)

                if debug and s == 0:
                    dsl = dbg_ext.ap()
                    nc.sync.dma_start(out=bass.AP(dsl.tensor, dsl.offset, [[dsl.ap[0][0], 128], [1, A * 128]]),
                                      in_=samp[:, :])
                    for j, t_ in enumerate((W00, vmask, iz, u0, v0)):
                        nc.sync.dma_start(
                            out=bass.AP(dsl.tensor, dsl.offset + A * 128 + j * 8 * A,
                                        [[dsl.ap[0][0], 128], [1, 8 * A]]),
                            in_=t_[:, :])
                    dwl = dbw_ext.ap()
                    nc.sync.dma_start(out=bass.AP(dwl.tensor, dwl.offset, [[dwl.ap[0][0], 128], [1, M]]),
                                      in_=ktw[:, :])
                    nc.sync.dma_start(out=bass.AP(dwl.tensor, dwl.offset + M, [[dwl.ap[0][0], 128], [1, M]]),
                                      in_=kbw[:, :])

            nc.sync.dma_start(out=out_ext.ap(), in_=acc[:, :])

    nc.finalize()
    return nc


def assemble(sums_row):
    JtWJ = np.zeros((2, 6, 6), np.float32)
    Rhs = np.zeros((2, 6), np.float32)
    for b in range(2):
        eidx = 0
        for x_ in range(6):
            for y_ in range(x_, 6):
                v = sums_row[b * 32 + eidx]
                JtWJ[b, x_, y_] = v
                JtWJ[b, y_, x_] = v
                eidx += 1
        Rhs[b] = sums_row[b * 32 + 21:b * 32 + 27]
    return JtWJ, Rhs


def solve_update(T2, JtWJ, Rhs):
    tr = np.trace(JtWJ, axis1=-2, axis2=-1)
    Hm = JtWJ + (tr * 1e-6)[:, None, None] * np.eye(6, dtype=np.float32)
    xi = np.linalg.solve(Hm.astype(np.float64), Rhs.astype(np.float64)[..., None])[..., 0]
    return (T2 @ se3_exp(-xi)).astype(np.float32)


def kernel(pose_twist, I0, I1, invD0, invD1, intrinsics):
    from concourse.bass_utils import run_bass_kernel_spmd

    nc = _NC_CACHE.get("nc")
    if nc is None:
        nc = build_nc()
        _NC_CACHE["nc"] = nc

    pose_twist = np.asarray(pose_twist, np.float32)
    I0 = np.asarray(I0, np.float32); I1 = np.asarray(I1, np.float32)
    invD0 = np.asarray(invD0, np.float32); invD1 = np.asarray(invD1, np.float32)
    intrinsics = np.asarray(intrinsics, np.float32)

    core_inputs = []
    T_cur = []
    for core in range(8):
        sl = slice(2 * core, 2 * core + 2)
        inp, st = host_precompute(pose_twist[sl], I0[sl], I1[sl], invD0[sl],
                                  invD1[sl], intrinsics[sl])
        core_inputs.append(inp)
        T_cur.append(st["T0"])

    LAST_EXEC_NS.clear()
    LAST_TRACES.clear()
    LAST_WALL.clear()
    import time as _time
    for it in range(N_ITERS):
        in_maps = []
        for core in range(8):
            m = dict(core_inputs[core])
            m.update(host_iter_params(T_cur[core], intrinsics[2 * core:2 * core + 2]))
            in_maps.append(m)
        _t0 = _time.time()
        res = run_bass_kernel_spmd(nc, in_maps, list(range(8)), trace=PROFILE)
        LAST_WALL.append(round(_time.time() - _t0, 4))
        if PROFILE:
            if res.exec_time_ns is not None:
                LAST_EXEC_NS.append(res.exec_time_ns)
            if res.instructions_and_trace is not None:
                LAST_TRACES.append(res.instructions_and_trace[1])
        for core in range(8):
            sums = res.results[core]["sums"].sum(axis=0)
            JtWJ, Rhs = assemble(sums)
            T_cur[core] = solve_update(T_cur[core], JtWJ, Rhs)

    return np.concatenate(T_cur, axis=0).astype(np.float32)



# revision 23
# speedup vs baseline: 3.3453x; 3.3453x over previous
"""Trainium2 Bass kernel for nn_InvDirectImageAlign (inverse-compositional image alignment).

v3: ONE compiled NEFF runs all 5 Gauss-Newton iterations on device
(hardware For_i loop). Per core: 2 batch elements. Device does warp,
bilinear grid_sample (GPSIMD ap_gather from fp16 pair-dup band tables),
the JtWJ/Rhs normal equations via TensorEngine matmuls of a per-pixel
fp16 factor matrix G (JtWJ = sum_c G_c^T G_c), the 6x6 Cholesky solve,
se3_exp (Taylor series - angles are <<1 here) and the pose composition.
Inputs upload once; output is just the final 4x4 poses.

Chunking: (batch, 16-row y-band, 224-col x-half) = 80 chunks/core; the 8
GPSIMD partition-groups each own one chunk per superstep; 10 supersteps.
Two pixel layouts, bridged only by PE transposes of gathered data:
  mod-128:    pixel j of chunk(g,s) at partition j%128, free col (g, j//128)
  wrapped-16: pixel j at partition 16g + j%16, free col j//16   (ap_gather's
              index layout)
"""
import numpy as np

B, C, H, W = 16, 3, 320, 448
HW = H * W
N_ITERS = 5
LAMBDA = 0.01
HUBER_DELTA = 0.1
EPS = 1e-6

BH = 16            # band rows per chunk
CW = 224           # band cols per chunk
N = BH * CW        # 3584 px per chunk
A = N // 128       # 28
M = N // 16        # 224
NS = 10            # supersteps
TR = 67            # table rows (16 + 25 + 26)
TC = 266           # table cols (224 + 20 + 21 + 1)
NELEM = TR * TC    # 17822 pairs
YPAD = 25
XPAD = 20


def skew3(w):
    x, y, z = w[..., 0], w[..., 1], w[..., 2]
    O = np.zeros_like(x)
    return np.stack([np.stack([O, -z, y], -1),
                     np.stack([z, O, -x], -1),
                     np.stack([-y, x, O], -1)], -2)


def se3_exp(xi):
    xi = np.asarray(xi, np.float64)
    v, w = xi[:, :3], xi[:, 3:]
    th2 = np.sum(w * w, -1)[:, None, None]
    th2c = np.maximum(th2, 1e-16)
    th = np.sqrt(th2c)
    small = th2 < 1e-10
    Aa = np.where(small, 1.0 - th2 / 6.0, np.sin(th) / th)
    Bc = np.where(small, 0.5 - th2 / 24.0, (1.0 - np.cos(th)) / th2c)
    Cc = np.where(small, 1.0 / 6.0 - th2 / 120.0, (1.0 - Aa) / th2c)
    K = skew3(w)
    K2 = K @ K
    I = np.eye(3)
    R = I + Aa * K + Bc * K2
    V = I + Bc * K + Cc * K2
    t = np.einsum('bij,bj->bi', V, v)
    T = np.zeros((xi.shape[0], 4, 4))
    T[:, :3, :3] = R
    T[:, :3, 3] = t
    T[:, 3, 3] = 1.0
    return T.astype(np.float32)


def feature_gradient(img):
    p = np.pad(img, ((0, 0), (0, 0), (0, 0), (1, 1)), mode='edge')
    dx = 0.5 * (p[..., 2:] - p[..., :-2])
    p = np.pad(img, ((0, 0), (0, 0), (1, 1), (0, 0)), mode='edge')
    dy = 0.5 * (p[..., 2:, :] - p[..., :-2, :])
    return dx.astype(np.float32), dy.astype(np.float32)


def chunk_of(g, s):
    b = g // 4
    local = (g % 4) * 10 + s
    return b, local // 2, local % 2


def bases_of(yb, xh):
    r0, c0 = yb * BH, xh * CW
    rbase = int(np.clip(r0 - YPAD, 0, H - TR))
    cbase = int(np.clip(c0 - XPAD, 0, W - (TC - 1)))
    return rbase, cbase


def mod128_cols_batch(x):
    """[2,K,H,W] -> [128, NS*8*A*K] vectorized (one core's 2 batches)."""
    K = x.shape[1]
    # chunk (b, yb, xh): local = yb*2+xh; g = b*4 + local//10; s = local%10
    a = x.reshape(2, K, 20, BH, 2, CW)          # b K yb row xh col
    a = a.transpose(0, 2, 4, 1, 3, 5)           # b yb xh K row col
    a = a.reshape(2, 40, K, N)                  # local = yb*2+xh
    a = a.reshape(2, 4, 10, K, A, 128)          # b g4 s K a p
    a = a.transpose(5, 2, 0, 1, 4, 3)           # p s b g4 a K
    return np.ascontiguousarray(a.reshape(128, NS, 8, A, K).reshape(128, -1))


def wrap16_cols_batch(x, K):
    """[2,K,H,W] -> [128, NS*M*K] (partition 16g + j%16, col (j//16)*K + k)."""
    a = x.reshape(2, K, 20, BH, 2, CW)
    a = a.transpose(0, 2, 4, 1, 3, 5).reshape(2, 40, K, N)
    a = a.reshape(2, 4, 10, K, M, 16)           # b g4 s K m plo
    a = a.transpose(0, 1, 5, 2, 4, 3)           # b g4 plo s m K
    # partition = 16*(b*4+g4) + plo
    return np.ascontiguousarray(a.reshape(128, NS, M, K).reshape(128, -1))


def host_precompute_all(pose_twist, I0, I1, invD0, invD1, intr):
    """Vectorized over all B=16; returns per-core input dicts + T0 per core."""
    T0 = se3_exp(pose_twist)
    fx = intr[:, 0][:, None, None]; fy = intr[:, 1][:, None, None]
    cx = intr[:, 2][:, None, None]; cy = intr[:, 3][:, None, None]
    uu = np.arange(W, dtype=np.float32)[None, None, :]
    vv = np.arange(H, dtype=np.float32)[None, :, None]
    iD = np.maximum(invD1[:, 0], EPS).astype(np.float32)
    z1 = (1.0 / iD).astype(np.float32)
    x1 = ((uu - cx) / fx * z1).astype(np.float32)
    y1 = ((vv - cy) / fy * z1).astype(np.float32)
    R0, t0 = T0[:, :3, :3], T0[:, :3, 3]
    X0 = np.einsum('bij,bhwj->bhwi', R0, np.stack([x1, y1, z1], -1)) + t0[:, None, None, :]
    X0 = X0.astype(np.float32)
    z0 = X0[..., 2]
    z0s = np.where(np.abs(z0) > EPS, z0, EPS).astype(np.float32)
    iz = (1.0 / z0s).astype(np.float32)
    xh_, yh_ = X0[..., 0], X0[..., 1]
    # A6 = -Jw[...,0,:], B6 = -Jw[...,1,:], T6 = Jt[...,2,:]
    # Jp rows: [fx*iz, 0, -fx*x*iz^2], [0, fy*iz, -fy*y*iz^2]; Jt = [I3 | -skew(X0)]
    O = np.zeros_like(z0)
    Jp = np.stack([np.stack([fx * iz + O, O, -fx * xh_ * iz * iz], -1),
                   np.stack([O, fy * iz + O, -fy * yh_ * iz * iz], -1)], -2).astype(np.float32)
    I3 = np.broadcast_to(np.eye(3, dtype=np.float32), X0.shape[:3] + (3, 3))
    Jt = np.concatenate([I3, -skew3(X0)], -1).astype(np.float32)
    Jw = np.einsum('bhwij,bhwjk->bhwik', Jp, Jt).astype(np.float32)
    A6 = (-Jw[..., 0, :]).astype(np.float32)
    B6 = (-Jw[..., 1, :]).astype(np.float32)
    T6 = Jt[..., 2, :].astype(np.float32)

    dI0x, dI0y = feature_gradient(I0)
    dD0x, dD0y = feature_gradient(invD0)
    planes12 = np.concatenate([dI0x, dI0y, dD0x, dD0y, I0, invD0], axis=1).astype(np.float16)
    flat = planes12.reshape(B, 12, HW)
    pd = np.zeros((B, 12, HW + 1, 2), np.float16)
    pd[:, :, 1:, 0] = flat
    pd[:, :, :HW, 1] = flat

    ABT = np.concatenate([np.moveaxis(A6, -1, 1), np.moveaxis(B6, -1, 1),
                          np.moveaxis(T6, -1, 1)], axis=1)  # [B, 18, H, W] k = j*6+x
    X1 = np.stack([x1, y1, z1], 1)                          # [B, 3, H, W]
    I1f = np.asarray(I1, np.float32)

    bw = np.zeros((128, NS, 4), np.float32)
    for g in range(8):
        for s in range(NS):
            _, yb, xh2 = chunk_of(g, s)
            rbase, cbase = bases_of(yb, xh2)
            bw[16 * g:16 * g + 16, s, 0] = rbase
            bw[16 * g:16 * g + 16, s, 1] = cbase - 1          # xf min
            bw[16 * g:16 * g + 16, s, 2] = cbase + (TC - 2)   # xf max
            bw[16 * g:16 * g + 16, s, 3] = 1 - cbase          # kx offset
    bw = np.ascontiguousarray(bw.reshape(128, NS * 4))
    idn = np.eye(128, dtype=np.float16)

    core_inputs, T0s = [], []
    for core in range(8):
        sl = slice(2 * core, 2 * core + 2)
        inp = {}
        inp["pd"] = np.ascontiguousarray(pd[sl].reshape(2, 12, (HW + 1) * 2))
        inp["abtm"] = mod128_cols_batch(ABT[sl].astype(np.float16))
        inp["x1m"] = mod128_cols_batch(X1[sl])
        inp["x1w"] = wrap16_cols_batch(X1[sl], 3)
        inp["i1m"] = mod128_cols_batch(I1f[sl].astype(np.float16))
        inp["bw"] = bw
        inp["idn"] = idn
        q = np.zeros((2, 16), np.float32)
        q[:, :9] = T0[sl, :3, :3].reshape(2, 9)
        q[:, 9:12] = T0[sl, :3, 3]
        q[:, 12:16] = intr[sl]
        rtm = np.zeros((128, 16, 8), np.float32)
        rtw = np.zeros((128, 16), np.float32)
        for g in range(8):
            bb = g // 4
            rtm[:, :, g] = q[bb][None, :]
            rtw[16 * g:16 * g + 16, :] = q[bb][None, :]
        inp["rtm"] = np.ascontiguousarray(rtm.reshape(128, 16 * 8))
        inp["rtw"] = rtw
        inp["t0q"] = np.ascontiguousarray(T0[sl].reshape(2, 16).astype(np.float32))
        inp["intr2"] = np.ascontiguousarray(intr[sl].astype(np.float32))
        core_inputs.append(inp)
        T0s.append(T0[sl])
    return core_inputs, T0s


_NC_CACHE = {}
PROFILE = False
LAST_EXEC_NS = []
LAST_TRACES = []
LAST_WALL = []


def build_nc():
    import concourse.bacc as bacc
    import concourse.bass as bass
    import concourse.tile as tile
    from concourse import mybir

    fp32 = mybir.dt.float32
    fp16 = mybir.dt.float16
    i16 = mybir.dt.int16
    i32 = mybir.dt.int32
    AL = mybir.AluOpType
    ACT = mybir.ActivationFunctionType
    AX = mybir.AxisListType

    nc = bacc.Bacc("TRN2", target_bir_lowering=False, debug=False, num_devices=8)

    pd_in = nc.dram_tensor("pd", [2, 12, (HW + 1) * 2], fp16, kind="ExternalInput")
    abtm_in = nc.dram_tensor("abtm", [128, NS * 8 * A * 18], fp16, kind="ExternalInput")
    x1m_in = nc.dram_tensor("x1m", [128, NS * 8 * A * 3], fp32, kind="ExternalInput")
    x1w_in = nc.dram_tensor("x1w", [128, NS * M * 3], fp32, kind="ExternalInput")
    i1m_in = nc.dram_tensor("i1m", [128, NS * 8 * A * 3], fp16, kind="ExternalInput")
    bw_in = nc.dram_tensor("bw", [128, NS * 4], fp32, kind="ExternalInput")
    idn_in = nc.dram_tensor("idn", [128, 128], fp16, kind="ExternalInput")
    rtm_in = nc.dram_tensor("rtm", [128, 16 * 8], fp32, kind="ExternalInput")
    rtw_in = nc.dram_tensor("rtw", [128, 16], fp32, kind="ExternalInput")
    t0_in = nc.dram_tensor("t0q", [2, 16], fp32, kind="ExternalInput")
    intr_in = nc.dram_tensor("intr2", [2, 4], fp32, kind="ExternalInput")
    tout_ext = nc.dram_tensor("tout", [2, 16], fp32, kind="ExternalOutput")
    qscr = nc.dram_tensor("qscr", [2, 16], fp32, kind="Internal")

    with tile.TileContext(nc) as tc:
        with tc.tile_pool(name="cst", bufs=1) as cpool, \
             tc.tile_pool(name="tblp", bufs=1) as tpool, \
             tc.tile_pool(name="strm", bufs=2) as sp, \
             tc.tile_pool(name="scr", bufs=1) as sc, \
             tc.tile_pool(name="gath", bufs=1) as gp, \
             tc.tile_pool(name="ps", bufs=2, space="PSUM") as pp, \
             tc.tile_pool(name="jp", bufs=1, space="PSUM") as jp:

            rtm = cpool.tile([128, 16 * 8], fp32, tag="rtm")
            rtw = cpool.tile([128, 16], fp32, tag="rtw")
            bwc = cpool.tile([128, NS * 4], fp32, tag="bw")
            idn = cpool.tile([128, 128], fp16, tag="idn")
            Tq = cpool.tile([2, 16], fp32, tag="Tq")
            intr = cpool.tile([2, 4], fp32, tag="intr")
            nc.sync.dma_start(out=rtm[:, :], in_=rtm_in.ap())
            nc.sync.dma_start(out=rtw[:, :], in_=rtw_in.ap())
            nc.sync.dma_start(out=bwc[:, :], in_=bw_in.ap())
            nc.sync.dma_start(out=idn[:, :], in_=idn_in.ap())
            nc.sync.dma_start(out=Tq[:, :], in_=t0_in.ap())
            nc.sync.dma_start(out=intr[:, :], in_=intr_in.ap())

            psJ = [jp.tile([28, 28], fp32, name=f"psJ{b}", tag=f"psJ{b}") for b in range(2)]

            tbl0 = tpool.tile([128, NELEM * 2], fp16, tag="tbl")
            nc.vector.memset(tbl0[:, :], 0.0)

            def rq(qi):   # mod-128 per-group broadcast: dims (g x8, a x A step0)
                sl = rtm[:, qi * 8:(qi + 1) * 8]
                return bass.AP(sl.tensor, sl.offset, [list(sl.ap[0]), [1, 8], [0, A]])

            def rqw(qi):  # wrapped per-partition scalar bcast over M
                sl = rtw[:, qi:qi + 1]
                return bass.AP(sl.tensor, sl.offset, [list(sl.ap[0]), [0, M]])

            def rtwS(qi):  # wrapped per-partition scalar [128,1]
                return rtw[:, qi:qi + 1]

            def bwq(s, j):
                sl = bwc[:, s * 4 + j:s * 4 + j + 1]
                return bass.AP(sl.tensor, sl.offset, [list(sl.ap[0]), [0, M]])

            def bwS(s, j):
                return bwc[:, s * 4 + j:s * 4 + j + 1]

            TT = nc.vector.tensor_tensor
            TS = lambda out, in0, s1, op: nc.vector.tensor_scalar(out, in0, s1, None, op)
            TS2 = lambda out, in0, s1, s2, op0, op1: nc.vector.tensor_scalar(out, in0, s1, s2, op0, op1)
            STT = nc.vector.scalar_tensor_tensor

            with tc.For_i(0, N_ITERS) as _it:
                for s in range(NS):
                    tbl = tbl0
                    for g in range(8):
                        b, yb, xh = chunk_of(g, s)
                        rbase, cbase = bases_of(yb, xh)
                        start = (rbase * W + cbase) * 2
                        src0 = pd_in.ap()
                        src = bass.AP(src0.tensor,
                                      src0.offset + b * 12 * (HW + 1) * 2 + start,
                                      [[(HW + 1) * 2, 12], [W * 2, TR], [1, TC * 2]])
                        dsl = tbl[16 * g:16 * g + 12, :]
                        dst = bass.AP(dsl.tensor, dsl.offset,
                                      [[dsl.ap[0][0], 12], [TC * 2, TR], [1, TC * 2]])
                        nc.sync.dma_start(out=dst, in_=src)

                    x1w = sp.tile([128, M * 3], fp32, tag="x1w")
                    nc.sync.dma_start(out=x1w[:, :], in_=x1w_in.ap()[:, s * M * 3:(s + 1) * M * 3])
                    x1m = sp.tile([128, 8 * A * 3], fp32, tag="x1m")
                    nc.sync.dma_start(out=x1m[:, :], in_=x1m_in.ap()[:, s * 8 * A * 3:(s + 1) * 8 * A * 3])
                    abt = sp.tile([128, 8 * A * 18], fp16, tag="abt")
                    nc.sync.dma_start(out=abt[:, :], in_=abtm_in.ap()[:, s * 8 * A * 18:(s + 1) * 8 * A * 18])
                    i1 = sp.tile([128, 8 * A * 3], fp16, tag="i1")
                    nc.sync.dma_start(out=i1[:, :], in_=i1m_in.ap()[:, s * 8 * A * 3:(s + 1) * 8 * A * 3])

                    # ---------- wrapped-16 idx pipeline ----------
                    def xw(k):
                        sl = x1w[:, :]
                        return bass.AP(sl.tensor, sl.offset + k, [list(sl.ap[0]), [3, M]])

                    def tw(name):
                        return sc.tile([128, M], fp32, name="w_" + name + f"_{s}", tag="w_" + name)

                    t1w = tw("t1")
                    X0zw = tw("X0z")
                    STT(X0zw[:, :], xw(0), rtwS(6), rqw(11), AL.mult, AL.add)
                    STT(X0zw[:, :], xw(1), rtwS(7), X0zw[:, :], AL.mult, AL.add)
                    STT(X0zw[:, :], xw(2), rtwS(8), X0zw[:, :], AL.mult, AL.add)
                    X0xw = tw("X0x")
                    STT(X0xw[:, :], xw(0), rtwS(0), rqw(9), AL.mult, AL.add)
                    STT(X0xw[:, :], xw(1), rtwS(1), X0xw[:, :], AL.mult, AL.add)
                    STT(X0xw[:, :], xw(2), rtwS(2), X0xw[:, :], AL.mult, AL.add)
                    X0yw = tw("X0y")
                    STT(X0yw[:, :], xw(0), rtwS(3), rqw(10), AL.mult, AL.add)
                    STT(X0yw[:, :], xw(1), rtwS(4), X0yw[:, :], AL.mult, AL.add)
                    STT(X0yw[:, :], xw(2), rtwS(5), X0yw[:, :], AL.mult, AL.add)

                    izw = tw("iz")
                    TS(t1w[:, :], X0zw[:, :], EPS, AL.max)
                    nc.vector.reciprocal_approx_fast(izw[:, :], t1w[:, :])
                    u0w = tw("u0"); v0w = tw("v0")
                    TT(u0w[:, :], X0xw[:, :], izw[:, :], op=AL.mult)
                    STT(u0w[:, :], u0w[:, :], rtwS(12), rqw(14), AL.mult, AL.add)
                    TT(v0w[:, :], X0yw[:, :], izw[:, :], op=AL.mult)
                    STT(v0w[:, :], v0w[:, :], rtwS(13), rqw(15), AL.mult, AL.add)
                    TS2(u0w[:, :], u0w[:, :], -0.5 * (W - 1), 1.5 * (W - 1), AL.max, AL.min)
                    TS2(v0w[:, :], v0w[:, :], -0.5 * (H - 1), 1.5 * (H - 1), AL.max, AL.min)
                    x0fw = tw("x0f"); y0fw = tw("y0f")
                    fi32w = sc.tile([128, M], i32, name=f"fi32w_{s}", tag="fi32w")
                    TS(t1w[:, :], u0w[:, :], 0.5, AL.subtract)
                    nc.vector.tensor_copy(fi32w[:, :], t1w[:, :])
                    nc.vector.tensor_copy(x0fw[:, :], fi32w[:, :])
                    TS(t1w[:, :], v0w[:, :], 0.5, AL.subtract)
                    nc.vector.tensor_copy(fi32w[:, :], t1w[:, :])
                    nc.vector.tensor_copy(y0fw[:, :], fi32w[:, :])
                    xfw = tw("xf"); kxw = tw("kx"); yrw = tw("yr")
                    ktw = tw("kt"); kbw = tw("kb")
                    STT(xfw[:, :], x0fw[:, :], bwS(s, 1), bwq(s, 2), AL.max, AL.min)
                    nc.vector.tensor_scalar(kxw[:, :], xfw[:, :], bwS(s, 3), None, AL.add)
                    nc.vector.tensor_scalar(yrw[:, :], y0fw[:, :], bwS(s, 0), 0.0, AL.subtract, AL.max)
                    TS2(ktw[:, :], yrw[:, :], float(TR - 1), float(TC), AL.min, AL.mult)
                    TT(ktw[:, :], ktw[:, :], kxw[:, :], op=AL.add)
                    TS2(kbw[:, :], yrw[:, :], 1.0, float(TR - 1), AL.add, AL.min)
                    TS(kbw[:, :], kbw[:, :], float(TC), AL.mult)
                    TT(kbw[:, :], kbw[:, :], kxw[:, :], op=AL.add)
                    kidx = sc.tile([128, 2 * M], i16, name=f"kidx_{s}", tag="kidx")
                    nc.vector.tensor_copy(kidx[:, :M], ktw[:, :])
                    nc.vector.tensor_copy(kidx[:, M:], kbw[:, :])

                    gt2 = gp.tile([128, 2 * N * 2], fp16, tag="gt2")
                    nc.gpsimd.ap_gather(gt2[:, :], tbl[:, :], kidx[:, :],
                                        channels=128, num_elems=NELEM, d=2, num_idxs=2 * N)

                    # ---------- mod-128 warp pipeline ----------
                    def xm(k):
                        sl = x1m[:, :]
                        return bass.AP(sl.tensor, sl.offset + k, [list(sl.ap[0]), [3, 8 * A]])

                    def tm(name):
                        return sc.tile([128, 8 * A], fp32, name="m_" + name + f"_{s}", tag="m_" + name)

                    def matvec(dst, aps, qis, t1):
                        TT(dst[:, :], aps[0], qis[0], op=AL.mult)
                        TT(t1[:, :], aps[1], qis[1], op=AL.mult)
                        TT(dst[:, :], dst[:, :], t1[:, :], op=AL.add)
                        TT(t1[:, :], aps[2], qis[2], op=AL.mult)
                        TT(dst[:, :], dst[:, :], t1[:, :], op=AL.add)
                        TT(dst[:, :], dst[:, :], qis[3], op=AL.add)

                    m1 = tm("m1"); m2 = tm("m2")
                    X0z = tm("X0z")
                    matvec(X0z, [xm(0), xm(1), xm(2)], [rq(6), rq(7), rq(8), rq(11)], m1)
                    X0x = tm("X0x")
                    matvec(X0x, [xm(0), xm(1), xm(2)], [rq(0), rq(1), rq(2), rq(9)], m1)
                    X0y = tm("X0y")
                    matvec(X0y, [xm(0), xm(1), xm(2)], [rq(3), rq(4), rq(5), rq(10)], m1)
                    iz = tm("iz")
                    TS(m1[:, :], X0z[:, :], EPS, AL.max)
                    nc.vector.reciprocal_approx_fast(iz[:, :], m1[:, :])
                    u0 = tm("u0"); v0 = tm("v0")
                    TT(u0[:, :], X0x[:, :], iz[:, :], op=AL.mult)
                    TT(u0[:, :], u0[:, :], rq(12), op=AL.mult)
                    TT(u0[:, :], u0[:, :], rq(14), op=AL.add)
                    TT(v0[:, :], X0y[:, :], iz[:, :], op=AL.mult)
                    TT(v0[:, :], v0[:, :], rq(13), op=AL.mult)
                    TT(v0[:, :], v0[:, :], rq(15), op=AL.add)
                    vmask = tm("vmask")
                    TS(vmask[:, :], X0z[:, :], EPS, AL.is_gt)
                    STT(vmask[:, :], u0[:, :], 0.0, vmask[:, :], AL.is_gt, AL.mult)
                    STT(vmask[:, :], u0[:, :], float(W - 1), vmask[:, :], AL.is_lt, AL.mult)
                    STT(vmask[:, :], v0[:, :], 0.0, vmask[:, :], AL.is_gt, AL.mult)
                    STT(vmask[:, :], v0[:, :], float(H - 1), vmask[:, :], AL.is_lt, AL.mult)
                    TS2(u0[:, :], u0[:, :], -0.5 * (W - 1), 1.5 * (W - 1), AL.max, AL.min)
                    TS2(v0[:, :], v0[:, :], -0.5 * (H - 1), 1.5 * (H - 1), AL.max, AL.min)
                    wx = tm("wx"); wy = tm("wy"); x0f = tm("x0f"); y0f = tm("y0f")
                    fi32m = sc.tile([128, 8 * A], i32, name=f"fi32m_{s}", tag="fi32m")
                    TS(m1[:, :], u0[:, :], 0.5, AL.subtract)
                    nc.vector.tensor_copy(fi32m[:, :], m1[:, :])
                    nc.vector.tensor_copy(x0f[:, :], fi32m[:, :])
                    TT(wx[:, :], u0[:, :], x0f[:, :], op=AL.subtract)
                    TS(m1[:, :], v0[:, :], 0.5, AL.subtract)
                    nc.vector.tensor_copy(fi32m[:, :], m1[:, :])
                    nc.vector.tensor_copy(y0f[:, :], fi32m[:, :])
                    TT(wy[:, :], v0[:, :], y0f[:, :], op=AL.subtract)
                    mx0 = tm("mx0"); mx1 = tm("mx1"); my0 = tm("my0"); my1 = tm("my1")
                    TS(mx0[:, :], x0f[:, :], -0.5, AL.is_gt)
                    STT(mx0[:, :], x0f[:, :], float(W - 1) + 0.5, mx0[:, :], AL.is_lt, AL.mult)
                    TS(mx1[:, :], x0f[:, :], -1.5, AL.is_gt)
                    STT(mx1[:, :], x0f[:, :], float(W - 2) + 0.5, mx1[:, :], AL.is_lt, AL.mult)
                    TS(my0[:, :], y0f[:, :], -0.5, AL.is_gt)
                    STT(my0[:, :], y0f[:, :], float(H - 1) + 0.5, my0[:, :], AL.is_lt, AL.mult)
                    TS(my1[:, :], y0f[:, :], -1.5, AL.is_gt)
                    STT(my1[:, :], y0f[:, :], float(H - 2) + 0.5, my1[:, :], AL.is_lt, AL.mult)
                    W00 = tm("W00"); W01 = tm("W01"); W10 = tm("W10"); W11 = tm("W11")
                    TS2(m1[:, :], wx[:, :], 1.0, -1.0, AL.subtract, AL.mult)  # 1-wx
                    TS2(m2[:, :], wy[:, :], 1.0, -1.0, AL.subtract, AL.mult)  # 1-wy
                    TT(W00[:, :], m1[:, :], m2[:, :], op=AL.mult)
                    TT(W00[:, :], W00[:, :], mx0[:, :], op=AL.mult)
                    TT(W00[:, :], W00[:, :], my0[:, :], op=AL.mult)
                    TT(W01[:, :], wx[:, :], m2[:, :], op=AL.mult)
                    TT(W01[:, :], W01[:, :], mx1[:, :], op=AL.mult)
                    TT(W01[:, :], W01[:, :], my0[:, :], op=AL.mult)
                    TT(W10[:, :], m1[:, :], wy[:, :], op=AL.mult)
                    TT(W10[:, :], W10[:, :], mx0[:, :], op=AL.mult)
                    TT(W10[:, :], W10[:, :], my1[:, :], op=AL.mult)
                    TT(W11[:, :], wx[:, :], wy[:, :], op=AL.mult)
                    TT(W11[:, :], W11[:, :], mx1[:, :], op=AL.mult)
                    TT(W11[:, :], W11[:, :], my1[:, :], op=AL.mult)

                    # ---------- PE transpose + combine ----------
                    samp = sc.tile([128, A * 128], fp16, tag="samp")
                    ctmp = sc.tile([128, 512], fp16, tag="ctmp")
                    for a4 in range(A // 4):
                        ptall = pp.tile([128, 2048], fp16, tag="ptall")
                        for ci, base in enumerate((0, 1, 2 * N, 2 * N + 1)):
                            for aa in range(4):
                                a = a4 * 4 + aa
                                src = bass.AP(gt2.tensor, gt2.offset + base + a * 256,
                                              [list(gt2.ap[0]), [2, 128]])
                                nc.tensor.transpose(
                                    ptall[:, ci * 512 + aa * 128:ci * 512 + (aa + 1) * 128],
                                    src, idn[:, :])
                        for ci, wt_ in ((0, W00), (1, W01), (2, W10), (3, W11)):
                            pap = bass.AP(ptall.tensor, ptall.offset + ci * 512,
                                          [list(ptall.ap[0]), [128, 4], [16, 8], [1, 16]])
                            woff = wt_.offset + a4 * 4
                            wap = bass.AP(wt_.tensor, woff, [list(wt_.ap[0]), [1, 4], [A, 8], [0, 16]])
                            dst_off = samp.offset + a4 * 4 * 128
                            dap = bass.AP(samp.tensor, dst_off, [list(samp.ap[0]), [128, 4], [16, 8], [1, 16]])
                            if ci == 0:
                                TT(dap, pap, wap, op=AL.mult)
                            else:
                                tap = bass.AP(ctmp.tensor, ctmp.offset, [list(ctmp.ap[0]), [128, 4], [16, 8], [1, 16]])
                                TT(tap, pap, wap, op=AL.mult)
                                TT(dap, dap, tap, op=AL.add)

                    # ---------- residuals, huber weights, G build ----------
                    def sq(q):
                        sl = samp[:, :]
                        return bass.AP(sl.tensor, sl.offset + q, [list(sl.ap[0]), [16, 8], [128, A]])

                    def i1q(c):
                        sl = i1[:, :]
                        return bass.AP(sl.tensor, sl.offset + c, [list(sl.ap[0]), [3 * A, 8], [3, A]])

                    Gt = sc.tile([128, 28 * 224], fp16, tag="Gt")
                    g6a = sc.tile([128, 6 * 224], fp16, tag="g6a")
                    g6b = sc.tile([128, 6 * 224], fp16, tag="g6b")
                    one_m = tm("one_m")
                    TS2(one_m[:, :], vmask[:, :], 1.0, -1e-6, AL.subtract, AL.mult)  # (1-vm)*1e-6
                    rr = tm("rr"); bb_ = tm("bb"); ss = tm("ss")
                    ppv = tm("ppv"); qqv = tm("qqv")

                    def abt6(k0):  # [x(6) outer, chunk(224) inner], stride 18 per chunk
                        sl = abt[:, :]
                        return bass.AP(sl.tensor, sl.offset + k0, [list(sl.ap[0]), [1, 6], [18, 224]])

                    def gcols(c):  # G cols c*7 .. c*7+5: [x outer, chunk inner]
                        sl = Gt[:, :]
                        return bass.AP(sl.tensor, sl.offset + c * 7 * 224, [list(sl.ap[0]), [224, 6], [1, 224]])

                    def bc6(t):    # broadcast [128,224] over 6 x-cols
                        sl = t[:, :]
                        return bass.AP(sl.tensor, sl.offset, [list(sl.ap[0]), [0, 6], [1, 224]])

                    for c in range(3):
                        TT(rr[:, :], i1q(c), sq(8 + c), op=AL.subtract)
                        TT(rr[:, :], rr[:, :], vmask[:, :], op=AL.mult)
                        TT(rr[:, :], rr[:, :], one_m[:, :], op=AL.add)
                        nc.scalar.activation(bb_[:, :], rr[:, :], ACT.Abs)
                        TS(bb_[:, :], bb_[:, :], HUBER_DELTA, AL.max)
                        nc.vector.reciprocal_approx_fast(bb_[:, :], bb_[:, :])
                        nc.scalar.activation(ss[:, :], bb_[:, :], ACT.Sqrt, scale=HUBER_DELTA)
                        TT(ppv[:, :], ss[:, :], sq(0 + c), op=AL.mult)
                        TT(qqv[:, :], ss[:, :], sq(3 + c), op=AL.mult)
                        TT(g6a[:, :], abt6(0), bc6(ppv), op=AL.mult)
                        TT(g6b[:, :], abt6(6), bc6(qqv), op=AL.mult)
                        TT(gcols(c), g6a[:, :], g6b[:, :], op=AL.add)
                        TT(Gt[:, (c * 7 + 6) * 224:(c * 7 + 7) * 224], ss[:, :], rr[:, :], op=AL.mult)
                    # depth channel
                    TT(rr[:, :], iz[:, :], sq(11), op=AL.subtract)
                    TT(rr[:, :], rr[:, :], vmask[:, :], op=AL.mult)
                    TT(rr[:, :], rr[:, :], one_m[:, :], op=AL.add)
                    nc.scalar.activation(bb_[:, :], rr[:, :], ACT.Abs, scale=LAMBDA)
                    TS(bb_[:, :], bb_[:, :], HUBER_DELTA, AL.max)
                    nc.vector.reciprocal_approx_fast(bb_[:, :], bb_[:, :])
                    nc.scalar.activation(ss[:, :], bb_[:, :], ACT.Sqrt,
                                         scale=HUBER_DELTA * LAMBDA * LAMBDA)
                    TT(ppv[:, :], ss[:, :], sq(6), op=AL.mult)
                    TT(qqv[:, :], ss[:, :], sq(7), op=AL.mult)
                    TT(g6a[:, :], abt6(0), bc6(ppv), op=AL.mult)
                    TT(g6b[:, :], abt6(6), bc6(qqv), op=AL.mult)
                    TT(g6a[:, :], g6a[:, :], g6b[:, :], op=AL.add)
                    TT(g6b[:, :], abt6(12), bc6(ss), op=AL.mult)
                    TT(gcols(3), g6a[:, :], g6b[:, :], op=AL.add)
                    TT(Gt[:, (3 * 7 + 6) * 224:(3 * 7 + 7) * 224], ss[:, :], rr[:, :], op=AL.mult)

                    # ---------- PE: JtWJ accumulation ----------
                    for g in range(8):
                        b = g // 4
                        for a in range(A):
                            off = Gt.offset + g * A + a
                            gap = bass.AP(Gt.tensor, off, [list(Gt.ap[0]), [224, 28]])
                            first = (s == 0 and (g % 4) == 0 and a == 0)
                            last = (s == NS - 1 and (g % 4) == 3 and a == A - 1)
                            nc.tensor.matmul(psJ[b][:, :], gap, gap,
                                             start=first, stop=last,
                                             skip_group_check=True)

                # ---------- per-iteration: extract JtWJ/Rhs, solve, update pose ----------
                S28 = sc.tile([28, 56], fp32, tag="S28")
                for b in range(2):
                    nc.vector.tensor_copy(S28[:, b * 28:(b + 1) * 28], psJ[b][:, :])
                D28 = sc.tile([7, 56], fp32, tag="D28")
                for b in range(2):
                    for c in range(4):
                        src = S28[c * 7:(c + 1) * 7, b * 28 + c * 7:b * 28 + c * 7 + 7]
                        dsl = D28[:, b * 28 + c * 7:b * 28 + c * 7 + 7]
                        nc.sync.dma_start(out=dsl, in_=src)
                M7 = sc.tile([7, 14], fp32, tag="M7")
                for b in range(2):
                    din = bass.AP(D28.tensor, D28.offset + b * 28,
                                  [list(D28.ap[0]), [1, 7], [7, 4]])
                    nc.vector.tensor_reduce(M7[:, b * 7:(b + 1) * 7], din, axis=AX.X, op=AL.add)
                # Mb [2, 49]: row b = M7 block b flattened (x-major)
                Mb = sc.tile([2, 49], fp32, tag="Mb")
                for b in range(2):
                    msrc = bass.AP(M7.tensor, M7.offset + b * 7, [[M7.ap[0][0], 7], [1, 7]])
                    mdsl = Mb[b:b + 1, 0:1]
                    mdst = bass.AP(mdsl.tensor, mdsl.offset, [[Mb.ap[0][0], 1], [7, 7], [1, 7]])
                    nc.sync.dma_start(out=mdst, in_=msrc)
                # tr = sum diag(JtWJ); LM ridge on diag
                trt = sc.tile([2, 1], fp32, tag="trt")
                diag = bass.AP(Mb.tensor, Mb.offset, [list(Mb.ap[0]), [8, 6]])
                nc.vector.tensor_reduce(trt[:, :], diag, axis=AX.X, op=AL.add)
                trb = bass.AP(trt.tensor, trt.offset, [list(trt.ap[0]), [0, 6]])
                STT(diag, trb, 1e-6, diag, AL.mult, AL.add)

                # Cholesky LL^T = Hm (6x6, both batches in 2 partitions)
                Lt = sc.tile([2, 36], fp32, tag="Lt")
                lsrc = bass.AP(Mb.tensor, Mb.offset, [list(Mb.ap[0]), [7, 6], [1, 6]])
                nc.vector.tensor_copy(Lt[:, :], lsrc)
                rhs = sc.tile([2, 6], fp32, tag="rhs")
                rsrc = bass.AP(Mb.tensor, Mb.offset + 6, [list(Mb.ap[0]), [7, 6]])
                nc.vector.tensor_copy(rhs[:, :], rsrc)
                idg = sc.tile([2, 6], fp32, tag="idg")
                tmpj = sc.tile([2, 36], fp32, tag="tmpj")
                red = sc.tile([2, 6], fp32, tag="redj")
                for j in range(6):
                    jj = Lt[:, 6 * j + j:6 * j + j + 1]
                    if j > 0:
                        ljk = Lt[:, 6 * j:6 * j + j]
                        TT(tmpj[:, :j], ljk, ljk, op=AL.mult)
                        nc.vector.tensor_reduce(red[:, 0:1], tmpj[:, :j], axis=AX.X, op=AL.add)
                        TT(jj, jj, red[:, 0:1], op=AL.subtract)
                    nc.scalar.activation(jj, jj, ACT.Sqrt)
                    nc.vector.reciprocal(idg[:, j:j + 1], jj)
                    nr = 5 - j
                    if nr > 0:
                        colap = bass.AP(Lt.tensor, Lt.offset + 6 * (j + 1) + j, [list(Lt.ap[0]), [6, nr]])
                        if j > 0:
                            lik = bass.AP(Lt.tensor, Lt.offset + 6 * (j + 1), [list(Lt.ap[0]), [6, nr], [1, j]])
                            ljkb = bass.AP(Lt.tensor, Lt.offset + 6 * j, [list(Lt.ap[0]), [0, nr], [1, j]])
                            TT(tmpj[:, :nr * j], lik, ljkb, op=AL.mult)
                            tin = bass.AP(tmpj.tensor, tmpj.offset, [list(tmpj.ap[0]), [j, nr], [1, j]])
                            nc.vector.tensor_reduce(red[:, :nr], tin, axis=AX.X, op=AL.add)
                            TT(colap, colap, red[:, :nr], op=AL.subtract)
                        nc.vector.tensor_scalar(colap, colap, idg[:, j:j + 1], None, AL.mult)
                # forward substitution: L y = rhs (in place on rhs)
                for j in range(6):
                    yj = rhs[:, j:j + 1]
                    if j > 0:
                        ljk = Lt[:, 6 * j:6 * j + j]
                        TT(tmpj[:, :j], ljk, rhs[:, :j], op=AL.mult)
                        nc.vector.tensor_reduce(red[:, 0:1], tmpj[:, :j], axis=AX.X, op=AL.add)
                        TT(yj, yj, red[:, 0:1], op=AL.subtract)
                    nc.vector.tensor_scalar(yj, yj, idg[:, j:j + 1], None, AL.mult)
                # back substitution: L^T x = y -> xi = -x stored in xi tile
                for j in range(5, -1, -1):
                    xj = rhs[:, j:j + 1]
                    nk = 5 - j
                    if nk > 0:
                        lkj = bass.AP(Lt.tensor, Lt.offset + 6 * (j + 1) + j, [list(Lt.ap[0]), [6, nk]])
                        TT(tmpj[:, :nk], lkj, rhs[:, j + 1:6], op=AL.mult)
                        nc.vector.tensor_reduce(red[:, 0:1], tmpj[:, :nk], axis=AX.X, op=AL.add)
                        TT(xj, xj, red[:, 0:1], op=AL.subtract)
                    nc.vector.tensor_scalar(xj, xj, idg[:, j:j + 1], None, AL.mult)
                xi = sc.tile([2, 6], fp32, tag="xi")
                TS(xi[:, :], rhs[:, :], -1.0, AL.mult)

                # se3_exp(xi) via Taylor series (|w| << 1 in this regime)
                w3 = xi[:, 3:6]
                wsq = sc.tile([2, 3], fp32, tag="wsq")
                TT(wsq[:, :], w3, w3, op=AL.mult)
                th2 = sc.tile([2, 1], fp32, tag="th2")
                nc.vector.tensor_reduce(th2[:, :], wsq[:, :], axis=AX.X, op=AL.add)
                coA = sc.tile([2, 1], fp32, tag="coA")
                coB = sc.tile([2, 1], fp32, tag="coB")
                coC = sc.tile([2, 1], fp32, tag="coC")
                hh = sc.tile([2, 1], fp32, tag="hh")
                TS2(hh[:, :], th2[:, :], 1.0 / 120.0, -1.0 / 6.0, AL.mult, AL.add)
                nc.vector.tensor_scalar(coA[:, :], th2[:, :], hh[:, :], 1.0, AL.mult, AL.add)
                TS2(hh[:, :], th2[:, :], 1.0 / 720.0, -1.0 / 24.0, AL.mult, AL.add)
                nc.vector.tensor_scalar(coB[:, :], th2[:, :], hh[:, :], 0.5, AL.mult, AL.add)
                TS2(hh[:, :], th2[:, :], 1.0 / 5040.0, -1.0 / 120.0, AL.mult, AL.add)
                nc.vector.tensor_scalar(coC[:, :], th2[:, :], hh[:, :], 1.0 / 6.0, AL.mult, AL.add)
                # K, K2
                Kt = sc.tile([2, 9], fp32, tag="Kt")
                nc.vector.memset(Kt[:, :], 0.0)
                TS(Kt[:, 1:2], xi[:, 5:6], -1.0, AL.mult)   # -z
                nc.vector.tensor_copy(Kt[:, 2:3], xi[:, 4:5])  # y
                nc.vector.tensor_copy(Kt[:, 3:4], xi[:, 5:6])  # z
                TS(Kt[:, 5:6], xi[:, 3:4], -1.0, AL.mult)   # -x
                TS(Kt[:, 6:7], xi[:, 4:5], -1.0, AL.mult)   # -y
                nc.vector.tensor_copy(Kt[:, 7:8], xi[:, 3:4])  # x
                K2t = sc.tile([2, 9], fp32, tag="K2t")
                wiap = bass.AP(xi.tensor, xi.offset + 3, [list(xi.ap[0]), [1, 3], [0, 3]])
                wjap = bass.AP(xi.tensor, xi.offset + 3, [list(xi.ap[0]), [0, 3], [1, 3]])
                TT(K2t[:, :], wiap, wjap, op=AL.mult)
                k2diag = bass.AP(K2t.tensor, K2t.offset, [list(K2t.ap[0]), [4, 3]])
                nc.vector.tensor_scalar(k2diag, k2diag, th2[:, :], None, AL.subtract)
                Rt = sc.tile([2, 9], fp32, tag="Rt")
                Vt = sc.tile([2, 9], fp32, tag="Vt")
                t9 = sc.tile([2, 9], fp32, tag="t9")
                nc.vector.tensor_scalar(Rt[:, :], Kt[:, :], coA[:, :], None, AL.mult)
                nc.vector.tensor_scalar(t9[:, :], K2t[:, :], coB[:, :], None, AL.mult)
                TT(Rt[:, :], Rt[:, :], t9[:, :], op=AL.add)
                rdiag = bass.AP(Rt.tensor, Rt.offset, [list(Rt.ap[0]), [4, 3]])
                TS(rdiag, rdiag, 1.0, AL.add)
                nc.vector.tensor_scalar(Vt[:, :], Kt[:, :], coB[:, :], None, AL.mult)
                nc.vector.tensor_scalar(t9[:, :], K2t[:, :], coC[:, :], None, AL.mult)
                TT(Vt[:, :], Vt[:, :], t9[:, :], op=AL.add)
                vdiag = bass.AP(Vt.tensor, Vt.offset, [list(Vt.ap[0]), [4, 3]])
                TS(vdiag, vdiag, 1.0, AL.add)
                # t = V @ v
                vbc = bass.AP(xi.tensor, xi.offset, [list(xi.ap[0]), [0, 3], [1, 3]])
                TT(t9[:, :], Vt[:, :], vbc, op=AL.mult)
                tv = sc.tile([2, 3], fp32, tag="tv")
                t9v = bass.AP(t9.tensor, t9.offset, [list(t9.ap[0]), [3, 3], [1, 3]])
                nc.vector.tensor_reduce(tv[:, :], t9v, axis=AX.X, op=AL.add)
                # E = [[R, t],[0,0,0,1]] as [2,16]
                Et = sc.tile([2, 16], fp32, tag="Et")
                nc.vector.memset(Et[:, :], 0.0)
                edst = bass.AP(Et.tensor, Et.offset, [list(Et.ap[0]), [4, 3], [1, 3]])
                esrc = bass.AP(Rt.tensor, Rt.offset, [list(Rt.ap[0]), [3, 3], [1, 3]])
                nc.vector.tensor_copy(edst, esrc)
                edst2 = bass.AP(Et.tensor, Et.offset + 3, [list(Et.ap[0]), [4, 3]])
                nc.vector.tensor_copy(edst2, tv[:, :])
                TS(Et[:, 15:16], Et[:, 15:16], 1.0, AL.add)
                # newT = T @ E
                nT = sc.tile([2, 16], fp32, tag="nT")
                for k in range(4):
                    tcol = bass.AP(Tq.tensor, Tq.offset + k, [list(Tq.ap[0]), [4, 4], [0, 4]])
                    erow = bass.AP(Et.tensor, Et.offset + 4 * k, [list(Et.ap[0]), [0, 4], [1, 4]])
                    if k == 0:
                        TT(nT[:, :], tcol, erow, op=AL.mult)
                    else:
                        TT(tmpj[:, :16], tcol, erow, op=AL.mult)
                        TT(nT[:, :], nT[:, :], tmpj[:, :16], op=AL.add)
                nc.vector.tensor_copy(Tq[:, :], nT[:, :])
                # rebuild q = [R(9) | t(3) | intr(4)] and broadcast to rtm/rtw
                qt = sc.tile([2, 16], fp32, tag="qt")
                qr = bass.AP(Tq.tensor, Tq.offset, [list(Tq.ap[0]), [4, 3], [1, 3]])
                nc.vector.tensor_copy(qt[:, 0:9], qr)
                qtcol = bass.AP(Tq.tensor, Tq.offset + 3, [list(Tq.ap[0]), [4, 3]])
                nc.vector.tensor_copy(qt[:, 9:12], qtcol)
                nc.vector.tensor_copy(qt[:, 12:16], intr[:, :])
                nc.sync.dma_start(out=qscr.ap(), in_=qt[:, :])
                qsap = qscr.ap()
                for b in range(2):
                    qsrc = bass.AP(qsap.tensor, qsap.offset + b * 16, [[0, 64], [1, 16]])
                    nc.sync.dma_start(out=rtw[b * 64:(b + 1) * 64, :], in_=qsrc)
                for g in range(8):
                    b = g // 4
                    qsrc = bass.AP(qsap.tensor, qsap.offset + b * 16, [[0, 128], [1, 16]])
                    rdst = bass.AP(rtm.tensor, rtm.offset + g, [list(rtm.ap[0]), [8, 16]])
                    nc.sync.dma_start(out=rdst, in_=qsrc)

            nc.sync.dma_start(out=tout_ext.ap(), in_=Tq[:, :])

    nc.finalize()
    return nc


def kernel(pose_twist, I0, I1, invD0, invD1, intrinsics):
    from concourse.bass_utils import run_bass_kernel_spmd

    nc = _NC_CACHE.get("nc")
    if nc is None:
        nc = build_nc()
        _NC_CACHE["nc"] = nc

    pose_twist = np.asarray(pose_twist, np.float32)
    I0 = np.asarray(I0, np.float32); I1 = np.asarray(I1, np.float32)
    invD0 = np.asarray(invD0, np.float32); invD1 = np.asarray(invD1, np.float32)
    intrinsics = np.asarray(intrinsics, np.float32)

    import time as _time
    LAST_WALL.clear(); LAST_EXEC_NS.clear(); LAST_TRACES.clear()
    t0 = _time.time()
    in_maps, _ = host_precompute_all(pose_twist, I0, I1, invD0, invD1, intrinsics)
    t1 = _time.time()
    res = run_bass_kernel_spmd(nc, in_maps, list(range(8)), trace=PROFILE)
    t2 = _time.time()
    LAST_WALL.extend([round(t1 - t0, 3), round(t2 - t1, 3)])
    if PROFILE:
        if res.exec_time_ns is not None:
            LAST_EXEC_NS.append(res.exec_time_ns)
        if res.instructions_and_trace is not None:
            LAST_TRACES.append(res.instructions_and_trace[1])

    outs = []
    for core in range(8):
        outs.append(res.results[core]["tout"].reshape(2, 4, 4))
    return np.concatenate(outs, axis=0).astype(np.float32)


# revision 31
# speedup vs baseline: 7.4434x; 2.2251x over previous
"""Trainium2 Bass kernel for nn_InvDirectImageAlign (inverse-compositional image alignment).

v3: ONE compiled NEFF runs all 5 Gauss-Newton iterations on device
(hardware For_i loop). Per core: 2 batch elements. Device does warp,
bilinear grid_sample (GPSIMD ap_gather from fp16 pair-dup band tables),
the JtWJ/Rhs normal equations via TensorEngine matmuls of a per-pixel
fp16 factor matrix G (JtWJ = sum_c G_c^T G_c), the 6x6 Cholesky solve,
se3_exp (Taylor series - angles are <<1 here) and the pose composition.
Inputs upload once; output is just the final 4x4 poses.

Chunking: (batch, 16-row y-band, 224-col x-half) = 80 chunks/core; the 8
GPSIMD partition-groups each own one chunk per superstep; 10 supersteps.
Two pixel layouts, bridged only by PE transposes of gathered data:
  mod-128:    pixel j of chunk(g,s) at partition j%128, free col (g, j//128)
  wrapped-16: pixel j at partition 16g + j%16, free col j//16   (ap_gather's
              index layout)
"""
import numpy as np

B, C, H, W = 16, 3, 320, 448
HW = H * W
N_ITERS = 5
LAMBDA = 0.01
HUBER_DELTA = 0.1
EPS = 1e-6

BH = 16            # band rows per chunk
CW = 224           # band cols per chunk
N = BH * CW        # 3584 px per chunk
A = N // 128       # 28
M = N // 16        # 224
NS = 10            # supersteps
TR = 67            # table rows (16 + 25 + 26)
TC = 266           # table cols (224 + 20 + 21 + 1)
NELEM = TR * TC    # 17822 pairs
YPAD = 25
XPAD = 20


def skew3(w):
    x, y, z = w[..., 0], w[..., 1], w[..., 2]
    O = np.zeros_like(x)
    return np.stack([np.stack([O, -z, y], -1),
                     np.stack([z, O, -x], -1),
                     np.stack([-y, x, O], -1)], -2)


def se3_exp(xi):
    xi = np.asarray(xi, np.float64)
    v, w = xi[:, :3], xi[:, 3:]
    th2 = np.sum(w * w, -1)[:, None, None]
    th2c = np.maximum(th2, 1e-16)
    th = np.sqrt(th2c)
    small = th2 < 1e-10
    Aa = np.where(small, 1.0 - th2 / 6.0, np.sin(th) / th)
    Bc = np.where(small, 0.5 - th2 / 24.0, (1.0 - np.cos(th)) / th2c)
    Cc = np.where(small, 1.0 / 6.0 - th2 / 120.0, (1.0 - Aa) / th2c)
    K = skew3(w)
    K2 = K @ K
    I = np.eye(3)
    R = I + Aa * K + Bc * K2
    V = I + Bc * K + Cc * K2
    t = np.einsum('bij,bj->bi', V, v)
    T = np.zeros((xi.shape[0], 4, 4))
    T[:, :3, :3] = R
    T[:, :3, 3] = t
    T[:, 3, 3] = 1.0
    return T.astype(np.float32)


def feature_gradient(img):
    p = np.pad(img, ((0, 0), (0, 0), (0, 0), (1, 1)), mode='edge')
    dx = 0.5 * (p[..., 2:] - p[..., :-2])
    p = np.pad(img, ((0, 0), (0, 0), (1, 1), (0, 0)), mode='edge')
    dy = 0.5 * (p[..., 2:, :] - p[..., :-2, :])
    return dx.astype(np.float32), dy.astype(np.float32)


def chunk_of(g, s):
    b = g // 4
    local = (g % 4) * 10 + s
    return b, local // 2, local % 2


def bases_of(yb, xh):
    r0, c0 = yb * BH, xh * CW
    rbase = int(np.clip(r0 - YPAD, 0, H - TR))
    cbase = int(np.clip(c0 - XPAD, 0, W - (TC - 1)))
    return rbase, cbase


def mod128_cols_batch(x):
    """[2,K,H,W] -> [128, NS*8*A*K] vectorized (one core's 2 batches)."""
    K = x.shape[1]
    # chunk (b, yb, xh): local = yb*2+xh; g = b*4 + local//10; s = local%10
    a = x.reshape(2, K, 20, BH, 2, CW)          # b K yb row xh col
    a = a.transpose(0, 2, 4, 1, 3, 5)           # b yb xh K row col
    a = a.reshape(2, 40, K, N)                  # local = yb*2+xh
    a = a.reshape(2, 4, 10, K, A, 128)          # b g4 s K a p
    a = a.transpose(5, 2, 0, 1, 4, 3)           # p s b g4 a K
    return np.ascontiguousarray(a.reshape(128, NS, 8, A, K).reshape(128, -1))


def wrap16_cols_batch(x, K):
    """[2,K,H,W] -> [128, NS*M*K] (partition 16g + j%16, col (j//16)*K + k)."""
    a = x.reshape(2, K, 20, BH, 2, CW)
    a = a.transpose(0, 2, 4, 1, 3, 5).reshape(2, 40, K, N)
    a = a.reshape(2, 4, 10, K, M, 16)           # b g4 s K m plo
    a = a.transpose(0, 1, 5, 2, 4, 3)           # b g4 plo s m K
    # partition = 16*(b*4+g4) + plo
    return np.ascontiguousarray(a.reshape(128, NS, M, K).reshape(128, -1))


def host_precompute_all(pose_twist, I0, I1, invD0, invD1, intr):
    """Vectorized over all B=16; returns per-core input dicts + T0 per core."""
    T0 = se3_exp(pose_twist)
    fx = intr[:, 0][:, None, None]; fy = intr[:, 1][:, None, None]
    cx = intr[:, 2][:, None, None]; cy = intr[:, 3][:, None, None]
    uu = np.arange(W, dtype=np.float32)[None, None, :]
    vv = np.arange(H, dtype=np.float32)[None, :, None]
    iD = np.maximum(invD1[:, 0], EPS).astype(np.float32)
    z1 = (1.0 / iD).astype(np.float32)
    xn = ((uu - cx) / fx).astype(np.float32)     # [B,1,W]
    yn = ((vv - cy) / fy).astype(np.float32)     # [B,H,1]
    x1 = xn * z1
    y1 = yn * z1
    R0, t0 = T0[:, :3, :3].astype(np.float32), T0[:, :3, 3].astype(np.float32)
    r = R0.reshape(B, 9)

    def sc(v):
        return v[:, None, None]
    X0x = sc(r[:, 0]) * x1 + sc(r[:, 1]) * y1 + sc(r[:, 2]) * z1 + sc(t0[:, 0])
    X0y = sc(r[:, 3]) * x1 + sc(r[:, 4]) * y1 + sc(r[:, 5]) * z1 + sc(t0[:, 1])
    X0z = sc(r[:, 6]) * x1 + sc(r[:, 7]) * y1 + sc(r[:, 8]) * z1 + sc(t0[:, 2])
    z0s = np.where(np.abs(X0z) > EPS, X0z, EPS)
    iz = 1.0 / z0s
    # closed-form A6/B6/T6 (A6=-Jw[...,0,:], B6=-Jw[...,1,:], T6=Jt[...,2,:])
    fxiz = fx * iz
    fyiz = fy * iz
    ziz = X0z * iz
    A2 = fxiz * (X0x * iz)          # fx x iz^2
    B2 = fyiz * (X0y * iz)          # fy y iz^2
    ABT = np.empty((B, 18, H, W), np.float16)
    ABT[:, 0] = -fxiz
    ABT[:, 1] = 0
    ABT[:, 2] = A2
    ABT[:, 3] = A2 * X0y
    ABT[:, 4] = -(fxiz * ziz + A2 * X0x)
    ABT[:, 5] = fxiz * X0y
    ABT[:, 6] = 0
    ABT[:, 7] = -fyiz
    ABT[:, 8] = B2
    ABT[:, 9] = fyiz * ziz + B2 * X0y
    ABT[:, 10] = -(B2 * X0x)
    ABT[:, 11] = -(fyiz * X0x)
    ABT[:, 12] = 0
    ABT[:, 13] = 0
    ABT[:, 14] = 1
    ABT[:, 15] = X0y
    ABT[:, 16] = -X0x
    ABT[:, 17] = 0

    dI0x, dI0y = feature_gradient(I0)
    dD0x, dD0y = feature_gradient(invD0)
    planes12 = np.concatenate([dI0x, dI0y, dD0x, dD0y, I0, invD0], axis=1).astype(np.float16)
    flat = planes12.reshape(B, 12, HW)
    pd = np.zeros((B, 12, HW + 1, 2), np.float16)
    pd[:, :, 1:, 0] = flat
    pd[:, :, :HW, 1] = flat

    X1 = np.stack([x1, y1, z1], 1).astype(np.float16)       # [B, 3, H, W]
    I1f = np.asarray(I1, np.float32)

    bw = np.zeros((128, NS, 4), np.float32)
    for g in range(8):
        for s in range(NS):
            _, yb, xh2 = chunk_of(g, s)
            rbase, cbase = bases_of(yb, xh2)
            bw[16 * g:16 * g + 16, s, 0] = rbase
            bw[16 * g:16 * g + 16, s, 1] = cbase - 1          # xf min
            bw[16 * g:16 * g + 16, s, 2] = cbase + (TC - 2)   # xf max
            bw[16 * g:16 * g + 16, s, 3] = 1 - cbase          # kx offset
    bw = np.ascontiguousarray(bw.reshape(128, NS * 4))
    idn = np.eye(128, dtype=np.float16)

    I1h = I1f.astype(np.float16)
    core_inputs, T0s = [], []
    for core in range(8):
        sl = slice(2 * core, 2 * core + 2)
        inp = {}
        inp["pd"] = np.ascontiguousarray(pd[sl].reshape(2, 12, (HW + 1) * 2))
        inp["abtm"] = mod128_cols_batch(ABT[sl])
        inp["x1m"] = mod128_cols_batch(X1[sl])
        inp["x1w"] = wrap16_cols_batch(X1[sl], 3)
        inp["i1m"] = mod128_cols_batch(I1h[sl])
        inp["bw"] = bw
        inp["idn"] = idn
        q = np.zeros((2, 16), np.float32)
        q[:, :9] = T0[sl, :3, :3].reshape(2, 9)
        q[:, 9:12] = T0[sl, :3, 3]
        q[:, 12:16] = intr[sl]
        rtm = np.zeros((128, 16, 8), np.float32)
        rtw = np.zeros((128, 16), np.float32)
        for g in range(8):
            bb = g // 4
            rtm[:, :, g] = q[bb][None, :]
            rtw[16 * g:16 * g + 16, :] = q[bb][None, :]
        inp["rtm"] = np.ascontiguousarray(rtm.reshape(128, 16 * 8))
        inp["rtw"] = rtw
        inp["t0q"] = np.ascontiguousarray(T0[sl].reshape(2, 16).astype(np.float32))
        inp["intr2"] = np.ascontiguousarray(intr[sl].astype(np.float32))
        core_inputs.append(inp)
        T0s.append(T0[sl])
    return core_inputs, T0s


_NC_CACHE = {}
PROFILE = False
LAST_EXEC_NS = []
LAST_TRACES = []
LAST_WALL = []


def build_nc():
    import concourse.bacc as bacc
    import concourse.bass as bass
    import concourse.tile as tile
    from concourse import mybir

    fp32 = mybir.dt.float32
    fp16 = mybir.dt.float16
    i16 = mybir.dt.int16
    i32 = mybir.dt.int32
    AL = mybir.AluOpType
    ACT = mybir.ActivationFunctionType
    AX = mybir.AxisListType

    nc = bacc.Bacc("TRN2", target_bir_lowering=False, debug=False, num_devices=8)

    pd_in = nc.dram_tensor("pd", [2, 12, (HW + 1) * 2], fp16, kind="ExternalInput")
    abtm_in = nc.dram_tensor("abtm", [128, NS * 8 * A * 18], fp16, kind="ExternalInput")
    x1m_in = nc.dram_tensor("x1m", [128, NS * 8 * A * 3], fp16, kind="ExternalInput")
    x1w_in = nc.dram_tensor("x1w", [128, NS * M * 3], fp16, kind="ExternalInput")
    i1m_in = nc.dram_tensor("i1m", [128, NS * 8 * A * 3], fp16, kind="ExternalInput")
    bw_in = nc.dram_tensor("bw", [128, NS * 4], fp32, kind="ExternalInput")
    idn_in = nc.dram_tensor("idn", [128, 128], fp16, kind="ExternalInput")
    rtm_in = nc.dram_tensor("rtm", [128, 16 * 8], fp32, kind="ExternalInput")
    rtw_in = nc.dram_tensor("rtw", [128, 16], fp32, kind="ExternalInput")
    t0_in = nc.dram_tensor("t0q", [2, 16], fp32, kind="ExternalInput")
    intr_in = nc.dram_tensor("intr2", [2, 4], fp32, kind="ExternalInput")
    tout_ext = nc.dram_tensor("tout", [2, 16], fp32, kind="ExternalOutput")
    qscr = nc.dram_tensor("qscr", [2, 16], fp32, kind="Internal")

    with tile.TileContext(nc) as tc:
        with tc.tile_pool(name="cst", bufs=1) as cpool, \
             tc.tile_pool(name="tblp", bufs=1) as tpool, \
             tc.tile_pool(name="strm", bufs=2) as sp, \
             tc.tile_pool(name="scr", bufs=1) as sc, \
             tc.tile_pool(name="gath", bufs=1) as gp, \
             tc.tile_pool(name="ps", bufs=2, space="PSUM") as pp, \
             tc.tile_pool(name="jp", bufs=1, space="PSUM") as jp:

            rtm = cpool.tile([128, 16 * 8], fp32, tag="rtm")
            rtw = cpool.tile([128, 16], fp32, tag="rtw")
            bwc = cpool.tile([128, NS * 4], fp32, tag="bw")
            idn = cpool.tile([128, 128], fp16, tag="idn")
            Tq = cpool.tile([2, 16], fp32, tag="Tq")
            intr = cpool.tile([2, 4], fp32, tag="intr")
            nc.sync.dma_start(out=rtm[:, :], in_=rtm_in.ap())
            nc.sync.dma_start(out=rtw[:, :], in_=rtw_in.ap())
            nc.sync.dma_start(out=bwc[:, :], in_=bw_in.ap())
            nc.sync.dma_start(out=idn[:, :], in_=idn_in.ap())
            nc.sync.dma_start(out=Tq[:, :], in_=t0_in.ap())
            nc.sync.dma_start(out=intr[:, :], in_=intr_in.ap())

            psJ = [jp.tile([28, 28], fp32, name=f"psJ{b}", tag=f"psJ{b}") for b in range(2)]

            tbl0 = tpool.tile([128, NELEM * 2], fp16, tag="tbl")
            nc.vector.memset(tbl0[:, :], 0.0)

            def rq(qi):   # mod-128 per-group broadcast: dims (g x8, a x A step0)
                sl = rtm[:, qi * 8:(qi + 1) * 8]
                return bass.AP(sl.tensor, sl.offset, [list(sl.ap[0]), [1, 8], [0, A]])

            def rqw(qi):  # wrapped per-partition scalar bcast over M
                sl = rtw[:, qi:qi + 1]
                return bass.AP(sl.tensor, sl.offset, [list(sl.ap[0]), [0, M]])

            def rtwS(qi):  # wrapped per-partition scalar [128,1]
                return rtw[:, qi:qi + 1]

            def bwq(s, j):
                sl = bwc[:, s * 4 + j:s * 4 + j + 1]
                return bass.AP(sl.tensor, sl.offset, [list(sl.ap[0]), [0, M]])

            def bwS(s, j):
                return bwc[:, s * 4 + j:s * 4 + j + 1]

            TT = nc.vector.tensor_tensor
            TS = lambda out, in0, s1, op: nc.vector.tensor_scalar(out, in0, s1, None, op)
            TS2 = lambda out, in0, s1, s2, op0, op1: nc.vector.tensor_scalar(out, in0, s1, s2, op0, op1)
            STT = nc.vector.scalar_tensor_tensor

            with tc.For_i(0, N_ITERS) as _it:
                for s in range(NS):
                    tbl = tbl0
                    for g in range(8):
                        b, yb, xh = chunk_of(g, s)
                        rbase, cbase = bases_of(yb, xh)
                        start = (rbase * W + cbase) * 2
                        src0 = pd_in.ap()
                        src = bass.AP(src0.tensor,
                                      src0.offset + b * 12 * (HW + 1) * 2 + start,
                                      [[(HW + 1) * 2, 12], [W * 2, TR], [1, TC * 2]])
                        dsl = tbl[16 * g:16 * g + 12, :]
                        dst = bass.AP(dsl.tensor, dsl.offset,
                                      [[dsl.ap[0][0], 12], [TC * 2, TR], [1, TC * 2]])
                        nc.sync.dma_start(out=dst, in_=src)

                    x1w = sp.tile([128, M * 3], fp16, tag="x1w")
                    nc.sync.dma_start(out=x1w[:, :], in_=x1w_in.ap()[:, s * M * 3:(s + 1) * M * 3])
                    x1m = sp.tile([128, 8 * A * 3], fp16, tag="x1m")
                    nc.sync.dma_start(out=x1m[:, :], in_=x1m_in.ap()[:, s * 8 * A * 3:(s + 1) * 8 * A * 3])
                    abt = sp.tile([128, 8 * A * 18], fp16, tag="abt")
                    nc.sync.dma_start(out=abt[:, :], in_=abtm_in.ap()[:, s * 8 * A * 18:(s + 1) * 8 * A * 18])
                    i1 = sp.tile([128, 8 * A * 3], fp16, tag="i1")
                    nc.sync.dma_start(out=i1[:, :], in_=i1m_in.ap()[:, s * 8 * A * 3:(s + 1) * 8 * A * 3])

                    # ---------- wrapped-16 idx pipeline ----------
                    def xw(k):
                        sl = x1w[:, :]
                        return bass.AP(sl.tensor, sl.offset + k, [list(sl.ap[0]), [3, M]])

                    def tw(name):
                        return sc.tile([128, M], fp32, name="w_" + name + f"_{s}", tag="w_" + name)

                    t1w = tw("t1")
                    X0zw = tw("X0z")
                    STT(X0zw[:, :], xw(0), rtwS(6), rqw(11), AL.mult, AL.add)
                    STT(X0zw[:, :], xw(1), rtwS(7), X0zw[:, :], AL.mult, AL.add)
                    STT(X0zw[:, :], xw(2), rtwS(8), X0zw[:, :], AL.mult, AL.add)
                    X0xw = tw("X0x")
                    STT(X0xw[:, :], xw(0), rtwS(0), rqw(9), AL.mult, AL.add)
                    STT(X0xw[:, :], xw(1), rtwS(1), X0xw[:, :], AL.mult, AL.add)
                    STT(X0xw[:, :], xw(2), rtwS(2), X0xw[:, :], AL.mult, AL.add)
                    X0yw = tw("X0y")
                    STT(X0yw[:, :], xw(0), rtwS(3), rqw(10), AL.mult, AL.add)
                    STT(X0yw[:, :], xw(1), rtwS(4), X0yw[:, :], AL.mult, AL.add)
                    STT(X0yw[:, :], xw(2), rtwS(5), X0yw[:, :], AL.mult, AL.add)

                    izw = tw("iz")
                    TS(t1w[:, :], X0zw[:, :], EPS, AL.max)
                    nc.vector.reciprocal_approx_fast(izw[:, :], t1w[:, :])
                    u0w = tw("u0"); v0w = tw("v0")
                    TT(u0w[:, :], X0xw[:, :], izw[:, :], op=AL.mult)
                    STT(u0w[:, :], u0w[:, :], rtwS(12), rqw(14), AL.mult, AL.add)
                    TT(v0w[:, :], X0yw[:, :], izw[:, :], op=AL.mult)
                    STT(v0w[:, :], v0w[:, :], rtwS(13), rqw(15), AL.mult, AL.add)
                    TS2(u0w[:, :], u0w[:, :], -0.5 * (W - 1), 1.5 * (W - 1), AL.max, AL.min)
                    TS2(v0w[:, :], v0w[:, :], -0.5 * (H - 1), 1.5 * (H - 1), AL.max, AL.min)
                    x0fw = tw("x0f"); y0fw = tw("y0f")
                    fi32w = sc.tile([128, M], i32, name=f"fi32w_{s}", tag="fi32w")
                    TS(t1w[:, :], u0w[:, :], 0.5, AL.subtract)
                    nc.vector.tensor_copy(fi32w[:, :], t1w[:, :])
                    nc.vector.tensor_copy(x0fw[:, :], fi32w[:, :])
                    TS(t1w[:, :], v0w[:, :], 0.5, AL.subtract)
                    nc.vector.tensor_copy(fi32w[:, :], t1w[:, :])
                    nc.vector.tensor_copy(y0fw[:, :], fi32w[:, :])
                    xfw = tw("xf"); kxw = tw("kx"); yrw = tw("yr")
                    ktw = tw("kt"); kbw = tw("kb")
                    STT(xfw[:, :], x0fw[:, :], bwS(s, 1), bwq(s, 2), AL.max, AL.min)
                    nc.vector.tensor_scalar(kxw[:, :], xfw[:, :], bwS(s, 3), None, AL.add)
                    nc.vector.tensor_scalar(yrw[:, :], y0fw[:, :], bwS(s, 0), 0.0, AL.subtract, AL.max)
                    TS2(ktw[:, :], yrw[:, :], float(TR - 1), float(TC), AL.min, AL.mult)
                    TT(ktw[:, :], ktw[:, :], kxw[:, :], op=AL.add)
                    TS2(kbw[:, :], yrw[:, :], 1.0, float(TR - 1), AL.add, AL.min)
                    TS(kbw[:, :], kbw[:, :], float(TC), AL.mult)
                    TT(kbw[:, :], kbw[:, :], kxw[:, :], op=AL.add)
                    kidx = sc.tile([128, 2 * M], i16, name=f"kidx_{s}", tag="kidx")
                    nc.vector.tensor_copy(kidx[:, :M], ktw[:, :])
                    nc.vector.tensor_copy(kidx[:, M:], kbw[:, :])

                    gt2 = gp.tile([128, 2 * N * 2], fp16, tag="gt2")
                    nc.gpsimd.ap_gather(gt2[:, :], tbl[:, :], kidx[:, :],
                                        channels=128, num_elems=NELEM, d=2, num_idxs=2 * N)

                    # ---------- mod-128 warp pipeline ----------
                    def xm(k):
                        sl = x1m[:, :]
                        return bass.AP(sl.tensor, sl.offset + k, [list(sl.ap[0]), [3, 8 * A]])

                    def tm(name):
                        return sc.tile([128, 8 * A], fp32, name="m_" + name + f"_{s}", tag="m_" + name)

                    def matvec(dst, aps, qis, t1):
                        TT(dst[:, :], aps[0], qis[0], op=AL.mult)
                        TT(t1[:, :], aps[1], qis[1], op=AL.mult)
                        TT(dst[:, :], dst[:, :], t1[:, :], op=AL.add)
                        TT(t1[:, :], aps[2], qis[2], op=AL.mult)
                        TT(dst[:, :], dst[:, :], t1[:, :], op=AL.add)
                        TT(dst[:, :], dst[:, :], qis[3], op=AL.add)

                    m1 = tm("m1"); m2 = tm("m2")
                    X0z = tm("X0z")
                    matvec(X0z, [xm(0), xm(1), xm(2)], [rq(6), rq(7), rq(8), rq(11)], m1)
                    X0x = tm("X0x")
                    matvec(X0x, [xm(0), xm(1), xm(2)], [rq(0), rq(1), rq(2), rq(9)], m1)
                    X0y = tm("X0y")
                    matvec(X0y, [xm(0), xm(1), xm(2)], [rq(3), rq(4), rq(5), rq(10)], m1)
                    iz = tm("iz")
                    TS(m1[:, :], X0z[:, :], EPS, AL.max)
                    nc.vector.reciprocal_approx_fast(iz[:, :], m1[:, :])
                    u0 = tm("u0"); v0 = tm("v0")
                    TT(u0[:, :], X0x[:, :], iz[:, :], op=AL.mult)
                    TT(u0[:, :], u0[:, :], rq(12), op=AL.mult)
                    TT(u0[:, :], u0[:, :], rq(14), op=AL.add)
                    TT(v0[:, :], X0y[:, :], iz[:, :], op=AL.mult)
                    TT(v0[:, :], v0[:, :], rq(13), op=AL.mult)
                    TT(v0[:, :], v0[:, :], rq(15), op=AL.add)
                    vmask = tm("vmask")
                    TS(vmask[:, :], X0z[:, :], EPS, AL.is_gt)
                    STT(vmask[:, :], u0[:, :], 0.0, vmask[:, :], AL.is_gt, AL.mult)
                    STT(vmask[:, :], u0[:, :], float(W - 1), vmask[:, :], AL.is_lt, AL.mult)
                    STT(vmask[:, :], v0[:, :], 0.0, vmask[:, :], AL.is_gt, AL.mult)
                    STT(vmask[:, :], v0[:, :], float(H - 1), vmask[:, :], AL.is_lt, AL.mult)
                    TS2(u0[:, :], u0[:, :], -0.5 * (W - 1), 1.5 * (W - 1), AL.max, AL.min)
                    TS2(v0[:, :], v0[:, :], -0.5 * (H - 1), 1.5 * (H - 1), AL.max, AL.min)
                    wx = tm("wx"); wy = tm("wy"); x0f = tm("x0f"); y0f = tm("y0f")
                    fi32m = sc.tile([128, 8 * A], i32, name=f"fi32m_{s}", tag="fi32m")
                    TS(m1[:, :], u0[:, :], 0.5, AL.subtract)
                    nc.vector.tensor_copy(fi32m[:, :], m1[:, :])
                    nc.vector.tensor_copy(x0f[:, :], fi32m[:, :])
                    TT(wx[:, :], u0[:, :], x0f[:, :], op=AL.subtract)
                    TS(m1[:, :], v0[:, :], 0.5, AL.subtract)
                    nc.vector.tensor_copy(fi32m[:, :], m1[:, :])
                    nc.vector.tensor_copy(y0f[:, :], fi32m[:, :])
                    TT(wy[:, :], v0[:, :], y0f[:, :], op=AL.subtract)
                    mx0 = tm("mx0"); mx1 = tm("mx1"); my0 = tm("my0"); my1 = tm("my1")
                    TS(mx0[:, :], x0f[:, :], -0.5, AL.is_gt)
                    STT(mx0[:, :], x0f[:, :], float(W - 1) + 0.5, mx0[:, :], AL.is_lt, AL.mult)
                    TS(mx1[:, :], x0f[:, :], -1.5, AL.is_gt)
                    STT(mx1[:, :], x0f[:, :], float(W - 2) + 0.5, mx1[:, :], AL.is_lt, AL.mult)
                    TS(my0[:, :], y0f[:, :], -0.5, AL.is_gt)
                    STT(my0[:, :], y0f[:, :], float(H - 1) + 0.5, my0[:, :], AL.is_lt, AL.mult)
                    TS(my1[:, :], y0f[:, :], -1.5, AL.is_gt)
                    STT(my1[:, :], y0f[:, :], float(H - 2) + 0.5, my1[:, :], AL.is_lt, AL.mult)
                    W00 = tm("W00"); W01 = tm("W01"); W10 = tm("W10"); W11 = tm("W11")
                    TS2(m1[:, :], wx[:, :], 1.0, -1.0, AL.subtract, AL.mult)  # 1-wx
                    TS2(m2[:, :], wy[:, :], 1.0, -1.0, AL.subtract, AL.mult)  # 1-wy
                    TT(W00[:, :], m1[:, :], m2[:, :], op=AL.mult)
                    TT(W00[:, :], W00[:, :], mx0[:, :], op=AL.mult)
                    TT(W00[:, :], W00[:, :], my0[:, :], op=AL.mult)
                    TT(W01[:, :], wx[:, :], m2[:, :], op=AL.mult)
                    TT(W01[:, :], W01[:, :], mx1[:, :], op=AL.mult)
                    TT(W01[:, :], W01[:, :], my0[:, :], op=AL.mult)
                    TT(W10[:, :], m1[:, :], wy[:, :], op=AL.mult)
                    TT(W10[:, :], W10[:, :], mx0[:, :], op=AL.mult)
                    TT(W10[:, :], W10[:, :], my1[:, :], op=AL.mult)
                    TT(W11[:, :], wx[:, :], wy[:, :], op=AL.mult)
                    TT(W11[:, :], W11[:, :], mx1[:, :], op=AL.mult)
                    TT(W11[:, :], W11[:, :], my1[:, :], op=AL.mult)

                    # ---------- PE transpose + combine ----------
                    samp = sc.tile([128, A * 128], fp16, tag="samp")
                    ctmp = sc.tile([128, 512], fp16, tag="ctmp")
                    for a4 in range(A // 4):
                        ptall = pp.tile([128, 2048], fp16, tag="ptall")
                        for ci, base in enumerate((0, 1, 2 * N, 2 * N + 1)):
                            for aa in range(4):
                                a = a4 * 4 + aa
                                src = bass.AP(gt2.tensor, gt2.offset + base + a * 256,
                                              [list(gt2.ap[0]), [2, 128]])
                                nc.tensor.transpose(
                                    ptall[:, ci * 512 + aa * 128:ci * 512 + (aa + 1) * 128],
                                    src, idn[:, :])
                        for ci, wt_ in ((0, W00), (1, W01), (2, W10), (3, W11)):
                            pap = bass.AP(ptall.tensor, ptall.offset + ci * 512,
                                          [list(ptall.ap[0]), [128, 4], [16, 8], [1, 16]])
                            woff = wt_.offset + a4 * 4
                            wap = bass.AP(wt_.tensor, woff, [list(wt_.ap[0]), [1, 4], [A, 8], [0, 16]])
                            dst_off = samp.offset + a4 * 4 * 128
                            dap = bass.AP(samp.tensor, dst_off, [list(samp.ap[0]), [128, 4], [16, 8], [1, 16]])
                            if ci == 0:
                                TT(dap, pap, wap, op=AL.mult)
                            else:
                                tap = bass.AP(ctmp.tensor, ctmp.offset, [list(ctmp.ap[0]), [128, 4], [16, 8], [1, 16]])
                                TT(tap, pap, wap, op=AL.mult)
                                TT(dap, dap, tap, op=AL.add)

                    # ---------- residuals, huber weights, G build ----------
                    def sq(q):
                        sl = samp[:, :]
                        return bass.AP(sl.tensor, sl.offset + q, [list(sl.ap[0]), [16, 8], [128, A]])

                    def i1q(c):
                        sl = i1[:, :]
                        return bass.AP(sl.tensor, sl.offset + c, [list(sl.ap[0]), [3 * A, 8], [3, A]])

                    Gt = sc.tile([128, 28 * 224], fp16, tag="Gt")
                    g6a = sc.tile([128, 6 * 224], fp16, tag="g6a")
                    g6b = sc.tile([128, 6 * 224], fp16, tag="g6b")
                    one_m = tm("one_m")
                    TS2(one_m[:, :], vmask[:, :], 1.0, -1e-6, AL.subtract, AL.mult)  # (1-vm)*1e-6
                    rr = tm("rr"); bb_ = tm("bb"); ss = tm("ss")
                    ppv = tm("ppv"); qqv = tm("qqv")

                    def abt6(k0):  # [x(6) outer, chunk(224) inner], stride 18 per chunk
                        sl = abt[:, :]
                        return bass.AP(sl.tensor, sl.offset + k0, [list(sl.ap[0]), [1, 6], [18, 224]])

                    def gcols(c):  # G cols c*7 .. c*7+5: [x outer, chunk inner]
                        sl = Gt[:, :]
                        return bass.AP(sl.tensor, sl.offset + c * 7 * 224, [list(sl.ap[0]), [224, 6], [1, 224]])

                    def bc6(t):    # broadcast [128,224] over 6 x-cols
                        sl = t[:, :]
                        return bass.AP(sl.tensor, sl.offset, [list(sl.ap[0]), [0, 6], [1, 224]])

                    for c in range(3):
                        TT(rr[:, :], i1q(c), sq(8 + c), op=AL.subtract)
                        TT(rr[:, :], rr[:, :], vmask[:, :], op=AL.mult)
                        TT(rr[:, :], rr[:, :], one_m[:, :], op=AL.add)
                        nc.scalar.activation(bb_[:, :], rr[:, :], ACT.Abs)
                        TS(bb_[:, :], bb_[:, :], HUBER_DELTA, AL.max)
                        nc.vector.reciprocal_approx_fast(bb_[:, :], bb_[:, :])
                        nc.scalar.activation(ss[:, :], bb_[:, :], ACT.Sqrt, scale=HUBER_DELTA)
                        TT(ppv[:, :], ss[:, :], sq(0 + c), op=AL.mult)
                        TT(qqv[:, :], ss[:, :], sq(3 + c), op=AL.mult)
                        TT(g6a[:, :], abt6(0), bc6(ppv), op=AL.mult)
                        TT(g6b[:, :], abt6(6), bc6(qqv), op=AL.mult)
                        TT(gcols(c), g6a[:, :], g6b[:, :], op=AL.add)
                        TT(Gt[:, (c * 7 + 6) * 224:(c * 7 + 7) * 224], ss[:, :], rr[:, :], op=AL.mult)
                    # depth channel
                    TT(rr[:, :], iz[:, :], sq(11), op=AL.subtract)
                    TT(rr[:, :], rr[:, :], vmask[:, :], op=AL.mult)
                    TT(rr[:, :], rr[:, :], one_m[:, :], op=AL.add)
                    nc.scalar.activation(bb_[:, :], rr[:, :], ACT.Abs, scale=LAMBDA)
                    TS(bb_[:, :], bb_[:, :], HUBER_DELTA, AL.max)
                    nc.vector.reciprocal_approx_fast(bb_[:, :], bb_[:, :])
                    nc.scalar.activation(ss[:, :], bb_[:, :], ACT.Sqrt,
                                         scale=HUBER_DELTA * LAMBDA * LAMBDA)
                    TT(ppv[:, :], ss[:, :], sq(6), op=AL.mult)
                    TT(qqv[:, :], ss[:, :], sq(7), op=AL.mult)
                    TT(g6a[:, :], abt6(0), bc6(ppv), op=AL.mult)
                    TT(g6b[:, :], abt6(6), bc6(qqv), op=AL.mult)
                    TT(g6a[:, :], g6a[:, :], g6b[:, :], op=AL.add)
                    TT(g6b[:, :], abt6(12), bc6(ss), op=AL.mult)
                    TT(gcols(3), g6a[:, :], g6b[:, :], op=AL.add)
                    TT(Gt[:, (3 * 7 + 6) * 224:(3 * 7 + 7) * 224], ss[:, :], rr[:, :], op=AL.mult)

                    # ---------- PE: JtWJ accumulation ----------
                    for g in range(8):
                        b = g // 4
                        for a in range(A):
                            off = Gt.offset + g * A + a
                            gap = bass.AP(Gt.tensor, off, [list(Gt.ap[0]), [224, 28]])
                            first = (s == 0 and (g % 4) == 0 and a == 0)
                            last = (s == NS - 1 and (g % 4) == 3 and a == A - 1)
                            nc.tensor.matmul(psJ[b][:, :], gap, gap,
                                             start=first, stop=last,
                                             skip_group_check=True)

                # ---------- per-iteration: extract JtWJ/Rhs, solve, update pose ----------
                S28 = sc.tile([28, 56], fp32, tag="S28")
                for b in range(2):
                    nc.vector.tensor_copy(S28[:, b * 28:(b + 1) * 28], psJ[b][:, :])
                D28 = sc.tile([7, 56], fp32, tag="D28")
                for b in range(2):
                    for c in range(4):
                        src = S28[c * 7:(c + 1) * 7, b * 28 + c * 7:b * 28 + c * 7 + 7]
                        dsl = D28[:, b * 28 + c * 7:b * 28 + c * 7 + 7]
                        nc.sync.dma_start(out=dsl, in_=src)
                M7 = sc.tile([7, 14], fp32, tag="M7")
                for b in range(2):
                    din = bass.AP(D28.tensor, D28.offset + b * 28,
                                  [list(D28.ap[0]), [1, 7], [7, 4]])
                    nc.vector.tensor_reduce(M7[:, b * 7:(b + 1) * 7], din, axis=AX.X, op=AL.add)
                # Mb [2, 49]: row b = M7 block b flattened (x-major)
                Mb = sc.tile([2, 49], fp32, tag="Mb")
                for b in range(2):
                    msrc = bass.AP(M7.tensor, M7.offset + b * 7, [[M7.ap[0][0], 7], [1, 7]])
                    mdsl = Mb[b:b + 1, 0:1]
                    mdst = bass.AP(mdsl.tensor, mdsl.offset, [[Mb.ap[0][0], 1], [7, 7], [1, 7]])
                    nc.sync.dma_start(out=mdst, in_=msrc)
                # tr = sum diag(JtWJ); LM ridge on diag
                trt = sc.tile([2, 1], fp32, tag="trt")
                diag = bass.AP(Mb.tensor, Mb.offset, [list(Mb.ap[0]), [8, 6]])
                nc.vector.tensor_reduce(trt[:, :], diag, axis=AX.X, op=AL.add)
                trb = bass.AP(trt.tensor, trt.offset, [list(trt.ap[0]), [0, 6]])
                STT(diag, trb, 1e-6, diag, AL.mult, AL.add)

                # Cholesky LL^T = Hm (6x6, both batches in 2 partitions)
                Lt = sc.tile([2, 36], fp32, tag="Lt")
                lsrc = bass.AP(Mb.tensor, Mb.offset, [list(Mb.ap[0]), [7, 6], [1, 6]])
                nc.vector.tensor_copy(Lt[:, :], lsrc)
                rhs = sc.tile([2, 6], fp32, tag="rhs")
                rsrc = bass.AP(Mb.tensor, Mb.offset + 6, [list(Mb.ap[0]), [7, 6]])
                nc.vector.tensor_copy(rhs[:, :], rsrc)
                idg = sc.tile([2, 6], fp32, tag="idg")
                tmpj = sc.tile([2, 36], fp32, tag="tmpj")
                red = sc.tile([2, 6], fp32, tag="redj")
                for j in range(6):
                    jj = Lt[:, 6 * j + j:6 * j + j + 1]
                    if j > 0:
                        ljk = Lt[:, 6 * j:6 * j + j]
                        TT(tmpj[:, :j], ljk, ljk, op=AL.mult)
                        nc.vector.tensor_reduce(red[:, 0:1], tmpj[:, :j], axis=AX.X, op=AL.add)
                        TT(jj, jj, red[:, 0:1], op=AL.subtract)
                    nc.scalar.activation(jj, jj, ACT.Sqrt)
                    nc.vector.reciprocal(idg[:, j:j + 1], jj)
                    nr = 5 - j
                    if nr > 0:
                        colap = bass.AP(Lt.tensor, Lt.offset + 6 * (j + 1) + j, [list(Lt.ap[0]), [6, nr]])
                        if j > 0:
                            lik = bass.AP(Lt.tensor, Lt.offset + 6 * (j + 1), [list(Lt.ap[0]), [6, nr], [1, j]])
                            ljkb = bass.AP(Lt.tensor, Lt.offset + 6 * j, [list(Lt.ap[0]), [0, nr], [1, j]])
                            TT(tmpj[:, :nr * j], lik, ljkb, op=AL.mult)
                            tin = bass.AP(tmpj.tensor, tmpj.offset, [list(tmpj.ap[0]), [j, nr], [1, j]])
                            nc.vector.tensor_reduce(red[:, :nr], tin, axis=AX.X, op=AL.add)
                            TT(colap, colap, red[:, :nr], op=AL.subtract)
                        nc.vector.tensor_scalar(colap, colap, idg[:, j:j + 1], None, AL.mult)
                # forward substitution: L y = rhs (in place on rhs)
                for j in range(6):
                    yj = rhs[:, j:j + 1]
                    if j > 0:
                        ljk = Lt[:, 6 * j:6 * j + j]
                        TT(tmpj[:, :j], ljk, rhs[:, :j], op=AL.mult)
                        nc.vector.tensor_reduce(red[:, 0:1], tmpj[:, :j], axis=AX.X, op=AL.add)
                        TT(yj, yj, red[:, 0:1], op=AL.subtract)
                    nc.vector.tensor_scalar(yj, yj, idg[:, j:j + 1], None, AL.mult)
                # back substitution: L^T x = y -> xi = -x stored in xi tile
                for j in range(5, -1, -1):
                    xj = rhs[:, j:j + 1]
                    nk = 5 - j
                    if nk > 0:
                        lkj = bass.AP(Lt.tensor, Lt.offset + 6 * (j + 1) + j, [list(Lt.ap[0]), [6, nk]])
                        TT(tmpj[:, :nk], lkj, rhs[:, j + 1:6], op=AL.mult)
                        nc.vector.tensor_reduce(red[:, 0:1], tmpj[:, :nk], axis=AX.X, op=AL.add)
                        TT(xj, xj, red[:, 0:1], op=AL.subtract)
                    nc.vector.tensor_scalar(xj, xj, idg[:, j:j + 1], None, AL.mult)
                xi = sc.tile([2, 6], fp32, tag="xi")
                TS(xi[:, :], rhs[:, :], -1.0, AL.mult)

                # se3_exp(xi) via Taylor series (|w| << 1 in this regime)
                w3 = xi[:, 3:6]
                wsq = sc.tile([2, 3], fp32, tag="wsq")
                TT(wsq[:, :], w3, w3, op=AL.mult)
                th2 = sc.tile([2, 1], fp32, tag="th2")
                nc.vector.tensor_reduce(th2[:, :], wsq[:, :], axis=AX.X, op=AL.add)
                coA = sc.tile([2, 1], fp32, tag="coA")
                coB = sc.tile([2, 1], fp32, tag="coB")
                coC = sc.tile([2, 1], fp32, tag="coC")
                hh = sc.tile([2, 1], fp32, tag="hh")
                TS2(hh[:, :], th2[:, :], 1.0 / 120.0, -1.0 / 6.0, AL.mult, AL.add)
                nc.vector.tensor_scalar(coA[:, :], th2[:, :], hh[:, :], 1.0, AL.mult, AL.add)
                TS2(hh[:, :], th2[:, :], 1.0 / 720.0, -1.0 / 24.0, AL.mult, AL.add)
                nc.vector.tensor_scalar(coB[:, :], th2[:, :], hh[:, :], 0.5, AL.mult, AL.add)
                TS2(hh[:, :], th2[:, :], 1.0 / 5040.0, -1.0 / 120.0, AL.mult, AL.add)
                nc.vector.tensor_scalar(coC[:, :], th2[:, :], hh[:, :], 1.0 / 6.0, AL.mult, AL.add)
                # K, K2
                Kt = sc.tile([2, 9], fp32, tag="Kt")
                nc.vector.memset(Kt[:, :], 0.0)
                TS(Kt[:, 1:2], xi[:, 5:6], -1.0, AL.mult)   # -z
                nc.vector.tensor_copy(Kt[:, 2:3], xi[:, 4:5])  # y
                nc.vector.tensor_copy(Kt[:, 3:4], xi[:, 5:6])  # z
                TS(Kt[:, 5:6], xi[:, 3:4], -1.0, AL.mult)   # -x
                TS(Kt[:, 6:7], xi[:, 4:5], -1.0, AL.mult)   # -y
                nc.vector.tensor_copy(Kt[:, 7:8], xi[:, 3:4])  # x
                K2t = sc.tile([2, 9], fp32, tag="K2t")
                wiap = bass.AP(xi.tensor, xi.offset + 3, [list(xi.ap[0]), [1, 3], [0, 3]])
                wjap = bass.AP(xi.tensor, xi.offset + 3, [list(xi.ap[0]), [0, 3], [1, 3]])
                TT(K2t[:, :], wiap, wjap, op=AL.mult)
                k2diag = bass.AP(K2t.tensor, K2t.offset, [list(K2t.ap[0]), [4, 3]])
                nc.vector.tensor_scalar(k2diag, k2diag, th2[:, :], None, AL.subtract)
                Rt = sc.tile([2, 9], fp32, tag="Rt")
                Vt = sc.tile([2, 9], fp32, tag="Vt")
                t9 = sc.tile([2, 9], fp32, tag="t9")
                nc.vector.tensor_scalar(Rt[:, :], Kt[:, :], coA[:, :], None, AL.mult)
                nc.vector.tensor_scalar(t9[:, :], K2t[:, :], coB[:, :], None, AL.mult)
                TT(Rt[:, :], Rt[:, :], t9[:, :], op=AL.add)
                rdiag = bass.AP(Rt.tensor, Rt.offset, [list(Rt.ap[0]), [4, 3]])
                TS(rdiag, rdiag, 1.0, AL.add)
                nc.vector.tensor_scalar(Vt[:, :], Kt[:, :], coB[:, :], None, AL.mult)
                nc.vector.tensor_scalar(t9[:, :], K2t[:, :], coC[:, :], None, AL.mult)
                TT(Vt[:, :], Vt[:, :], t9[:, :], op=AL.add)
                vdiag = bass.AP(Vt.tensor, Vt.offset, [list(Vt.ap[0]), [4, 3]])
                TS(vdiag, vdiag, 1.0, AL.add)
                # t = V @ v
                vbc = bass.AP(xi.tensor, xi.offset, [list(xi.ap[0]), [0, 3], [1, 3]])
                TT(t9[:, :], Vt[:, :], vbc, op=AL.mult)
                tv = sc.tile([2, 3], fp32, tag="tv")
                t9v = bass.AP(t9.tensor, t9.offset, [list(t9.ap[0]), [3, 3], [1, 3]])
                nc.vector.tensor_reduce(tv[:, :], t9v, axis=AX.X, op=AL.add)
                # E = [[R, t],[0,0,0,1]] as [2,16]
                Et = sc.tile([2, 16], fp32, tag="Et")
                nc.vector.memset(Et[:, :], 0.0)
                edst = bass.AP(Et.tensor, Et.offset, [list(Et.ap[0]), [4, 3], [1, 3]])
                esrc = bass.AP(Rt.tensor, Rt.offset, [list(Rt.ap[0]), [3, 3], [1, 3]])
                nc.vector.tensor_copy(edst, esrc)
                edst2 = bass.AP(Et.tensor, Et.offset + 3, [list(Et.ap[0]), [4, 3]])
                nc.vector.tensor_copy(edst2, tv[:, :])
                TS(Et[:, 15:16], Et[:, 15:16], 1.0, AL.add)
                # newT = T @ E
                nT = sc.tile([2, 16], fp32, tag="nT")
                for k in range(4):
                    tcol = bass.AP(Tq.tensor, Tq.offset + k, [list(Tq.ap[0]), [4, 4], [0, 4]])
                    erow = bass.AP(Et.tensor, Et.offset + 4 * k, [list(Et.ap[0]), [0, 4], [1, 4]])
                    if k == 0:
                        TT(nT[:, :], tcol, erow, op=AL.mult)
                    else:
                        TT(tmpj[:, :16], tcol, erow, op=AL.mult)
                        TT(nT[:, :], nT[:, :], tmpj[:, :16], op=AL.add)
                nc.vector.tensor_copy(Tq[:, :], nT[:, :])
                # rebuild q = [R(9) | t(3) | intr(4)] and broadcast to rtm/rtw
                qt = sc.tile([2, 16], fp32, tag="qt")
                qr = bass.AP(Tq.tensor, Tq.offset, [list(Tq.ap[0]), [4, 3], [1, 3]])
                nc.vector.tensor_copy(qt[:, 0:9], qr)
                qtcol = bass.AP(Tq.tensor, Tq.offset + 3, [list(Tq.ap[0]), [4, 3]])
                nc.vector.tensor_copy(qt[:, 9:12], qtcol)
                nc.vector.tensor_copy(qt[:, 12:16], intr[:, :])
                nc.sync.dma_start(out=qscr.ap(), in_=qt[:, :])
                qsap = qscr.ap()
                for b in range(2):
                    qsrc = bass.AP(qsap.tensor, qsap.offset + b * 16, [[0, 64], [1, 16]])
                    nc.sync.dma_start(out=rtw[b * 64:(b + 1) * 64, :], in_=qsrc)
                for g in range(8):
                    b = g // 4
                    qsrc = bass.AP(qsap.tensor, qsap.offset + b * 16, [[0, 128], [1, 16]])
                    rdst = bass.AP(rtm.tensor, rtm.offset + g, [list(rtm.ap[0]), [8, 16]])
                    nc.sync.dma_start(out=rdst, in_=qsrc)

            nc.sync.dma_start(out=tout_ext.ap(), in_=Tq[:, :])

    nc.finalize()
    return nc


def kernel(pose_twist, I0, I1, invD0, invD1, intrinsics):
    from concourse.bass_utils import run_bass_kernel_spmd

    nc = _NC_CACHE.get("nc")
    if nc is None:
        nc = build_nc()
        _NC_CACHE["nc"] = nc

    pose_twist = np.asarray(pose_twist, np.float32)
    I0 = np.asarray(I0, np.float32); I1 = np.asarray(I1, np.float32)
    invD0 = np.asarray(invD0, np.float32); invD1 = np.asarray(invD1, np.float32)
    intrinsics = np.asarray(intrinsics, np.float32)

    import time as _time
    LAST_WALL.clear(); LAST_EXEC_NS.clear(); LAST_TRACES.clear()
    t0 = _time.time()
    in_maps, _ = host_precompute_all(pose_twist, I0, I1, invD0, invD1, intrinsics)
    t1 = _time.time()
    res = run_bass_kernel_spmd(nc, in_maps, list(range(8)), trace=PROFILE)
    t2 = _time.time()
    LAST_WALL.extend([round(t1 - t0, 3), round(t2 - t1, 3)])
    if PROFILE:
        if res.exec_time_ns is not None:
            LAST_EXEC_NS.append(res.exec_time_ns)
        if res.instructions_and_trace is not None:
            LAST_TRACES.append(res.instructions_and_trace[1])

    outs = []
    for core in range(8):
        outs.append(res.results[core]["tout"].reshape(2, 4, 4))
    return np.concatenate(outs, axis=0).astype(np.float32)


# revision 38
# speedup vs baseline: 11.6004x; 1.5585x over previous
"""Trainium2 Bass kernel for nn_InvDirectImageAlign (inverse-compositional image alignment).

v3: ONE compiled NEFF runs all 5 Gauss-Newton iterations on device
(hardware For_i loop). Per core: 2 batch elements. Device does warp,
bilinear grid_sample (GPSIMD ap_gather from fp16 pair-dup band tables),
the JtWJ/Rhs normal equations via TensorEngine matmuls of a per-pixel
fp16 factor matrix G (JtWJ = sum_c G_c^T G_c), the 6x6 Cholesky solve,
se3_exp (Taylor series - angles are <<1 here) and the pose composition.
Inputs upload once; output is just the final 4x4 poses.

Chunking: (batch, 16-row y-band, 224-col x-half) = 80 chunks/core; the 8
GPSIMD partition-groups each own one chunk per superstep; 10 supersteps.
Two pixel layouts, bridged only by PE transposes of gathered data:
  mod-128:    pixel j of chunk(g,s) at partition j%128, free col (g, j//128)
  wrapped-16: pixel j at partition 16g + j%16, free col j//16   (ap_gather's
              index layout)
"""
import numpy as np

B, C, H, W = 16, 3, 320, 448
HW = H * W
N_ITERS = 5
LAMBDA = 0.01
HUBER_DELTA = 0.1
EPS = 1e-6

BH = 16            # band rows per chunk
CW = 224           # band cols per chunk
N = BH * CW        # 3584 px per chunk
A = N // 128       # 28
M = N // 16        # 224
NS = 10            # supersteps
TR = 67            # table rows (16 + 25 + 26)
TC = 266           # table cols (224 + 20 + 21 + 1)
NELEM = TR * TC    # 17822 pairs
YPAD = 25
XPAD = 20


def skew3(w):
    x, y, z = w[..., 0], w[..., 1], w[..., 2]
    O = np.zeros_like(x)
    return np.stack([np.stack([O, -z, y], -1),
                     np.stack([z, O, -x], -1),
                     np.stack([-y, x, O], -1)], -2)


def se3_exp(xi):
    xi = np.asarray(xi, np.float64)
    v, w = xi[:, :3], xi[:, 3:]
    th2 = np.sum(w * w, -1)[:, None, None]
    th2c = np.maximum(th2, 1e-16)
    th = np.sqrt(th2c)
    small = th2 < 1e-10
    Aa = np.where(small, 1.0 - th2 / 6.0, np.sin(th) / th)
    Bc = np.where(small, 0.5 - th2 / 24.0, (1.0 - np.cos(th)) / th2c)
    Cc = np.where(small, 1.0 / 6.0 - th2 / 120.0, (1.0 - Aa) / th2c)
    K = skew3(w)
    K2 = K @ K
    I = np.eye(3)
    R = I + Aa * K + Bc * K2
    V = I + Bc * K + Cc * K2
    t = np.einsum('bij,bj->bi', V, v)
    T = np.zeros((xi.shape[0], 4, 4))
    T[:, :3, :3] = R
    T[:, :3, 3] = t
    T[:, 3, 3] = 1.0
    return T.astype(np.float32)


def feature_gradient(img):
    p = np.pad(img, ((0, 0), (0, 0), (0, 0), (1, 1)), mode='edge')
    dx = 0.5 * (p[..., 2:] - p[..., :-2])
    p = np.pad(img, ((0, 0), (0, 0), (1, 1), (0, 0)), mode='edge')
    dy = 0.5 * (p[..., 2:, :] - p[..., :-2, :])
    return dx.astype(np.float32), dy.astype(np.float32)


def chunk_of(g, s):
    b = g // 4
    local = (g % 4) * 10 + s
    return b, local // 2, local % 2


def bases_of(yb, xh):
    r0, c0 = yb * BH, xh * CW
    rbase = int(np.clip(r0 - YPAD, 0, H - TR))
    cbase = int(np.clip(c0 - XPAD, 0, W - (TC - 1)))
    return rbase, cbase


def mod128_cols_batch(x):
    """[2,K,H,W] -> [128, NS*8*A*K] vectorized (one core's 2 batches)."""
    K = x.shape[1]
    # chunk (b, yb, xh): local = yb*2+xh; g = b*4 + local//10; s = local%10
    a = x.reshape(2, K, 20, BH, 2, CW)          # b K yb row xh col
    a = a.transpose(0, 2, 4, 1, 3, 5)           # b yb xh K row col
    a = a.reshape(2, 40, K, N)                  # local = yb*2+xh
    a = a.reshape(2, 4, 10, K, A, 128)          # b g4 s K a p
    a = a.transpose(5, 2, 0, 1, 4, 3)           # p s b g4 a K
    return np.ascontiguousarray(a.reshape(128, NS, 8, A, K).reshape(128, -1))


def wrap16_cols_batch(x, K):
    """[2,K,H,W] -> [128, NS*M*K] (partition 16g + j%16, col (j//16)*K + k)."""
    a = x.reshape(2, K, 20, BH, 2, CW)
    a = a.transpose(0, 2, 4, 1, 3, 5).reshape(2, 40, K, N)
    a = a.reshape(2, 4, 10, K, M, 16)           # b g4 s K m plo
    a = a.transpose(0, 1, 5, 2, 4, 3)           # b g4 plo s m K
    # partition = 16*(b*4+g4) + plo
    return np.ascontiguousarray(a.reshape(128, NS, M, K).reshape(128, -1))


def host_precompute_all(pose_twist, I0, I1, invD0, invD1, intr):
    """Vectorized over all B=16; returns per-core input dicts + T0 per core."""
    T0 = se3_exp(pose_twist)
    fx = intr[:, 0][:, None, None]; fy = intr[:, 1][:, None, None]
    cx = intr[:, 2][:, None, None]; cy = intr[:, 3][:, None, None]
    uu = np.arange(W, dtype=np.float32)[None, None, :]
    vv = np.arange(H, dtype=np.float32)[None, :, None]
    iD = np.maximum(invD1[:, 0], EPS).astype(np.float32)
    z1 = (1.0 / iD).astype(np.float32)
    xn = ((uu - cx) / fx).astype(np.float32)     # [B,1,W]
    yn = ((vv - cy) / fy).astype(np.float32)     # [B,H,1]
    x1 = xn * z1
    y1 = yn * z1
    dI0x, dI0y = feature_gradient(I0)
    dD0x, dD0y = feature_gradient(invD0)
    planes12 = np.concatenate([dI0x, dI0y, dD0x, dD0y, I0, invD0], axis=1).astype(np.float16)
    flat = planes12.reshape(B, 12, HW)
    pd = np.zeros((B, 12, HW + 1, 2), np.float16)
    pd[:, :, 1:, 0] = flat
    pd[:, :, :HW, 1] = flat

    X1 = np.stack([x1, y1, z1], 1).astype(np.float16)       # [B, 3, H, W]
    I1f = np.asarray(I1, np.float32)

    bw = np.zeros((128, NS, 4), np.float32)
    for g in range(8):
        for s in range(NS):
            _, yb, xh2 = chunk_of(g, s)
            rbase, cbase = bases_of(yb, xh2)
            bw[16 * g:16 * g + 16, s, 0] = rbase
            bw[16 * g:16 * g + 16, s, 1] = cbase - 1          # xf min
            bw[16 * g:16 * g + 16, s, 2] = cbase + (TC - 2)   # xf max
            bw[16 * g:16 * g + 16, s, 3] = 1 - cbase          # kx offset
    bw = np.ascontiguousarray(bw.reshape(128, NS * 4))
    idn = np.eye(128, dtype=np.float16)

    I1h = I1f.astype(np.float16)
    core_inputs, T0s = [], []
    for core in range(8):
        sl = slice(2 * core, 2 * core + 2)
        inp = {}
        inp["pd"] = np.ascontiguousarray(pd[sl].reshape(2, 12, (HW + 1) * 2))
        inp["x1m"] = mod128_cols_batch(X1[sl])
        inp["x1w"] = wrap16_cols_batch(X1[sl], 3)
        inp["i1m"] = mod128_cols_batch(I1h[sl])
        inp["bw"] = bw
        inp["idn"] = idn
        q = np.zeros((2, 16), np.float32)
        q[:, :9] = T0[sl, :3, :3].reshape(2, 9)
        q[:, 9:12] = T0[sl, :3, 3]
        q[:, 12:16] = intr[sl]
        rtm = np.zeros((128, 16, 8), np.float32)
        rtw = np.zeros((128, 16), np.float32)
        for g in range(8):
            bb = g // 4
            rtm[:, :, g] = q[bb][None, :]
            rtw[16 * g:16 * g + 16, :] = q[bb][None, :]
        inp["rtm"] = np.ascontiguousarray(rtm.reshape(128, 16 * 8))
        inp["rtw"] = rtw
        inp["t0q"] = np.ascontiguousarray(T0[sl].reshape(2, 16).astype(np.float32))
        inp["intr2"] = np.ascontiguousarray(intr[sl].astype(np.float32))
        core_inputs.append(inp)
        T0s.append(T0[sl])
    return core_inputs, T0s


_NC_CACHE = {}
PROFILE = False
LAST_EXEC_NS = []
LAST_TRACES = []
LAST_WALL = []


def build_nc():
    import concourse.bacc as bacc
    import concourse.bass as bass
    import concourse.tile as tile
    from concourse import mybir

    fp32 = mybir.dt.float32
    fp16 = mybir.dt.float16
    i16 = mybir.dt.int16
    i32 = mybir.dt.int32
    AL = mybir.AluOpType
    ACT = mybir.ActivationFunctionType
    AX = mybir.AxisListType

    nc = bacc.Bacc("TRN2", target_bir_lowering=False, debug=False, num_devices=8)

    pd_in = nc.dram_tensor("pd", [2, 12, (HW + 1) * 2], fp16, kind="ExternalInput")
    x1m_in = nc.dram_tensor("x1m", [128, NS * 8 * A * 3], fp16, kind="ExternalInput")
    x1w_in = nc.dram_tensor("x1w", [128, NS * M * 3], fp16, kind="ExternalInput")
    i1m_in = nc.dram_tensor("i1m", [128, NS * 8 * A * 3], fp16, kind="ExternalInput")
    bw_in = nc.dram_tensor("bw", [128, NS * 4], fp32, kind="ExternalInput")
    idn_in = nc.dram_tensor("idn", [128, 128], fp16, kind="ExternalInput")
    rtm_in = nc.dram_tensor("rtm", [128, 16 * 8], fp32, kind="ExternalInput")
    rtw_in = nc.dram_tensor("rtw", [128, 16], fp32, kind="ExternalInput")
    t0_in = nc.dram_tensor("t0q", [2, 16], fp32, kind="ExternalInput")
    intr_in = nc.dram_tensor("intr2", [2, 4], fp32, kind="ExternalInput")
    tout_ext = nc.dram_tensor("tout", [2, 16], fp32, kind="ExternalOutput")
    qscr = nc.dram_tensor("qscr", [2, 16], fp32, kind="Internal")

    with tile.TileContext(nc) as tc:
        with tc.tile_pool(name="cst", bufs=1) as cpool, \
             tc.tile_pool(name="tblp", bufs=1) as tpool, \
             tc.tile_pool(name="strm", bufs=2) as sp, \
             tc.tile_pool(name="scr", bufs=1) as sc, \
             tc.tile_pool(name="gath", bufs=1) as gp, \
             tc.tile_pool(name="ps", bufs=2, space="PSUM") as pp, \
             tc.tile_pool(name="jp", bufs=1, space="PSUM") as jp:

            rtm = cpool.tile([128, 16 * 8], fp32, tag="rtm")
            rtm0 = cpool.tile([128, 16 * 8], fp32, tag="rtm0")
            rtw = cpool.tile([128, 16], fp32, tag="rtw")
            bwc = cpool.tile([128, NS * 4], fp32, tag="bw")
            idn = cpool.tile([128, 128], fp16, tag="idn")
            Tq = cpool.tile([2, 16], fp32, tag="Tq")
            intr = cpool.tile([2, 4], fp32, tag="intr")
            nc.sync.dma_start(out=rtm[:, :], in_=rtm_in.ap())
            nc.sync.dma_start(out=rtm0[:, :], in_=rtm_in.ap())
            nc.sync.dma_start(out=rtw[:, :], in_=rtw_in.ap())
            nc.sync.dma_start(out=bwc[:, :], in_=bw_in.ap())
            nc.sync.dma_start(out=idn[:, :], in_=idn_in.ap())
            nc.sync.dma_start(out=Tq[:, :], in_=t0_in.ap())
            nc.sync.dma_start(out=intr[:, :], in_=intr_in.ap())

            psJ = [jp.tile([28, 28], fp32, name=f"psJ{b}", tag=f"psJ{b}") for b in range(2)]

            tbl0 = tpool.tile([128, NELEM * 2], fp16, tag="tbl")
            nc.vector.memset(tbl0[:, :], 0.0)

            def rq(qi):   # mod-128 per-group broadcast: dims (g x8, a x A step0)
                sl = rtm[:, qi * 8:(qi + 1) * 8]
                return bass.AP(sl.tensor, sl.offset, [list(sl.ap[0]), [1, 8], [0, A]])

            def rqw(qi):  # wrapped per-partition scalar bcast over M
                sl = rtw[:, qi:qi + 1]
                return bass.AP(sl.tensor, sl.offset, [list(sl.ap[0]), [0, M]])

            def rtwS(qi):  # wrapped per-partition scalar [128,1]
                return rtw[:, qi:qi + 1]

            def bwq(s, j):
                sl = bwc[:, s * 4 + j:s * 4 + j + 1]
                return bass.AP(sl.tensor, sl.offset, [list(sl.ap[0]), [0, M]])

            def bwS(s, j):
                return bwc[:, s * 4 + j:s * 4 + j + 1]

            TT = nc.vector.tensor_tensor
            TS = lambda out, in0, s1, op: nc.vector.tensor_scalar(out, in0, s1, None, op)
            TS2 = lambda out, in0, s1, s2, op0, op1: nc.vector.tensor_scalar(out, in0, s1, s2, op0, op1)
            STT = nc.vector.scalar_tensor_tensor

            with tc.For_i(0, N_ITERS) as _it:
                for s in range(NS):
                    tbl = tbl0
                    for g in range(8):
                        b, yb, xh = chunk_of(g, s)
                        rbase, cbase = bases_of(yb, xh)
                        start = (rbase * W + cbase) * 2
                        src0 = pd_in.ap()
                        src = bass.AP(src0.tensor,
                                      src0.offset + b * 12 * (HW + 1) * 2 + start,
                                      [[(HW + 1) * 2, 12], [W * 2, TR], [1, TC * 2]])
                        dsl = tbl[16 * g:16 * g + 12, :]
                        dst = bass.AP(dsl.tensor, dsl.offset,
                                      [[dsl.ap[0][0], 12], [TC * 2, TR], [1, TC * 2]])
                        nc.sync.dma_start(out=dst, in_=src)

                    x1w = sp.tile([128, M * 3], fp16, tag="x1w")
                    nc.sync.dma_start(out=x1w[:, :], in_=x1w_in.ap()[:, s * M * 3:(s + 1) * M * 3])
                    x1m = sp.tile([128, 8 * A * 3], fp16, tag="x1m")
                    nc.sync.dma_start(out=x1m[:, :], in_=x1m_in.ap()[:, s * 8 * A * 3:(s + 1) * 8 * A * 3])
                    i1 = sp.tile([128, 8 * A * 3], fp16, tag="i1")
                    nc.sync.dma_start(out=i1[:, :], in_=i1m_in.ap()[:, s * 8 * A * 3:(s + 1) * 8 * A * 3])

                    # ---------- wrapped-16 idx pipeline ----------
                    def xw(k):
                        sl = x1w[:, :]
                        return bass.AP(sl.tensor, sl.offset + k, [list(sl.ap[0]), [3, M]])

                    def tw(name):
                        return sc.tile([128, M], fp32, name="w_" + name + f"_{s}", tag="w_" + name)

                    t1w = tw("t1")
                    X0zw = tw("X0z")
                    STT(X0zw[:, :], xw(0), rtwS(6), rqw(11), AL.mult, AL.add)
                    STT(X0zw[:, :], xw(1), rtwS(7), X0zw[:, :], AL.mult, AL.add)
                    STT(X0zw[:, :], xw(2), rtwS(8), X0zw[:, :], AL.mult, AL.add)
                    X0xw = tw("X0x")
                    STT(X0xw[:, :], xw(0), rtwS(0), rqw(9), AL.mult, AL.add)
                    STT(X0xw[:, :], xw(1), rtwS(1), X0xw[:, :], AL.mult, AL.add)
                    STT(X0xw[:, :], xw(2), rtwS(2), X0xw[:, :], AL.mult, AL.add)
                    X0yw = tw("X0y")
                    STT(X0yw[:, :], xw(0), rtwS(3), rqw(10), AL.mult, AL.add)
                    STT(X0yw[:, :], xw(1), rtwS(4), X0yw[:, :], AL.mult, AL.add)
                    STT(X0yw[:, :], xw(2), rtwS(5), X0yw[:, :], AL.mult, AL.add)

                    izw = tw("iz")
                    TS(t1w[:, :], X0zw[:, :], EPS, AL.max)
                    nc.vector.reciprocal_approx_fast(izw[:, :], t1w[:, :])
                    u0w = tw("u0"); v0w = tw("v0")
                    TT(u0w[:, :], X0xw[:, :], izw[:, :], op=AL.mult)
                    STT(u0w[:, :], u0w[:, :], rtwS(12), rqw(14), AL.mult, AL.add)
                    TT(v0w[:, :], X0yw[:, :], izw[:, :], op=AL.mult)
                    STT(v0w[:, :], v0w[:, :], rtwS(13), rqw(15), AL.mult, AL.add)
                    TS2(u0w[:, :], u0w[:, :], -0.5 * (W - 1), 1.5 * (W - 1), AL.max, AL.min)
                    TS2(v0w[:, :], v0w[:, :], -0.5 * (H - 1), 1.5 * (H - 1), AL.max, AL.min)
                    x0fw = tw("x0f"); y0fw = tw("y0f")
                    fi32w = sc.tile([128, M], i32, name=f"fi32w_{s}", tag="fi32w")
                    TS(t1w[:, :], u0w[:, :], 0.5, AL.subtract)
                    nc.vector.tensor_copy(fi32w[:, :], t1w[:, :])
                    nc.vector.tensor_copy(x0fw[:, :], fi32w[:, :])
                    TS(t1w[:, :], v0w[:, :], 0.5, AL.subtract)
                    nc.vector.tensor_copy(fi32w[:, :], t1w[:, :])
                    nc.vector.tensor_copy(y0fw[:, :], fi32w[:, :])
                    xfw = tw("xf"); kxw = tw("kx"); yrw = tw("yr")
                    ktw = tw("kt"); kbw = tw("kb")
                    STT(xfw[:, :], x0fw[:, :], bwS(s, 1), bwq(s, 2), AL.max, AL.min)
                    nc.vector.tensor_scalar(kxw[:, :], xfw[:, :], bwS(s, 3), None, AL.add)
                    nc.vector.tensor_scalar(yrw[:, :], y0fw[:, :], bwS(s, 0), 0.0, AL.subtract, AL.max)
                    TS2(ktw[:, :], yrw[:, :], float(TR - 1), float(TC), AL.min, AL.mult)
                    TT(ktw[:, :], ktw[:, :], kxw[:, :], op=AL.add)
                    TS2(kbw[:, :], yrw[:, :], 1.0, float(TR - 1), AL.add, AL.min)
                    TS(kbw[:, :], kbw[:, :], float(TC), AL.mult)
                    TT(kbw[:, :], kbw[:, :], kxw[:, :], op=AL.add)
                    kidx = sc.tile([128, 2 * M], i16, name=f"kidx_{s}", tag="kidx")
                    nc.vector.tensor_copy(kidx[:, :M], ktw[:, :])
                    nc.vector.tensor_copy(kidx[:, M:], kbw[:, :])

                    gt2 = gp.tile([128, 2 * N * 2], fp16, tag="gt2")
                    nc.gpsimd.ap_gather(gt2[:, :], tbl[:, :], kidx[:, :],
                                        channels=128, num_elems=NELEM, d=2, num_idxs=2 * N)

                    # ---------- mod-128 warp pipeline ----------
                    def xm(k):
                        sl = x1m[:, :]
                        return bass.AP(sl.tensor, sl.offset + k, [list(sl.ap[0]), [3, 8 * A]])

                    def tm(name):
                        return sc.tile([128, 8 * A], fp32, name="m_" + name + f"_{s}", tag="m_" + name)

                    def matvec(dst, aps, qis, t1):
                        TT(dst[:, :], aps[0], qis[0], op=AL.mult)
                        TT(t1[:, :], aps[1], qis[1], op=AL.mult)
                        TT(dst[:, :], dst[:, :], t1[:, :], op=AL.add)
                        TT(t1[:, :], aps[2], qis[2], op=AL.mult)
                        TT(dst[:, :], dst[:, :], t1[:, :], op=AL.add)
                        TT(dst[:, :], dst[:, :], qis[3], op=AL.add)

                    # ---- on-device A6/B6/T6 at the initial pose (rtm0) ----
                    abt = sc.tile([128, 8 * A * 18], fp16, name=f"abt_{s}", tag="abt")

                    def acol(k):
                        sl = abt[:, :]
                        return bass.AP(sl.tensor, sl.offset + k, [list(sl.ap[0]), [18, 224]])

                    def rq0(qi):
                        sl = rtm0[:, qi * 8:(qi + 1) * 8]
                        return bass.AP(sl.tensor, sl.offset, [list(sl.ap[0]), [1, 8], [0, A]])

                    j1 = tm("j1"); j2 = tm("j2")
                    jx = tm("jx"); jy = tm("jy"); jz = tm("jz"); jiz = tm("jiz")
                    matvec(jz, [xm(0), xm(1), xm(2)], [rq0(6), rq0(7), rq0(8), rq0(11)], j1)
                    matvec(jx, [xm(0), xm(1), xm(2)], [rq0(0), rq0(1), rq0(2), rq0(9)], j1)
                    matvec(jy, [xm(0), xm(1), xm(2)], [rq0(3), rq0(4), rq0(5), rq0(10)], j1)
                    TS(j1[:, :], jz[:, :], EPS, AL.max)
                    nc.vector.reciprocal_approx_fast(jiz[:, :], j1[:, :])
                    fxiz = tm("fxiz"); fyiz = tm("fyiz"); zizt = tm("zizt")
                    A2t = tm("A2t"); B2t = tm("B2t")
                    TT(fxiz[:, :], jiz[:, :], rq0(12), op=AL.mult)
                    TT(fyiz[:, :], jiz[:, :], rq0(13), op=AL.mult)
                    TT(zizt[:, :], jz[:, :], jiz[:, :], op=AL.mult)
                    TT(j1[:, :], jx[:, :], jiz[:, :], op=AL.mult)
                    TT(A2t[:, :], fxiz[:, :], j1[:, :], op=AL.mult)
                    TT(j1[:, :], jy[:, :], jiz[:, :], op=AL.mult)
                    TT(B2t[:, :], fyiz[:, :], j1[:, :], op=AL.mult)
                    TS(acol(0), fxiz[:, :], -1.0, AL.mult)
                    TS(acol(1), fxiz[:, :], 0.0, AL.mult)
                    nc.vector.tensor_copy(acol(2), A2t[:, :])
                    TT(acol(3), A2t[:, :], jy[:, :], op=AL.mult)
                    TT(j1[:, :], fxiz[:, :], zizt[:, :], op=AL.mult)
                    TT(j2[:, :], A2t[:, :], jx[:, :], op=AL.mult)
                    TT(j1[:, :], j1[:, :], j2[:, :], op=AL.add)
                    TS(acol(4), j1[:, :], -1.0, AL.mult)
                    TT(acol(5), fxiz[:, :], jy[:, :], op=AL.mult)
                    TS(acol(6), fxiz[:, :], 0.0, AL.mult)
                    TS(acol(7), fyiz[:, :], -1.0, AL.mult)
                    nc.vector.tensor_copy(acol(8), B2t[:, :])
                    TT(j1[:, :], fyiz[:, :], zizt[:, :], op=AL.mult)
                    TT(j2[:, :], B2t[:, :], jy[:, :], op=AL.mult)
                    TT(acol(9), j1[:, :], j2[:, :], op=AL.add)
                    TT(j1[:, :], B2t[:, :], jx[:, :], op=AL.mult)
                    TS(acol(10), j1[:, :], -1.0, AL.mult)
                    TT(j1[:, :], fyiz[:, :], jx[:, :], op=AL.mult)
                    TS(acol(11), j1[:, :], -1.0, AL.mult)
                    TS(acol(12), fxiz[:, :], 0.0, AL.mult)
                    TS(acol(13), fxiz[:, :], 0.0, AL.mult)
                    TS2(acol(14), fxiz[:, :], 0.0, 1.0, AL.mult, AL.add)
                    nc.vector.tensor_copy(acol(15), jy[:, :])
                    TS(acol(16), jx[:, :], -1.0, AL.mult)
                    TS(acol(17), fxiz[:, :], 0.0, AL.mult)

                    m1 = tm("m1"); m2 = tm("m2")
                    X0z = tm("X0z")
                    matvec(X0z, [xm(0), xm(1), xm(2)], [rq(6), rq(7), rq(8), rq(11)], m1)
                    X0x = tm("X0x")
                    matvec(X0x, [xm(0), xm(1), xm(2)], [rq(0), rq(1), rq(2), rq(9)], m1)
                    X0y = tm("X0y")
                    matvec(X0y, [xm(0), xm(1), xm(2)], [rq(3), rq(4), rq(5), rq(10)], m1)
                    iz = tm("iz")
                    TS(m1[:, :], X0z[:, :], EPS, AL.max)
                    nc.vector.reciprocal_approx_fast(iz[:, :], m1[:, :])
                    u0 = tm("u0"); v0 = tm("v0")
                    TT(u0[:, :], X0x[:, :], iz[:, :], op=AL.mult)
                    TT(u0[:, :], u0[:, :], rq(12), op=AL.mult)
                    TT(u0[:, :], u0[:, :], rq(14), op=AL.add)
                    TT(v0[:, :], X0y[:, :], iz[:, :], op=AL.mult)
                    TT(v0[:, :], v0[:, :], rq(13), op=AL.mult)
                    TT(v0[:, :], v0[:, :], rq(15), op=AL.add)
                    vmask = tm("vmask")
                    TS(vmask[:, :], X0z[:, :], EPS, AL.is_gt)
                    STT(vmask[:, :], u0[:, :], 0.0, vmask[:, :], AL.is_gt, AL.mult)
                    STT(vmask[:, :], u0[:, :], float(W - 1), vmask[:, :], AL.is_lt, AL.mult)
                    STT(vmask[:, :], v0[:, :], 0.0, vmask[:, :], AL.is_gt, AL.mult)
                    STT(vmask[:, :], v0[:, :], float(H - 1), vmask[:, :], AL.is_lt, AL.mult)
                    TS2(u0[:, :], u0[:, :], -0.5 * (W - 1), 1.5 * (W - 1), AL.max, AL.min)
                    TS2(v0[:, :], v0[:, :], -0.5 * (H - 1), 1.5 * (H - 1), AL.max, AL.min)
                    wx = tm("wx"); wy = tm("wy"); x0f = tm("x0f"); y0f = tm("y0f")
                    fi32m = sc.tile([128, 8 * A], i32, name=f"fi32m_{s}", tag="fi32m")
                    TS(m1[:, :], u0[:, :], 0.5, AL.subtract)
                    nc.vector.tensor_copy(fi32m[:, :], m1[:, :])
                    nc.vector.tensor_copy(x0f[:, :], fi32m[:, :])
                    TT(wx[:, :], u0[:, :], x0f[:, :], op=AL.subtract)
                    TS(m1[:, :], v0[:, :], 0.5, AL.subtract)
                    nc.vector.tensor_copy(fi32m[:, :], m1[:, :])
                    nc.vector.tensor_copy(y0f[:, :], fi32m[:, :])
                    TT(wy[:, :], v0[:, :], y0f[:, :], op=AL.subtract)
                    mx0 = tm("mx0"); mx1 = tm("mx1"); my0 = tm("my0"); my1 = tm("my1")
                    TS(mx0[:, :], x0f[:, :], -0.5, AL.is_gt)
                    STT(mx0[:, :], x0f[:, :], float(W - 1) + 0.5, mx0[:, :], AL.is_lt, AL.mult)
                    TS(mx1[:, :], x0f[:, :], -1.5, AL.is_gt)
                    STT(mx1[:, :], x0f[:, :], float(W - 2) + 0.5, mx1[:, :], AL.is_lt, AL.mult)
                    TS(my0[:, :], y0f[:, :], -0.5, AL.is_gt)
                    STT(my0[:, :], y0f[:, :], float(H - 1) + 0.5, my0[:, :], AL.is_lt, AL.mult)
                    TS(my1[:, :], y0f[:, :], -1.5, AL.is_gt)
                    STT(my1[:, :], y0f[:, :], float(H - 2) + 0.5, my1[:, :], AL.is_lt, AL.mult)
                    W00 = tm("W00"); W01 = tm("W01"); W10 = tm("W10"); W11 = tm("W11")
                    TS2(m1[:, :], wx[:, :], 1.0, -1.0, AL.subtract, AL.mult)  # 1-wx
                    TS2(m2[:, :], wy[:, :], 1.0, -1.0, AL.subtract, AL.mult)  # 1-wy
                    TT(W00[:, :], m1[:, :], m2[:, :], op=AL.mult)
                    TT(W00[:, :], W00[:, :], mx0[:, :], op=AL.mult)
                    TT(W00[:, :], W00[:, :], my0[:, :], op=AL.mult)
                    TT(W01[:, :], wx[:, :], m2[:, :], op=AL.mult)
                    TT(W01[:, :], W01[:, :], mx1[:, :], op=AL.mult)
                    TT(W01[:, :], W01[:, :], my0[:, :], op=AL.mult)
                    TT(W10[:, :], m1[:, :], wy[:, :], op=AL.mult)
                    TT(W10[:, :], W10[:, :], mx0[:, :], op=AL.mult)
                    TT(W10[:, :], W10[:, :], my1[:, :], op=AL.mult)
                    TT(W11[:, :], wx[:, :], wy[:, :], op=AL.mult)
                    TT(W11[:, :], W11[:, :], mx1[:, :], op=AL.mult)
                    TT(W11[:, :], W11[:, :], my1[:, :], op=AL.mult)

                    # ---------- PE transpose + combine ----------
                    samp = sc.tile([128, A * 128], fp16, tag="samp")
                    ctmp = sc.tile([128, 512], fp16, tag="ctmp")
                    for a4 in range(A // 4):
                        ptall = pp.tile([128, 2048], fp16, tag="ptall")
                        for ci, base in enumerate((0, 1, 2 * N, 2 * N + 1)):
                            for aa in range(4):
                                a = a4 * 4 + aa
                                src = bass.AP(gt2.tensor, gt2.offset + base + a * 256,
                                              [list(gt2.ap[0]), [2, 128]])
                                nc.tensor.transpose(
                                    ptall[:, ci * 512 + aa * 128:ci * 512 + (aa + 1) * 128],
                                    src, idn[:, :])
                        for ci, wt_ in ((0, W00), (1, W01), (2, W10), (3, W11)):
                            pap = bass.AP(ptall.tensor, ptall.offset + ci * 512,
                                          [list(ptall.ap[0]), [128, 4], [16, 8], [1, 16]])
                            woff = wt_.offset + a4 * 4
                            wap = bass.AP(wt_.tensor, woff, [list(wt_.ap[0]), [1, 4], [A, 8], [0, 16]])
                            dst_off = samp.offset + a4 * 4 * 128
                            dap = bass.AP(samp.tensor, dst_off, [list(samp.ap[0]), [128, 4], [16, 8], [1, 16]])
                            if ci == 0:
                                TT(dap, pap, wap, op=AL.mult)
                            else:
                                tap = bass.AP(ctmp.tensor, ctmp.offset, [list(ctmp.ap[0]), [128, 4], [16, 8], [1, 16]])
                                TT(tap, pap, wap, op=AL.mult)
                                TT(dap, dap, tap, op=AL.add)

                    # ---------- residuals, huber weights, G build ----------
                    def sq(q):
                        sl = samp[:, :]
                        return bass.AP(sl.tensor, sl.offset + q, [list(sl.ap[0]), [16, 8], [128, A]])

                    def i1q(c):
                        sl = i1[:, :]
                        return bass.AP(sl.tensor, sl.offset + c, [list(sl.ap[0]), [3 * A, 8], [3, A]])

                    Gt = sc.tile([128, 28 * 224], fp16, tag="Gt")
                    g6a = sc.tile([128, 6 * 224], fp16, tag="g6a")
                    g6b = sc.tile([128, 6 * 224], fp16, tag="g6b")
                    one_m = tm("one_m")
                    TS2(one_m[:, :], vmask[:, :], 1.0, -1e-6, AL.subtract, AL.mult)  # (1-vm)*1e-6
                    rr = tm("rr"); bb_ = tm("bb"); ss = tm("ss")
                    ppv = tm("ppv"); qqv = tm("qqv")

                    def abt6(k0):  # [x(6) outer, chunk(224) inner], stride 18 per chunk
                        sl = abt[:, :]
                        return bass.AP(sl.tensor, sl.offset + k0, [list(sl.ap[0]), [1, 6], [18, 224]])

                    def gcols(c):  # G cols c*7 .. c*7+5: [x outer, chunk inner]
                        sl = Gt[:, :]
                        return bass.AP(sl.tensor, sl.offset + c * 7 * 224, [list(sl.ap[0]), [224, 6], [1, 224]])

                    def bc6(t):    # broadcast [128,224] over 6 x-cols
                        sl = t[:, :]
                        return bass.AP(sl.tensor, sl.offset, [list(sl.ap[0]), [0, 6], [1, 224]])

                    for c in range(3):
                        TT(rr[:, :], i1q(c), sq(8 + c), op=AL.subtract)
                        TT(rr[:, :], rr[:, :], vmask[:, :], op=AL.mult)
                        TT(rr[:, :], rr[:, :], one_m[:, :], op=AL.add)
                        nc.scalar.activation(bb_[:, :], rr[:, :], ACT.Abs)
                        TS(bb_[:, :], bb_[:, :], HUBER_DELTA, AL.max)
                        nc.vector.reciprocal_approx_fast(bb_[:, :], bb_[:, :])
                        nc.scalar.activation(ss[:, :], bb_[:, :], ACT.Sqrt, scale=HUBER_DELTA)
                        TT(ppv[:, :], ss[:, :], sq(0 + c), op=AL.mult)
                        TT(qqv[:, :], ss[:, :], sq(3 + c), op=AL.mult)
                        TT(g6a[:, :], abt6(0), bc6(ppv), op=AL.mult)
                        TT(g6b[:, :], abt6(6), bc6(qqv), op=AL.mult)
                        TT(gcols(c), g6a[:, :], g6b[:, :], op=AL.add)
                        TT(Gt[:, (c * 7 + 6) * 224:(c * 7 + 7) * 224], ss[:, :], rr[:, :], op=AL.mult)
                    # depth channel
                    TT(rr[:, :], iz[:, :], sq(11), op=AL.subtract)
                    TT(rr[:, :], rr[:, :], vmask[:, :], op=AL.mult)
                    TT(rr[:, :], rr[:, :], one_m[:, :], op=AL.add)
                    nc.scalar.activation(bb_[:, :], rr[:, :], ACT.Abs, scale=LAMBDA)
                    TS(bb_[:, :], bb_[:, :], HUBER_DELTA, AL.max)
                    nc.vector.reciprocal_approx_fast(bb_[:, :], bb_[:, :])
                    nc.scalar.activation(ss[:, :], bb_[:, :], ACT.Sqrt,
                                         scale=HUBER_DELTA * LAMBDA * LAMBDA)
                    TT(ppv[:, :], ss[:, :], sq(6), op=AL.mult)
                    TT(qqv[:, :], ss[:, :], sq(7), op=AL.mult)
                    TT(g6a[:, :], abt6(0), bc6(ppv), op=AL.mult)
                    TT(g6b[:, :], abt6(6), bc6(qqv), op=AL.mult)
                    TT(g6a[:, :], g6a[:, :], g6b[:, :], op=AL.add)
                    TT(g6b[:, :], abt6(12), bc6(ss), op=AL.mult)
                    TT(gcols(3), g6a[:, :], g6b[:, :], op=AL.add)
                    TT(Gt[:, (3 * 7 + 6) * 224:(3 * 7 + 7) * 224], ss[:, :], rr[:, :], op=AL.mult)

                    # ---------- PE: JtWJ accumulation ----------
                    for g in range(8):
                        b = g // 4
                        for a in range(A):
                            off = Gt.offset + g * A + a
                            gap = bass.AP(Gt.tensor, off, [list(Gt.ap[0]), [224, 28]])
                            first = (s == 0 and (g % 4) == 0 and a == 0)
                            last = (s == NS - 1 and (g % 4) == 3 and a == A - 1)
                            nc.tensor.matmul(psJ[b][:, :], gap, gap,
                                             start=first, stop=last,
                                             skip_group_check=True)

                # ---------- per-iteration: extract JtWJ/Rhs, solve, update pose ----------
                S28 = sc.tile([28, 56], fp32, tag="S28")
                for b in range(2):
                    nc.vector.tensor_copy(S28[:, b * 28:(b + 1) * 28], psJ[b][:, :])
                D28 = sc.tile([7, 56], fp32, tag="D28")
                for b in range(2):
                    for c in range(4):
                        src = S28[c * 7:(c + 1) * 7, b * 28 + c * 7:b * 28 + c * 7 + 7]
                        dsl = D28[:, b * 28 + c * 7:b * 28 + c * 7 + 7]
                        nc.sync.dma_start(out=dsl, in_=src)
                M7 = sc.tile([7, 14], fp32, tag="M7")
                for b in range(2):
                    din = bass.AP(D28.tensor, D28.offset + b * 28,
                                  [list(D28.ap[0]), [1, 7], [7, 4]])
                    nc.vector.tensor_reduce(M7[:, b * 7:(b + 1) * 7], din, axis=AX.X, op=AL.add)
                # Mb [2, 49]: row b = M7 block b flattened (x-major)
                Mb = sc.tile([2, 49], fp32, tag="Mb")
                for b in range(2):
                    msrc = bass.AP(M7.tensor, M7.offset + b * 7, [[M7.ap[0][0], 7], [1, 7]])
                    mdsl = Mb[b:b + 1, 0:1]
                    mdst = bass.AP(mdsl.tensor, mdsl.offset, [[Mb.ap[0][0], 1], [7, 7], [1, 7]])
                    nc.sync.dma_start(out=mdst, in_=msrc)
                # tr = sum diag(JtWJ); LM ridge on diag
                trt = sc.tile([2, 1], fp32, tag="trt")
                diag = bass.AP(Mb.tensor, Mb.offset, [list(Mb.ap[0]), [8, 6]])
                nc.vector.tensor_reduce(trt[:, :], diag, axis=AX.X, op=AL.add)
                trb = bass.AP(trt.tensor, trt.offset, [list(trt.ap[0]), [0, 6]])
                STT(diag, trb, 1e-6, diag, AL.mult, AL.add)

                # Cholesky LL^T = Hm (6x6, both batches in 2 partitions)
                Lt = sc.tile([2, 36], fp32, tag="Lt")
                lsrc = bass.AP(Mb.tensor, Mb.offset, [list(Mb.ap[0]), [7, 6], [1, 6]])
                nc.vector.tensor_copy(Lt[:, :], lsrc)
                rhs = sc.tile([2, 6], fp32, tag="rhs")
                rsrc = bass.AP(Mb.tensor, Mb.offset + 6, [list(Mb.ap[0]), [7, 6]])
                nc.vector.tensor_copy(rhs[:, :], rsrc)
                idg = sc.tile([2, 6], fp32, tag="idg")
                tmpj = sc.tile([2, 36], fp32, tag="tmpj")
                red = sc.tile([2, 6], fp32, tag="redj")
                for j in range(6):
                    jj = Lt[:, 6 * j + j:6 * j + j + 1]
                    if j > 0:
                        ljk = Lt[:, 6 * j:6 * j + j]
                        TT(tmpj[:, :j], ljk, ljk, op=AL.mult)
                        nc.vector.tensor_reduce(red[:, 0:1], tmpj[:, :j], axis=AX.X, op=AL.add)
                        TT(jj, jj, red[:, 0:1], op=AL.subtract)
                    nc.scalar.activation(jj, jj, ACT.Sqrt)
                    nc.vector.reciprocal(idg[:, j:j + 1], jj)
                    nr = 5 - j
                    if nr > 0:
                        colap = bass.AP(Lt.tensor, Lt.offset + 6 * (j + 1) + j, [list(Lt.ap[0]), [6, nr]])
                        if j > 0:
                            lik = bass.AP(Lt.tensor, Lt.offset + 6 * (j + 1), [list(Lt.ap[0]), [6, nr], [1, j]])
                            ljkb = bass.AP(Lt.tensor, Lt.offset + 6 * j, [list(Lt.ap[0]), [0, nr], [1, j]])
                            TT(tmpj[:, :nr * j], lik, ljkb, op=AL.mult)
                            tin = bass.AP(tmpj.tensor, tmpj.offset, [list(tmpj.ap[0]), [j, nr], [1, j]])
                            nc.vector.tensor_reduce(red[:, :nr], tin, axis=AX.X, op=AL.add)
                            TT(colap, colap, red[:, :nr], op=AL.subtract)
                        nc.vector.tensor_scalar(colap, colap, idg[:, j:j + 1], None, AL.mult)
                # forward substitution: L y = rhs (in place on rhs)
                for j in range(6):
                    yj = rhs[:, j:j + 1]
                    if j > 0:
                        ljk = Lt[:, 6 * j:6 * j + j]
                        TT(tmpj[:, :j], ljk, rhs[:, :j], op=AL.mult)
                        nc.vector.tensor_reduce(red[:, 0:1], tmpj[:, :j], axis=AX.X, op=AL.add)
                        TT(yj, yj, red[:, 0:1], op=AL.subtract)
                    nc.vector.tensor_scalar(yj, yj, idg[:, j:j + 1], None, AL.mult)
                # back substitution: L^T x = y -> xi = -x stored in xi tile
                for j in range(5, -1, -1):
                    xj = rhs[:, j:j + 1]
                    nk = 5 - j
                    if nk > 0:
                        lkj = bass.AP(Lt.tensor, Lt.offset + 6 * (j + 1) + j, [list(Lt.ap[0]), [6, nk]])
                        TT(tmpj[:, :nk], lkj, rhs[:, j + 1:6], op=AL.mult)
                        nc.vector.tensor_reduce(red[:, 0:1], tmpj[:, :nk], axis=AX.X, op=AL.add)
                        TT(xj, xj, red[:, 0:1], op=AL.subtract)
                    nc.vector.tensor_scalar(xj, xj, idg[:, j:j + 1], None, AL.mult)
                xi = sc.tile([2, 6], fp32, tag="xi")
                TS(xi[:, :], rhs[:, :], -1.0, AL.mult)

                # se3_exp(xi) via Taylor series (|w| << 1 in this regime)
                w3 = xi[:, 3:6]
                wsq = sc.tile([2, 3], fp32, tag="wsq")
                TT(wsq[:, :], w3, w3, op=AL.mult)
                th2 = sc.tile([2, 1], fp32, tag="th2")
                nc.vector.tensor_reduce(th2[:, :], wsq[:, :], axis=AX.X, op=AL.add)
                coA = sc.tile([2, 1], fp32, tag="coA")
                coB = sc.tile([2, 1], fp32, tag="coB")
                coC = sc.tile([2, 1], fp32, tag="coC")
                hh = sc.tile([2, 1], fp32, tag="hh")
                TS2(hh[:, :], th2[:, :], 1.0 / 120.0, -1.0 / 6.0, AL.mult, AL.add)
                nc.vector.tensor_scalar(coA[:, :], th2[:, :], hh[:, :], 1.0, AL.mult, AL.add)
                TS2(hh[:, :], th2[:, :], 1.0 / 720.0, -1.0 / 24.0, AL.mult, AL.add)
                nc.vector.tensor_scalar(coB[:, :], th2[:, :], hh[:, :], 0.5, AL.mult, AL.add)
                TS2(hh[:, :], th2[:, :], 1.0 / 5040.0, -1.0 / 120.0, AL.mult, AL.add)
                nc.vector.tensor_scalar(coC[:, :], th2[:, :], hh[:, :], 1.0 / 6.0, AL.mult, AL.add)
                # K, K2
                Kt = sc.tile([2, 9], fp32, tag="Kt")
                nc.vector.memset(Kt[:, :], 0.0)
                TS(Kt[:, 1:2], xi[:, 5:6], -1.0, AL.mult)   # -z
                nc.vector.tensor_copy(Kt[:, 2:3], xi[:, 4:5])  # y
                nc.vector.tensor_copy(Kt[:, 3:4], xi[:, 5:6])  # z
                TS(Kt[:, 5:6], xi[:, 3:4], -1.0, AL.mult)   # -x
                TS(Kt[:, 6:7], xi[:, 4:5], -1.0, AL.mult)   # -y
                nc.vector.tensor_copy(Kt[:, 7:8], xi[:, 3:4])  # x
                K2t = sc.tile([2, 9], fp32, tag="K2t")
                wiap = bass.AP(xi.tensor, xi.offset + 3, [list(xi.ap[0]), [1, 3], [0, 3]])
                wjap = bass.AP(xi.tensor, xi.offset + 3, [list(xi.ap[0]), [0, 3], [1, 3]])
                TT(K2t[:, :], wiap, wjap, op=AL.mult)
                k2diag = bass.AP(K2t.tensor, K2t.offset, [list(K2t.ap[0]), [4, 3]])
                nc.vector.tensor_scalar(k2diag, k2diag, th2[:, :], None, AL.subtract)
                Rt = sc.tile([2, 9], fp32, tag="Rt")
                Vt = sc.tile([2, 9], fp32, tag="Vt")
                t9 = sc.tile([2, 9], fp32, tag="t9")
                nc.vector.tensor_scalar(Rt[:, :], Kt[:, :], coA[:, :], None, AL.mult)
                nc.vector.tensor_scalar(t9[:, :], K2t[:, :], coB[:, :], None, AL.mult)
                TT(Rt[:, :], Rt[:, :], t9[:, :], op=AL.add)
                rdiag = bass.AP(Rt.tensor, Rt.offset, [list(Rt.ap[0]), [4, 3]])
                TS(rdiag, rdiag, 1.0, AL.add)
                nc.vector.tensor_scalar(Vt[:, :], Kt[:, :], coB[:, :], None, AL.mult)
                nc.vector.tensor_scalar(t9[:, :], K2t[:, :], coC[:, :], None, AL.mult)
                TT(Vt[:, :], Vt[:, :], t9[:, :], op=AL.add)
                vdiag = bass.AP(Vt.tensor, Vt.offset, [list(Vt.ap[0]), [4, 3]])
                TS(vdiag, vdiag, 1.0, AL.add)
                # t = V @ v
                vbc = bass.AP(xi.tensor, xi.offset, [list(xi.ap[0]), [0, 3], [1, 3]])
                TT(t9[:, :], Vt[:, :], vbc, op=AL.mult)
                tv = sc.tile([2, 3], fp32, tag="tv")
                t9v = bass.AP(t9.tensor, t9.offset, [list(t9.ap[0]), [3, 3], [1, 3]])
                nc.vector.tensor_reduce(tv[:, :], t9v, axis=AX.X, op=AL.add)
                # E = [[R, t],[0,0,0,1]] as [2,16]
                Et = sc.tile([2, 16], fp32, tag="Et")
                nc.vector.memset(Et[:, :], 0.0)
                edst = bass.AP(Et.tensor, Et.offset, [list(Et.ap[0]), [4, 3], [1, 3]])
                esrc = bass.AP(Rt.tensor, Rt.offset, [list(Rt.ap[0]), [3, 3], [1, 3]])
                nc.vector.tensor_copy(edst, esrc)
                edst2 = bass.AP(Et.tensor, Et.offset + 3, [list(Et.ap[0]), [4, 3]])
                nc.vector.tensor_copy(edst2, tv[:, :])
                TS(Et[:, 15:16], Et[:, 15:16], 1.0, AL.add)
                # newT = T @ E
                nT = sc.tile([2, 16], fp32, tag="nT")
                for k in range(4):
                    tcol = bass.AP(Tq.tensor, Tq.offset + k, [list(Tq.ap[0]), [4, 4], [0, 4]])
                    erow = bass.AP(Et.tensor, Et.offset + 4 * k, [list(Et.ap[0]), [0, 4], [1, 4]])
                    if k == 0:
                        TT(nT[:, :], tcol, erow, op=AL.mult)
                    else:
                        TT(tmpj[:, :16], tcol, erow, op=AL.mult)
                        TT(nT[:, :], nT[:, :], tmpj[:, :16], op=AL.add)
                nc.vector.tensor_copy(Tq[:, :], nT[:, :])
                # rebuild q = [R(9) | t(3) | intr(4)] and broadcast to rtm/rtw
                qt = sc.tile([2, 16], fp32, tag="qt")
                qr = bass.AP(Tq.tensor, Tq.offset, [list(Tq.ap[0]), [4, 3], [1, 3]])
                nc.vector.tensor_copy(qt[:, 0:9], qr)
                qtcol = bass.AP(Tq.tensor, Tq.offset + 3, [list(Tq.ap[0]), [4, 3]])
                nc.vector.tensor_copy(qt[:, 9:12], qtcol)
                nc.vector.tensor_copy(qt[:, 12:16], intr[:, :])
                nc.sync.dma_start(out=qscr.ap(), in_=qt[:, :])
                qsap = qscr.ap()
                for b in range(2):
                    qsrc = bass.AP(qsap.tensor, qsap.offset + b * 16, [[0, 64], [1, 16]])
                    nc.sync.dma_start(out=rtw[b * 64:(b + 1) * 64, :], in_=qsrc)
                for g in range(8):
                    b = g // 4
                    qsrc = bass.AP(qsap.tensor, qsap.offset + b * 16, [[0, 128], [1, 16]])
                    rdst = bass.AP(rtm.tensor, rtm.offset + g, [list(rtm.ap[0]), [8, 16]])
                    nc.sync.dma_start(out=rdst, in_=qsrc)

            nc.sync.dma_start(out=tout_ext.ap(), in_=Tq[:, :])

    nc.finalize()
    return nc


def kernel(pose_twist, I0, I1, invD0, invD1, intrinsics):
    from concourse.bass_utils import run_bass_kernel_spmd

    nc = _NC_CACHE.get("nc")
    if nc is None:
        nc = build_nc()
        _NC_CACHE["nc"] = nc

    pose_twist = np.asarray(pose_twist, np.float32)
    I0 = np.asarray(I0, np.float32); I1 = np.asarray(I1, np.float32)
    invD0 = np.asarray(invD0, np.float32); invD1 = np.asarray(invD1, np.float32)
    intrinsics = np.asarray(intrinsics, np.float32)

    import time as _time
    LAST_WALL.clear(); LAST_EXEC_NS.clear(); LAST_TRACES.clear()
    t0 = _time.time()
    in_maps, _ = host_precompute_all(pose_twist, I0, I1, invD0, invD1, intrinsics)
    t1 = _time.time()
    res = run_bass_kernel_spmd(nc, in_maps, list(range(8)), trace=PROFILE)
    t2 = _time.time()
    LAST_WALL.extend([round(t1 - t0, 3), round(t2 - t1, 3)])
    if PROFILE:
        if res.exec_time_ns is not None:
            LAST_EXEC_NS.append(res.exec_time_ns)
        if res.instructions_and_trace is not None:
            LAST_TRACES.append(res.instructions_and_trace[1])

    outs = []
    for core in range(8):
        outs.append(res.results[core]["tout"].reshape(2, 4, 4))
    return np.concatenate(outs, axis=0).astype(np.float32)


# revision 40
# speedup vs baseline: 13.1186x; 1.1309x over previous
"""Trainium2 Bass kernel for nn_InvDirectImageAlign (inverse-compositional image alignment).

v3: ONE compiled NEFF runs all 5 Gauss-Newton iterations on device
(hardware For_i loop). Per core: 2 batch elements. Device does warp,
bilinear grid_sample (GPSIMD ap_gather from fp16 pair-dup band tables),
the JtWJ/Rhs normal equations via TensorEngine matmuls of a per-pixel
fp16 factor matrix G (JtWJ = sum_c G_c^T G_c), the 6x6 Cholesky solve,
se3_exp (Taylor series - angles are <<1 here) and the pose composition.
Inputs upload once; output is just the final 4x4 poses.

Chunking: (batch, 16-row y-band, 224-col x-half) = 80 chunks/core; the 8
GPSIMD partition-groups each own one chunk per superstep; 10 supersteps.
Two pixel layouts, bridged only by PE transposes of gathered data:
  mod-128:    pixel j of chunk(g,s) at partition j%128, free col (g, j//128)
  wrapped-16: pixel j at partition 16g + j%16, free col j//16   (ap_gather's
              index layout)
"""
import numpy as np

B, C, H, W = 16, 3, 320, 448
HW = H * W
N_ITERS = 5
LAMBDA = 0.01
HUBER_DELTA = 0.1
EPS = 1e-6

BH = 16            # band rows per chunk
CW = 224           # band cols per chunk
N = BH * CW        # 3584 px per chunk
A = N // 128       # 28
M = N // 16        # 224
NS = 10            # supersteps
TR = 67            # table rows (16 + 25 + 26)
TC = 266           # table cols (224 + 20 + 21 + 1)
NELEM = TR * TC    # 17822 pairs
YPAD = 25
XPAD = 20


def skew3(w):
    x, y, z = w[..., 0], w[..., 1], w[..., 2]
    O = np.zeros_like(x)
    return np.stack([np.stack([O, -z, y], -1),
                     np.stack([z, O, -x], -1),
                     np.stack([-y, x, O], -1)], -2)


def se3_exp(xi):
    xi = np.asarray(xi, np.float64)
    v, w = xi[:, :3], xi[:, 3:]
    th2 = np.sum(w * w, -1)[:, None, None]
    th2c = np.maximum(th2, 1e-16)
    th = np.sqrt(th2c)
    small = th2 < 1e-10
    Aa = np.where(small, 1.0 - th2 / 6.0, np.sin(th) / th)
    Bc = np.where(small, 0.5 - th2 / 24.0, (1.0 - np.cos(th)) / th2c)
    Cc = np.where(small, 1.0 / 6.0 - th2 / 120.0, (1.0 - Aa) / th2c)
    K = skew3(w)
    K2 = K @ K
    I = np.eye(3)
    R = I + Aa * K + Bc * K2
    V = I + Bc * K + Cc * K2
    t = np.einsum('bij,bj->bi', V, v)
    T = np.zeros((xi.shape[0], 4, 4))
    T[:, :3, :3] = R
    T[:, :3, 3] = t
    T[:, 3, 3] = 1.0
    return T.astype(np.float32)


def feature_gradient(img):
    p = np.pad(img, ((0, 0), (0, 0), (0, 0), (1, 1)), mode='edge')
    dx = 0.5 * (p[..., 2:] - p[..., :-2])
    p = np.pad(img, ((0, 0), (0, 0), (1, 1), (0, 0)), mode='edge')
    dy = 0.5 * (p[..., 2:, :] - p[..., :-2, :])
    return dx.astype(np.float32), dy.astype(np.float32)


def chunk_of(g, s):
    b = g // 4
    local = (g % 4) * 10 + s
    return b, local // 2, local % 2


def bases_of(yb, xh):
    r0, c0 = yb * BH, xh * CW
    rbase = int(np.clip(r0 - YPAD, 0, H - TR))
    cbase = int(np.clip(c0 - XPAD, 0, W - (TC - 1)))
    return rbase, cbase


def mod128_cols_batch(x):
    """[2,K,H,W] -> [128, NS*8*A*K] vectorized (one core's 2 batches)."""
    K = x.shape[1]
    # chunk (b, yb, xh): local = yb*2+xh; g = b*4 + local//10; s = local%10
    a = x.reshape(2, K, 20, BH, 2, CW)          # b K yb row xh col
    a = a.transpose(0, 2, 4, 1, 3, 5)           # b yb xh K row col
    a = a.reshape(2, 40, K, N)                  # local = yb*2+xh
    a = a.reshape(2, 4, 10, K, A, 128)          # b g4 s K a p
    a = a.transpose(5, 2, 0, 1, 4, 3)           # p s b g4 a K
    return np.ascontiguousarray(a.reshape(128, NS, 8, A, K).reshape(128, -1))


def wrap16_cols_batch(x, K):
    """[2,K,H,W] -> [128, NS*M*K] (partition 16g + j%16, col (j//16)*K + k)."""
    a = x.reshape(2, K, 20, BH, 2, CW)
    a = a.transpose(0, 2, 4, 1, 3, 5).reshape(2, 40, K, N)
    a = a.reshape(2, 4, 10, K, M, 16)           # b g4 s K m plo
    a = a.transpose(0, 1, 5, 2, 4, 3)           # b g4 plo s m K
    # partition = 16*(b*4+g4) + plo
    return np.ascontiguousarray(a.reshape(128, NS, M, K).reshape(128, -1))


def host_precompute_all(pose_twist, I0, I1, invD0, invD1, intr):
    """Vectorized over all B=16; returns per-core input dicts + T0 per core."""
    T0 = se3_exp(pose_twist)
    fx = intr[:, 0][:, None, None]; fy = intr[:, 1][:, None, None]
    cx = intr[:, 2][:, None, None]; cy = intr[:, 3][:, None, None]
    uu = np.arange(W, dtype=np.float32)[None, None, :]
    vv = np.arange(H, dtype=np.float32)[None, :, None]
    iD = np.maximum(invD1[:, 0], EPS).astype(np.float32)
    z1 = (1.0 / iD).astype(np.float32)
    xn = ((uu - cx) / fx).astype(np.float32)     # [B,1,W]
    yn = ((vv - cy) / fy).astype(np.float32)     # [B,H,1]
    x1 = xn * z1
    y1 = yn * z1
    dI0x, dI0y = feature_gradient(I0)
    dD0x, dD0y = feature_gradient(invD0)
    planes12 = np.concatenate([dI0x, dI0y, dD0x, dD0y, I0, invD0], axis=1).astype(np.float16)
    flat = planes12.reshape(B, 12, HW)
    pds = np.zeros((B, 12, HW + 2), np.float16)
    pds[:, :, 1:HW + 1] = flat

    X1 = np.stack([x1, y1, z1], 1).astype(np.float32)       # [B, 3, H, W]
    I1f = np.asarray(I1, np.float32)

    bw = np.zeros((128, NS, 4), np.float32)
    for g in range(8):
        for s in range(NS):
            _, yb, xh2 = chunk_of(g, s)
            rbase, cbase = bases_of(yb, xh2)
            bw[16 * g:16 * g + 16, s, 0] = rbase
            bw[16 * g:16 * g + 16, s, 1] = cbase - 1          # xf min
            bw[16 * g:16 * g + 16, s, 2] = cbase + (TC - 2)   # xf max
            bw[16 * g:16 * g + 16, s, 3] = 1 - cbase          # kx offset
    bw = np.ascontiguousarray(bw.reshape(128, NS * 4))
    idn = np.eye(128, dtype=np.float16)

    I1h = I1f.astype(np.float16)
    core_inputs, T0s = [], []
    for core in range(8):
        sl = slice(2 * core, 2 * core + 2)
        inp = {}
        inp["pds"] = np.ascontiguousarray(pds[sl])
        inp["x1m"] = mod128_cols_batch(X1[sl])
        inp["x1w"] = wrap16_cols_batch(X1[sl], 3)
        inp["i1m"] = mod128_cols_batch(I1h[sl])
        inp["bw"] = bw
        inp["idn"] = idn
        q = np.zeros((2, 16), np.float32)
        q[:, :9] = T0[sl, :3, :3].reshape(2, 9)
        q[:, 9:12] = T0[sl, :3, 3]
        q[:, 12:16] = intr[sl]
        rtm = np.zeros((128, 16, 8), np.float32)
        rtw = np.zeros((128, 16), np.float32)
        for g in range(8):
            bb = g // 4
            rtm[:, :, g] = q[bb][None, :]
            rtw[16 * g:16 * g + 16, :] = q[bb][None, :]
        inp["rtm"] = np.ascontiguousarray(rtm.reshape(128, 16 * 8))
        inp["rtw"] = rtw
        inp["t0q"] = np.ascontiguousarray(T0[sl].reshape(2, 16).astype(np.float32))
        inp["intr2"] = np.ascontiguousarray(intr[sl].astype(np.float32))
        core_inputs.append(inp)
        T0s.append(T0[sl])
    return core_inputs, T0s


_NC_CACHE = {}
PROFILE = False
LAST_EXEC_NS = []
LAST_TRACES = []
LAST_WALL = []


def build_nc():
    import concourse.bacc as bacc
    import concourse.bass as bass
    import concourse.tile as tile
    from concourse import mybir

    fp32 = mybir.dt.float32
    fp16 = mybir.dt.float16
    i16 = mybir.dt.int16
    i32 = mybir.dt.int32
    AL = mybir.AluOpType
    ACT = mybir.ActivationFunctionType
    AX = mybir.AxisListType

    nc = bacc.Bacc("TRN2", target_bir_lowering=False, debug=False, num_devices=8)

    pd_in = nc.dram_tensor("pds", [2, 12, HW + 2], fp16, kind="ExternalInput")
    x1m_in = nc.dram_tensor("x1m", [128, NS * 8 * A * 3], fp32, kind="ExternalInput")
    x1w_in = nc.dram_tensor("x1w", [128, NS * M * 3], fp32, kind="ExternalInput")
    i1m_in = nc.dram_tensor("i1m", [128, NS * 8 * A * 3], fp16, kind="ExternalInput")
    bw_in = nc.dram_tensor("bw", [128, NS * 4], fp32, kind="ExternalInput")
    idn_in = nc.dram_tensor("idn", [128, 128], fp16, kind="ExternalInput")
    rtm_in = nc.dram_tensor("rtm", [128, 16 * 8], fp32, kind="ExternalInput")
    rtw_in = nc.dram_tensor("rtw", [128, 16], fp32, kind="ExternalInput")
    t0_in = nc.dram_tensor("t0q", [2, 16], fp32, kind="ExternalInput")
    intr_in = nc.dram_tensor("intr2", [2, 4], fp32, kind="ExternalInput")
    tout_ext = nc.dram_tensor("tout", [2, 16], fp32, kind="ExternalOutput")
    qscr = nc.dram_tensor("qscr", [2, 16], fp32, kind="Internal")

    with tile.TileContext(nc) as tc:
        with tc.tile_pool(name="cst", bufs=1) as cpool, \
             tc.tile_pool(name="tblp", bufs=1) as tpool, \
             tc.tile_pool(name="strm", bufs=2) as sp, \
             tc.tile_pool(name="scr", bufs=1) as sc, \
             tc.tile_pool(name="gath", bufs=1) as gp, \
             tc.tile_pool(name="ps", bufs=2, space="PSUM") as pp, \
             tc.tile_pool(name="jp", bufs=1, space="PSUM") as jp:

            rtm = cpool.tile([128, 16 * 8], fp32, tag="rtm")
            rtm0 = cpool.tile([128, 16 * 8], fp32, tag="rtm0")
            rtw = cpool.tile([128, 16], fp32, tag="rtw")
            bwc = cpool.tile([128, NS * 4], fp32, tag="bw")
            idn = cpool.tile([128, 128], fp16, tag="idn")
            Tq = cpool.tile([2, 16], fp32, tag="Tq")
            intr = cpool.tile([2, 4], fp32, tag="intr")
            nc.sync.dma_start(out=rtm[:, :], in_=rtm_in.ap())
            nc.sync.dma_start(out=rtm0[:, :], in_=rtm_in.ap())
            nc.sync.dma_start(out=rtw[:, :], in_=rtw_in.ap())
            nc.sync.dma_start(out=bwc[:, :], in_=bw_in.ap())
            nc.sync.dma_start(out=idn[:, :], in_=idn_in.ap())
            nc.sync.dma_start(out=Tq[:, :], in_=t0_in.ap())
            nc.sync.dma_start(out=intr[:, :], in_=intr_in.ap())

            psJ = [jp.tile([28, 28], fp32, name=f"psJ{b}", tag=f"psJ{b}") for b in range(2)]

            tbl0 = tpool.tile([128, NELEM * 2], fp16, tag="tbl")
            nc.vector.memset(tbl0[:, :], 0.0)
            stbl0 = tpool.tile([128, 34 * (TC + 1)], fp16, tag="stbl")
            nc.vector.memset(stbl0[:, :], 0.0)

            def rq(qi):   # mod-128 per-group broadcast: dims (g x8, a x A step0)
                sl = rtm[:, qi * 8:(qi + 1) * 8]
                return bass.AP(sl.tensor, sl.offset, [list(sl.ap[0]), [1, 8], [0, A]])

            def rqw(qi):  # wrapped per-partition scalar bcast over M
                sl = rtw[:, qi:qi + 1]
                return bass.AP(sl.tensor, sl.offset, [list(sl.ap[0]), [0, M]])

            def rtwS(qi):  # wrapped per-partition scalar [128,1]
                return rtw[:, qi:qi + 1]

            def bwq(s, j):
                sl = bwc[:, s * 4 + j:s * 4 + j + 1]
                return bass.AP(sl.tensor, sl.offset, [list(sl.ap[0]), [0, M]])

            def bwS(s, j):
                return bwc[:, s * 4 + j:s * 4 + j + 1]

            TT = nc.vector.tensor_tensor
            TS = lambda out, in0, s1, op: nc.vector.tensor_scalar(out, in0, s1, None, op)
            TS2 = lambda out, in0, s1, s2, op0, op1: nc.vector.tensor_scalar(out, in0, s1, s2, op0, op1)
            STT = nc.vector.scalar_tensor_tensor

            with tc.For_i(0, N_ITERS) as _it:
                for s in range(NS):
                    tbl = tbl0
                    for r0, nr in ((0, 34), (34, 33)):
                        for g in range(8):
                            b, yb, xh = chunk_of(g, s)
                            rbase, cbase = bases_of(yb, xh)
                            start = (rbase + r0) * W + cbase
                            src0 = pd_in.ap()
                            src = bass.AP(src0.tensor,
                                          src0.offset + b * 12 * (HW + 2) + start,
                                          [[HW + 2, 12], [W, nr], [1, TC + 1]])
                            dsl = stbl0[16 * g:16 * g + 12, :]
                            dst = bass.AP(dsl.tensor, dsl.offset,
                                          [[dsl.ap[0][0], 12], [TC + 1, nr], [1, TC + 1]])
                            nc.sync.dma_start(out=dst, in_=src)
                        for e in range(2):
                            pout = bass.AP(tbl.tensor, tbl.offset + e + r0 * 2 * TC,
                                           [list(tbl.ap[0]), [2 * TC, nr], [2, TC]])
                            pin = bass.AP(stbl0.tensor, stbl0.offset + e,
                                          [list(stbl0.ap[0]), [TC + 1, nr], [1, TC]])
                            nc.scalar.activation(pout, pin, ACT.Copy)

                    x1w = sp.tile([128, M * 3], fp32, tag="x1w")
                    nc.sync.dma_start(out=x1w[:, :], in_=x1w_in.ap()[:, s * M * 3:(s + 1) * M * 3])
                    x1m = sp.tile([128, 8 * A * 3], fp32, tag="x1m")
                    nc.sync.dma_start(out=x1m[:, :], in_=x1m_in.ap()[:, s * 8 * A * 3:(s + 1) * 8 * A * 3])
                    i1 = sp.tile([128, 8 * A * 3], fp16, tag="i1")
                    nc.sync.dma_start(out=i1[:, :], in_=i1m_in.ap()[:, s * 8 * A * 3:(s + 1) * 8 * A * 3])

                    # ---------- wrapped-16 idx pipeline ----------
                    def xw(k):
                        sl = x1w[:, :]
                        return bass.AP(sl.tensor, sl.offset + k, [list(sl.ap[0]), [3, M]])

                    def tw(name):
                        return sc.tile([128, M], fp32, name="w_" + name + f"_{s}", tag="w_" + name)

                    t1w = tw("t1")
                    X0zw = tw("X0z")
                    STT(X0zw[:, :], xw(0), rtwS(6), rqw(11), AL.mult, AL.add)
                    STT(X0zw[:, :], xw(1), rtwS(7), X0zw[:, :], AL.mult, AL.add)
                    STT(X0zw[:, :], xw(2), rtwS(8), X0zw[:, :], AL.mult, AL.add)
                    X0xw = tw("X0x")
                    STT(X0xw[:, :], xw(0), rtwS(0), rqw(9), AL.mult, AL.add)
                    STT(X0xw[:, :], xw(1), rtwS(1), X0xw[:, :], AL.mult, AL.add)
                    STT(X0xw[:, :], xw(2), rtwS(2), X0xw[:, :], AL.mult, AL.add)
                    X0yw = tw("X0y")
                    STT(X0yw[:, :], xw(0), rtwS(3), rqw(10), AL.mult, AL.add)
                    STT(X0yw[:, :], xw(1), rtwS(4), X0yw[:, :], AL.mult, AL.add)
                    STT(X0yw[:, :], xw(2), rtwS(5), X0yw[:, :], AL.mult, AL.add)

                    izw = tw("iz")
                    TS(t1w[:, :], X0zw[:, :], EPS, AL.max)
                    nc.vector.reciprocal_approx_fast(izw[:, :], t1w[:, :])
                    u0w = tw("u0"); v0w = tw("v0")
                    TT(u0w[:, :], X0xw[:, :], izw[:, :], op=AL.mult)
                    STT(u0w[:, :], u0w[:, :], rtwS(12), rqw(14), AL.mult, AL.add)
                    TT(v0w[:, :], X0yw[:, :], izw[:, :], op=AL.mult)
                    STT(v0w[:, :], v0w[:, :], rtwS(13), rqw(15), AL.mult, AL.add)
                    TS2(u0w[:, :], u0w[:, :], -0.5 * (W - 1), 1.5 * (W - 1), AL.max, AL.min)
                    TS2(v0w[:, :], v0w[:, :], -0.5 * (H - 1), 1.5 * (H - 1), AL.max, AL.min)
                    x0fw = tw("x0f"); y0fw = tw("y0f")
                    fi32w = sc.tile([128, M], i32, name=f"fi32w_{s}", tag="fi32w")
                    TS(t1w[:, :], u0w[:, :], 0.5, AL.subtract)
                    nc.vector.tensor_copy(fi32w[:, :], t1w[:, :])
                    nc.vector.tensor_copy(x0fw[:, :], fi32w[:, :])
                    TS(t1w[:, :], v0w[:, :], 0.5, AL.subtract)
                    nc.vector.tensor_copy(fi32w[:, :], t1w[:, :])
                    nc.vector.tensor_copy(y0fw[:, :], fi32w[:, :])
                    xfw = t1w; kxw = izw; yrw = X0zw
                    ktw = X0xw; kbw = X0yw
                    STT(xfw[:, :], x0fw[:, :], bwS(s, 1), bwq(s, 2), AL.max, AL.min)
                    nc.vector.tensor_scalar(kxw[:, :], xfw[:, :], bwS(s, 3), None, AL.add)
                    nc.vector.tensor_scalar(yrw[:, :], y0fw[:, :], bwS(s, 0), 0.0, AL.subtract, AL.max)
                    TS2(ktw[:, :], yrw[:, :], float(TR - 1), float(TC), AL.min, AL.mult)
                    TT(ktw[:, :], ktw[:, :], kxw[:, :], op=AL.add)
                    TS2(kbw[:, :], yrw[:, :], 1.0, float(TR - 1), AL.add, AL.min)
                    TS(kbw[:, :], kbw[:, :], float(TC), AL.mult)
                    TT(kbw[:, :], kbw[:, :], kxw[:, :], op=AL.add)
                    kidx = sc.tile([128, 2 * M], i16, name=f"kidx_{s}", tag="kidx")
                    nc.vector.tensor_copy(kidx[:, :M], ktw[:, :])
                    nc.vector.tensor_copy(kidx[:, M:], kbw[:, :])

                    gt2 = gp.tile([128, 2 * N * 2], fp16, tag="gt2")
                    nc.gpsimd.ap_gather(gt2[:, :], tbl[:, :], kidx[:, :],
                                        channels=128, num_elems=NELEM, d=2, num_idxs=2 * N)

                    # ---------- mod-128 warp pipeline ----------
                    def xm(k):
                        sl = x1m[:, :]
                        return bass.AP(sl.tensor, sl.offset + k, [list(sl.ap[0]), [3, 8 * A]])

                    def tm(name):
                        return sc.tile([128, 8 * A], fp32, name="m_" + name + f"_{s}", tag="m_" + name)

                    def matvec(dst, aps, qis, t1):
                        TT(dst[:, :], aps[0], qis[0], op=AL.mult)
                        TT(t1[:, :], aps[1], qis[1], op=AL.mult)
                        TT(dst[:, :], dst[:, :], t1[:, :], op=AL.add)
                        TT(t1[:, :], aps[2], qis[2], op=AL.mult)
                        TT(dst[:, :], dst[:, :], t1[:, :], op=AL.add)
                        TT(dst[:, :], dst[:, :], qis[3], op=AL.add)

                    # ---- on-device A6/B6/T6 at the initial pose (rtm0) ----
                    abt = sc.tile([128, 8 * A * 18], fp16, name=f"abt_{s}", tag="abt")

                    def acol(k):
                        sl = abt[:, :]
                        return bass.AP(sl.tensor, sl.offset + k, [list(sl.ap[0]), [18, 224]])

                    def rq0(qi):
                        sl = rtm0[:, qi * 8:(qi + 1) * 8]
                        return bass.AP(sl.tensor, sl.offset, [list(sl.ap[0]), [1, 8], [0, A]])

                    j1 = tm("j1"); j2 = tm("j2")
                    jx = tm("jx"); jy = tm("jy"); jz = tm("jz"); jiz = tm("jiz")
                    matvec(jz, [xm(0), xm(1), xm(2)], [rq0(6), rq0(7), rq0(8), rq0(11)], j1)
                    matvec(jx, [xm(0), xm(1), xm(2)], [rq0(0), rq0(1), rq0(2), rq0(9)], j1)
                    matvec(jy, [xm(0), xm(1), xm(2)], [rq0(3), rq0(4), rq0(5), rq0(10)], j1)
                    TS(j1[:, :], jz[:, :], EPS, AL.max)
                    nc.vector.reciprocal_approx_fast(jiz[:, :], j1[:, :])
                    fxiz = tm("fxiz"); fyiz = tm("fyiz"); zizt = tm("zizt")
                    A2t = tm("A2t"); B2t = tm("B2t")
                    TT(fxiz[:, :], jiz[:, :], rq0(12), op=AL.mult)
                    TT(fyiz[:, :], jiz[:, :], rq0(13), op=AL.mult)
                    TT(zizt[:, :], jz[:, :], jiz[:, :], op=AL.mult)
                    TT(j1[:, :], jx[:, :], jiz[:, :], op=AL.mult)
                    TT(A2t[:, :], fxiz[:, :], j1[:, :], op=AL.mult)
                    TT(j1[:, :], jy[:, :], jiz[:, :], op=AL.mult)
                    TT(B2t[:, :], fyiz[:, :], j1[:, :], op=AL.mult)
                    TS(acol(0), fxiz[:, :], -1.0, AL.mult)
                    TS(acol(1), fxiz[:, :], 0.0, AL.mult)
                    nc.vector.tensor_copy(acol(2), A2t[:, :])
                    TT(acol(3), A2t[:, :], jy[:, :], op=AL.mult)
                    TT(j1[:, :], fxiz[:, :], zizt[:, :], op=AL.mult)
                    TT(j2[:, :], A2t[:, :], jx[:, :], op=AL.mult)
                    TT(j1[:, :], j1[:, :], j2[:, :], op=AL.add)
                    TS(acol(4), j1[:, :], -1.0, AL.mult)
                    TT(acol(5), fxiz[:, :], jy[:, :], op=AL.mult)
                    TS(acol(6), fxiz[:, :], 0.0, AL.mult)
                    TS(acol(7), fyiz[:, :], -1.0, AL.mult)
                    nc.vector.tensor_copy(acol(8), B2t[:, :])
                    TT(j1[:, :], fyiz[:, :], zizt[:, :], op=AL.mult)
                    TT(j2[:, :], B2t[:, :], jy[:, :], op=AL.mult)
                    TT(acol(9), j1[:, :], j2[:, :], op=AL.add)
                    TT(j1[:, :], B2t[:, :], jx[:, :], op=AL.mult)
                    TS(acol(10), j1[:, :], -1.0, AL.mult)
                    TT(j1[:, :], fyiz[:, :], jx[:, :], op=AL.mult)
                    TS(acol(11), j1[:, :], -1.0, AL.mult)
                    TS(acol(12), fxiz[:, :], 0.0, AL.mult)
                    TS(acol(13), fxiz[:, :], 0.0, AL.mult)
                    TS2(acol(14), fxiz[:, :], 0.0, 1.0, AL.mult, AL.add)
                    nc.vector.tensor_copy(acol(15), jy[:, :])
                    TS(acol(16), jx[:, :], -1.0, AL.mult)
                    TS(acol(17), fxiz[:, :], 0.0, AL.mult)

                    m1 = j1; m2 = j2
                    X0z = jz
                    matvec(X0z, [xm(0), xm(1), xm(2)], [rq(6), rq(7), rq(8), rq(11)], m1)
                    X0x = jx
                    matvec(X0x, [xm(0), xm(1), xm(2)], [rq(0), rq(1), rq(2), rq(9)], m1)
                    X0y = jy
                    matvec(X0y, [xm(0), xm(1), xm(2)], [rq(3), rq(4), rq(5), rq(10)], m1)
                    iz = jiz
                    TS(m1[:, :], X0z[:, :], EPS, AL.max)
                    nc.vector.reciprocal_approx_fast(iz[:, :], m1[:, :])
                    u0 = fxiz; v0 = fyiz
                    TT(u0[:, :], X0x[:, :], iz[:, :], op=AL.mult)
                    TT(u0[:, :], u0[:, :], rq(12), op=AL.mult)
                    TT(u0[:, :], u0[:, :], rq(14), op=AL.add)
                    TT(v0[:, :], X0y[:, :], iz[:, :], op=AL.mult)
                    TT(v0[:, :], v0[:, :], rq(13), op=AL.mult)
                    TT(v0[:, :], v0[:, :], rq(15), op=AL.add)
                    vmask = zizt
                    TS(vmask[:, :], X0z[:, :], EPS, AL.is_gt)
                    STT(vmask[:, :], u0[:, :], 0.0, vmask[:, :], AL.is_gt, AL.mult)
                    STT(vmask[:, :], u0[:, :], float(W - 1), vmask[:, :], AL.is_lt, AL.mult)
                    STT(vmask[:, :], v0[:, :], 0.0, vmask[:, :], AL.is_gt, AL.mult)
                    STT(vmask[:, :], v0[:, :], float(H - 1), vmask[:, :], AL.is_lt, AL.mult)
                    TS2(u0[:, :], u0[:, :], -0.5 * (W - 1), 1.5 * (W - 1), AL.max, AL.min)
                    TS2(v0[:, :], v0[:, :], -0.5 * (H - 1), 1.5 * (H - 1), AL.max, AL.min)
                    wx = A2t; wy = B2t; x0f = tm("x0f"); y0f = tm("y0f")
                    fi32m = sc.tile([128, 8 * A], i32, name=f"fi32m_{s}", tag="fi32m")
                    TS(m1[:, :], u0[:, :], 0.5, AL.subtract)
                    nc.vector.tensor_copy(fi32m[:, :], m1[:, :])
                    nc.vector.tensor_copy(x0f[:, :], fi32m[:, :])
                    TT(wx[:, :], u0[:, :], x0f[:, :], op=AL.subtract)
                    TS(m1[:, :], v0[:, :], 0.5, AL.subtract)
                    nc.vector.tensor_copy(fi32m[:, :], m1[:, :])
                    nc.vector.tensor_copy(y0f[:, :], fi32m[:, :])
                    TT(wy[:, :], v0[:, :], y0f[:, :], op=AL.subtract)
                    mx0 = tm("mx0"); mx1 = tm("mx1"); my0 = tm("my0"); my1 = tm("my1")
                    TS(mx0[:, :], x0f[:, :], -0.5, AL.is_gt)
                    STT(mx0[:, :], x0f[:, :], float(W - 1) + 0.5, mx0[:, :], AL.is_lt, AL.mult)
                    TS(mx1[:, :], x0f[:, :], -1.5, AL.is_gt)
                    STT(mx1[:, :], x0f[:, :], float(W - 2) + 0.5, mx1[:, :], AL.is_lt, AL.mult)
                    TS(my0[:, :], y0f[:, :], -0.5, AL.is_gt)
                    STT(my0[:, :], y0f[:, :], float(H - 1) + 0.5, my0[:, :], AL.is_lt, AL.mult)
                    TS(my1[:, :], y0f[:, :], -1.5, AL.is_gt)
                    STT(my1[:, :], y0f[:, :], float(H - 2) + 0.5, my1[:, :], AL.is_lt, AL.mult)
                    W00 = tm("W00"); W01 = tm("W01"); W10 = tm("W10"); W11 = tm("W11")
                    TS2(m1[:, :], wx[:, :], 1.0, -1.0, AL.subtract, AL.mult)  # 1-wx
                    TS2(m2[:, :], wy[:, :], 1.0, -1.0, AL.subtract, AL.mult)  # 1-wy
                    TT(W00[:, :], m1[:, :], m2[:, :], op=AL.mult)
                    TT(W00[:, :], W00[:, :], mx0[:, :], op=AL.mult)
                    TT(W00[:, :], W00[:, :], my0[:, :], op=AL.mult)
                    TT(W01[:, :], wx[:, :], m2[:, :], op=AL.mult)
                    TT(W01[:, :], W01[:, :], mx1[:, :], op=AL.mult)
                    TT(W01[:, :], W01[:, :], my0[:, :], op=AL.mult)
                    TT(W10[:, :], m1[:, :], wy[:, :], op=AL.mult)
                    TT(W10[:, :], W10[:, :], mx0[:, :], op=AL.mult)
                    TT(W10[:, :], W10[:, :], my1[:, :], op=AL.mult)
                    TT(W11[:, :], wx[:, :], wy[:, :], op=AL.mult)
                    TT(W11[:, :], W11[:, :], mx1[:, :], op=AL.mult)
                    TT(W11[:, :], W11[:, :], my1[:, :], op=AL.mult)

                    # ---------- PE transpose + combine ----------
                    samp = sc.tile([128, A * 128], fp16, tag="samp")
                    ctmp = sc.tile([128, 512], fp16, tag="ctmp")
                    for a4 in range(A // 4):
                        ptall = pp.tile([128, 2048], fp16, tag="ptall")
                        for ci, base in enumerate((0, 1, 2 * N, 2 * N + 1)):
                            for aa in range(4):
                                a = a4 * 4 + aa
                                src = bass.AP(gt2.tensor, gt2.offset + base + a * 256,
                                              [list(gt2.ap[0]), [2, 128]])
                                nc.tensor.transpose(
                                    ptall[:, ci * 512 + aa * 128:ci * 512 + (aa + 1) * 128],
                                    src, idn[:, :])
                        for ci, wt_ in ((0, W00), (1, W01), (2, W10), (3, W11)):
                            pap = bass.AP(ptall.tensor, ptall.offset + ci * 512,
                                          [list(ptall.ap[0]), [128, 4], [16, 8], [1, 16]])
                            woff = wt_.offset + a4 * 4
                            wap = bass.AP(wt_.tensor, woff, [list(wt_.ap[0]), [1, 4], [A, 8], [0, 16]])
                            dst_off = samp.offset + a4 * 4 * 128
                            dap = bass.AP(samp.tensor, dst_off, [list(samp.ap[0]), [128, 4], [16, 8], [1, 16]])
                            if ci == 0:
                                TT(dap, pap, wap, op=AL.mult)
                            else:
                                tap = bass.AP(ctmp.tensor, ctmp.offset, [list(ctmp.ap[0]), [128, 4], [16, 8], [1, 16]])
                                TT(tap, pap, wap, op=AL.mult)
                                TT(dap, dap, tap, op=AL.add)

                    # ---------- residuals, huber weights, G build ----------
                    def sq(q):
                        sl = samp[:, :]
                        return bass.AP(sl.tensor, sl.offset + q, [list(sl.ap[0]), [16, 8], [128, A]])

                    def i1q(c):
                        sl = i1[:, :]
                        return bass.AP(sl.tensor, sl.offset + c, [list(sl.ap[0]), [3 * A, 8], [3, A]])

                    Gt = sc.tile([128, 28 * 224], fp16, tag="Gt")
                    g6a = sc.tile([128, 6 * 224], fp16, tag="g6a")
                    g6b = sc.tile([128, 6 * 224], fp16, tag="g6b")
                    one_m = tm("one_m")
                    TS2(one_m[:, :], vmask[:, :], 1.0, -1e-6, AL.subtract, AL.mult)  # (1-vm)*1e-6
                    rr = tm("rr"); bb_ = tm("bb"); ss = tm("ss")
                    ppv = tm("ppv"); qqv = tm("qqv")

                    def abt6(k0):  # [x(6) outer, chunk(224) inner], stride 18 per chunk
                        sl = abt[:, :]
                        return bass.AP(sl.tensor, sl.offset + k0, [list(sl.ap[0]), [1, 6], [18, 224]])

                    def gcols(c):  # G cols c*7 .. c*7+5: [x outer, chunk inner]
                        sl = Gt[:, :]
                        return bass.AP(sl.tensor, sl.offset + c * 7 * 224, [list(sl.ap[0]), [224, 6], [1, 224]])

                    def bc6(t):    # broadcast [128,224] over 6 x-cols
                        sl = t[:, :]
                        return bass.AP(sl.tensor, sl.offset, [list(sl.ap[0]), [0, 6], [1, 224]])

                    for c in range(3):
                        TT(rr[:, :], i1q(c), sq(8 + c), op=AL.subtract)
                        TT(rr[:, :], rr[:, :], vmask[:, :], op=AL.mult)
                        TT(rr[:, :], rr[:, :], one_m[:, :], op=AL.add)
                        nc.scalar.activation(bb_[:, :], rr[:, :], ACT.Abs)
                        TS(bb_[:, :], bb_[:, :], HUBER_DELTA, AL.max)
                        nc.vector.reciprocal_approx_fast(bb_[:, :], bb_[:, :])
                        nc.scalar.activation(ss[:, :], bb_[:, :], ACT.Sqrt, scale=HUBER_DELTA)
                        TT(ppv[:, :], ss[:, :], sq(0 + c), op=AL.mult)
                        TT(qqv[:, :], ss[:, :], sq(3 + c), op=AL.mult)
                        TT(g6a[:, :], abt6(0), bc6(ppv), op=AL.mult)
                        TT(g6b[:, :], abt6(6), bc6(qqv), op=AL.mult)
                        TT(gcols(c), g6a[:, :], g6b[:, :], op=AL.add)
                        TT(Gt[:, (c * 7 + 6) * 224:(c * 7 + 7) * 224], ss[:, :], rr[:, :], op=AL.mult)
                    # depth channel
                    TT(rr[:, :], iz[:, :], sq(11), op=AL.subtract)
                    TT(rr[:, :], rr[:, :], vmask[:, :], op=AL.mult)
                    TT(rr[:, :], rr[:, :], one_m[:, :], op=AL.add)
                    nc.scalar.activation(bb_[:, :], rr[:, :], ACT.Abs, scale=LAMBDA)
                    TS(bb_[:, :], bb_[:, :], HUBER_DELTA, AL.max)
                    nc.vector.reciprocal_approx_fast(bb_[:, :], bb_[:, :])
                    nc.scalar.activation(ss[:, :], bb_[:, :], ACT.Sqrt,
                                         scale=HUBER_DELTA * LAMBDA * LAMBDA)
                    TT(ppv[:, :], ss[:, :], sq(6), op=AL.mult)
                    TT(qqv[:, :], ss[:, :], sq(7), op=AL.mult)
                    TT(g6a[:, :], abt6(0), bc6(ppv), op=AL.mult)
                    TT(g6b[:, :], abt6(6), bc6(qqv), op=AL.mult)
                    TT(g6a[:, :], g6a[:, :], g6b[:, :], op=AL.add)
                    TT(g6b[:, :], abt6(12), bc6(ss), op=AL.mult)
                    TT(gcols(3), g6a[:, :], g6b[:, :], op=AL.add)
                    TT(Gt[:, (3 * 7 + 6) * 224:(3 * 7 + 7) * 224], ss[:, :], rr[:, :], op=AL.mult)

                    # ---------- PE: JtWJ accumulation ----------
                    for g in range(8):
                        b = g // 4
                        for a in range(A):
                            off = Gt.offset + g * A + a
                            gap = bass.AP(Gt.tensor, off, [list(Gt.ap[0]), [224, 28]])
                            first = (s == 0 and (g % 4) == 0 and a == 0)
                            last = (s == NS - 1 and (g % 4) == 3 and a == A - 1)
                            nc.tensor.matmul(psJ[b][:, :], gap, gap,
                                             start=first, stop=last,
                                             skip_group_check=True)

                # ---------- per-iteration: extract JtWJ/Rhs, solve, update pose ----------
                S28 = sc.tile([28, 56], fp32, tag="S28")
                for b in range(2):
                    nc.vector.tensor_copy(S28[:, b * 28:(b + 1) * 28], psJ[b][:, :])
                D28 = sc.tile([7, 56], fp32, tag="D28")
                for b in range(2):
                    for c in range(4):
                        src = S28[c * 7:(c + 1) * 7, b * 28 + c * 7:b * 28 + c * 7 + 7]
                        dsl = D28[:, b * 28 + c * 7:b * 28 + c * 7 + 7]
                        nc.sync.dma_start(out=dsl, in_=src)
                M7 = sc.tile([7, 14], fp32, tag="M7")
                for b in range(2):
                    din = bass.AP(D28.tensor, D28.offset + b * 28,
                                  [list(D28.ap[0]), [1, 7], [7, 4]])
                    nc.vector.tensor_reduce(M7[:, b * 7:(b + 1) * 7], din, axis=AX.X, op=AL.add)
                # Mb [2, 49]: row b = M7 block b flattened (x-major)
                Mb = sc.tile([2, 49], fp32, tag="Mb")
                for b in range(2):
                    msrc = bass.AP(M7.tensor, M7.offset + b * 7, [[M7.ap[0][0], 7], [1, 7]])
                    mdsl = Mb[b:b + 1, 0:1]
                    mdst = bass.AP(mdsl.tensor, mdsl.offset, [[Mb.ap[0][0], 1], [7, 7], [1, 7]])
                    nc.sync.dma_start(out=mdst, in_=msrc)
                # tr = sum diag(JtWJ); LM ridge on diag
                trt = sc.tile([2, 1], fp32, tag="trt")
                diag = bass.AP(Mb.tensor, Mb.offset, [list(Mb.ap[0]), [8, 6]])
                nc.vector.tensor_reduce(trt[:, :], diag, axis=AX.X, op=AL.add)
                trb = bass.AP(trt.tensor, trt.offset, [list(trt.ap[0]), [0, 6]])
                STT(diag, trb, 1e-6, diag, AL.mult, AL.add)

                # Cholesky LL^T = Hm (6x6, both batches in 2 partitions)
                Lt = sc.tile([2, 36], fp32, tag="Lt")
                lsrc = bass.AP(Mb.tensor, Mb.offset, [list(Mb.ap[0]), [7, 6], [1, 6]])
                nc.vector.tensor_copy(Lt[:, :], lsrc)
                rhs = sc.tile([2, 6], fp32, tag="rhs")
                rsrc = bass.AP(Mb.tensor, Mb.offset + 6, [list(Mb.ap[0]), [7, 6]])
                nc.vector.tensor_copy(rhs[:, :], rsrc)
                idg = sc.tile([2, 6], fp32, tag="idg")
                tmpj = sc.tile([2, 36], fp32, tag="tmpj")
                red = sc.tile([2, 6], fp32, tag="redj")
                for j in range(6):
                    jj = Lt[:, 6 * j + j:6 * j + j + 1]
                    if j > 0:
                        ljk = Lt[:, 6 * j:6 * j + j]
                        TT(tmpj[:, :j], ljk, ljk, op=AL.mult)
                        nc.vector.tensor_reduce(red[:, 0:1], tmpj[:, :j], axis=AX.X, op=AL.add)
                        TT(jj, jj, red[:, 0:1], op=AL.subtract)
                    nc.scalar.activation(jj, jj, ACT.Sqrt)
                    nc.vector.reciprocal(idg[:, j:j + 1], jj)
                    nr = 5 - j
                    if nr > 0:
                        colap = bass.AP(Lt.tensor, Lt.offset + 6 * (j + 1) + j, [list(Lt.ap[0]), [6, nr]])
                        if j > 0:
                            lik = bass.AP(Lt.tensor, Lt.offset + 6 * (j + 1), [list(Lt.ap[0]), [6, nr], [1, j]])
                            ljkb = bass.AP(Lt.tensor, Lt.offset + 6 * j, [list(Lt.ap[0]), [0, nr], [1, j]])
                            TT(tmpj[:, :nr * j], lik, ljkb, op=AL.mult)
                            tin = bass.AP(tmpj.tensor, tmpj.offset, [list(tmpj.ap[0]), [j, nr], [1, j]])
                            nc.vector.tensor_reduce(red[:, :nr], tin, axis=AX.X, op=AL.add)
                            TT(colap, colap, red[:, :nr], op=AL.subtract)
                        nc.vector.tensor_scalar(colap, colap, idg[:, j:j + 1], None, AL.mult)
                # forward substitution: L y = rhs (in place on rhs)
                for j in range(6):
                    yj = rhs[:, j:j + 1]
                    if j > 0:
                        ljk = Lt[:, 6 * j:6 * j + j]
                        TT(tmpj[:, :j], ljk, rhs[:, :j], op=AL.mult)
                        nc.vector.tensor_reduce(red[:, 0:1], tmpj[:, :j], axis=AX.X, op=AL.add)
                        TT(yj, yj, red[:, 0:1], op=AL.subtract)
                    nc.vector.tensor_scalar(yj, yj, idg[:, j:j + 1], None, AL.mult)
                # back substitution: L^T x = y -> xi = -x stored in xi tile
                for j in range(5, -1, -1):
                    xj = rhs[:, j:j + 1]
                    nk = 5 - j
                    if nk > 0:
                        lkj = bass.AP(Lt.tensor, Lt.offset + 6 * (j + 1) + j, [list(Lt.ap[0]), [6, nk]])
                        TT(tmpj[:, :nk], lkj, rhs[:, j + 1:6], op=AL.mult)
                        nc.vector.tensor_reduce(red[:, 0:1], tmpj[:, :nk], axis=AX.X, op=AL.add)
                        TT(xj, xj, red[:, 0:1], op=AL.subtract)
                    nc.vector.tensor_scalar(xj, xj, idg[:, j:j + 1], None, AL.mult)
                xi = sc.tile([2, 6], fp32, tag="xi")
                TS(xi[:, :], rhs[:, :], -1.0, AL.mult)

                # se3_exp(xi) via Taylor series (|w| << 1 in this regime)
                w3 = xi[:, 3:6]
                wsq = sc.tile([2, 3], fp32, tag="wsq")
                TT(wsq[:, :], w3, w3, op=AL.mult)
                th2 = sc.tile([2, 1], fp32, tag="th2")
                nc.vector.tensor_reduce(th2[:, :], wsq[:, :], axis=AX.X, op=AL.add)
                coA = sc.tile([2, 1], fp32, tag="coA")
                coB = sc.tile([2, 1], fp32, tag="coB")
                coC = sc.tile([2, 1], fp32, tag="coC")
                hh = sc.tile([2, 1], fp32, tag="hh")
                TS2(hh[:, :], th2[:, :], 1.0 / 120.0, -1.0 / 6.0, AL.mult, AL.add)
                nc.vector.tensor_scalar(coA[:, :], th2[:, :], hh[:, :], 1.0, AL.mult, AL.add)
                TS2(hh[:, :], th2[:, :], 1.0 / 720.0, -1.0 / 24.0, AL.mult, AL.add)
                nc.vector.tensor_scalar(coB[:, :], th2[:, :], hh[:, :], 0.5, AL.mult, AL.add)
                TS2(hh[:, :], th2[:, :], 1.0 / 5040.0, -1.0 / 120.0, AL.mult, AL.add)
                nc.vector.tensor_scalar(coC[:, :], th2[:, :], hh[:, :], 1.0 / 6.0, AL.mult, AL.add)
                # K, K2
                Kt = sc.tile([2, 9], fp32, tag="Kt")
                nc.vector.memset(Kt[:, :], 0.0)
                TS(Kt[:, 1:2], xi[:, 5:6], -1.0, AL.mult)   # -z
                nc.vector.tensor_copy(Kt[:, 2:3], xi[:, 4:5])  # y
                nc.vector.tensor_copy(Kt[:, 3:4], xi[:, 5:6])  # z
                TS(Kt[:, 5:6], xi[:, 3:4], -1.0, AL.mult)   # -x
                TS(Kt[:, 6:7], xi[:, 4:5], -1.0, AL.mult)   # -y
                nc.vector.tensor_copy(Kt[:, 7:8], xi[:, 3:4])  # x
                K2t = sc.tile([2, 9], fp32, tag="K2t")
                wiap = bass.AP(xi.tensor, xi.offset + 3, [list(xi.ap[0]), [1, 3], [0, 3]])
                wjap = bass.AP(xi.tensor, xi.offset + 3, [list(xi.ap[0]), [0, 3], [1, 3]])
                TT(K2t[:, :], wiap, wjap, op=AL.mult)
                k2diag = bass.AP(K2t.tensor, K2t.offset, [list(K2t.ap[0]), [4, 3]])
                nc.vector.tensor_scalar(k2diag, k2diag, th2[:, :], None, AL.subtract)
                Rt = sc.tile([2, 9], fp32, tag="Rt")
                Vt = sc.tile([2, 9], fp32, tag="Vt")
                t9 = sc.tile([2, 9], fp32, tag="t9")
                nc.vector.tensor_scalar(Rt[:, :], Kt[:, :], coA[:, :], None, AL.mult)
                nc.vector.tensor_scalar(t9[:, :], K2t[:, :], coB[:, :], None, AL.mult)
                TT(Rt[:, :], Rt[:, :], t9[:, :], op=AL.add)
                rdiag = bass.AP(Rt.tensor, Rt.offset, [list(Rt.ap[0]), [4, 3]])
                TS(rdiag, rdiag, 1.0, AL.add)
                nc.vector.tensor_scalar(Vt[:, :], Kt[:, :], coB[:, :], None, AL.mult)
                nc.vector.tensor_scalar(t9[:, :], K2t[:, :], coC[:, :], None, AL.mult)
                TT(Vt[:, :], Vt[:, :], t9[:, :], op=AL.add)
                vdiag = bass.AP(Vt.tensor, Vt.offset, [list(Vt.ap[0]), [4, 3]])
                TS(vdiag, vdiag, 1.0, AL.add)
                # t = V @ v
                vbc = bass.AP(xi.tensor, xi.offset, [list(xi.ap[0]), [0, 3], [1, 3]])
                TT(t9[:, :], Vt[:, :], vbc, op=AL.mult)
                tv = sc.tile([2, 3], fp32, tag="tv")
                t9v = bass.AP(t9.tensor, t9.offset, [list(t9.ap[0]), [3, 3], [1, 3]])
                nc.vector.tensor_reduce(tv[:, :], t9v, axis=AX.X, op=AL.add)
                # E = [[R, t],[0,0,0,1]] as [2,16]
                Et = sc.tile([2, 16], fp32, tag="Et")
                nc.vector.memset(Et[:, :], 0.0)
                edst = bass.AP(Et.tensor, Et.offset, [list(Et.ap[0]), [4, 3], [1, 3]])
                esrc = bass.AP(Rt.tensor, Rt.offset, [list(Rt.ap[0]), [3, 3], [1, 3]])
                nc.vector.tensor_copy(edst, esrc)
                edst2 = bass.AP(Et.tensor, Et.offset + 3, [list(Et.ap[0]), [4, 3]])
                nc.vector.tensor_copy(edst2, tv[:, :])
                TS(Et[:, 15:16], Et[:, 15:16], 1.0, AL.add)
                # newT = T @ E
                nT = sc.tile([2, 16], fp32, tag="nT")
                for k in range(4):
                    tcol = bass.AP(Tq.tensor, Tq.offset + k, [list(Tq.ap[0]), [4, 4], [0, 4]])
                    erow = bass.AP(Et.tensor, Et.offset + 4 * k, [list(Et.ap[0]), [0, 4], [1, 4]])
                    if k == 0:
                        TT(nT[:, :], tcol, erow, op=AL.mult)
                    else:
                        TT(tmpj[:, :16], tcol, erow, op=AL.mult)
                        TT(nT[:, :], nT[:, :], tmpj[:, :16], op=AL.add)
                nc.vector.tensor_copy(Tq[:, :], nT[:, :])
                # rebuild q = [R(9) | t(3) | intr(4)] and broadcast to rtm/rtw
                qt = sc.tile([2, 16], fp32, tag="qt")
                qr = bass.AP(Tq.tensor, Tq.offset, [list(Tq.ap[0]), [4, 3], [1, 3]])
                nc.vector.tensor_copy(qt[:, 0:9], qr)
                qtcol = bass.AP(Tq.tensor, Tq.offset + 3, [list(Tq.ap[0]), [4, 3]])
                nc.vector.tensor_copy(qt[:, 9:12], qtcol)
                nc.vector.tensor_copy(qt[:, 12:16], intr[:, :])
                nc.sync.dma_start(out=qscr.ap(), in_=qt[:, :])
                qsap = qscr.ap()
                for b in range(2):
                    qsrc = bass.AP(qsap.tensor, qsap.offset + b * 16, [[0, 64], [1, 16]])
                    nc.sync.dma_start(out=rtw[b * 64:(b + 1) * 64, :], in_=qsrc)
                for g in range(8):
                    b = g // 4
                    qsrc = bass.AP(qsap.tensor, qsap.offset + b * 16, [[0, 128], [1, 16]])
                    rdst = bass.AP(rtm.tensor, rtm.offset + g, [list(rtm.ap[0]), [8, 16]])
                    nc.sync.dma_start(out=rdst, in_=qsrc)

            nc.sync.dma_start(out=tout_ext.ap(), in_=Tq[:, :])

    nc.finalize()
    return nc


def kernel(pose_twist, I0, I1, invD0, invD1, intrinsics):
    from concourse.bass_utils import run_bass_kernel_spmd

    nc = _NC_CACHE.get("nc")
    if nc is None:
        nc = build_nc()
        _NC_CACHE["nc"] = nc

    pose_twist = np.asarray(pose_twist, np.float32)
    I0 = np.asarray(I0, np.float32); I1 = np.asarray(I1, np.float32)
    invD0 = np.asarray(invD0, np.float32); invD1 = np.asarray(invD1, np.float32)
    intrinsics = np.asarray(intrinsics, np.float32)

    import time as _time
    LAST_WALL.clear(); LAST_EXEC_NS.clear(); LAST_TRACES.clear()
    t0 = _time.time()
    in_maps, _ = host_precompute_all(pose_twist, I0, I1, invD0, invD1, intrinsics)
    t1 = _time.time()
    res = run_bass_kernel_spmd(nc, in_maps, list(range(8)), trace=PROFILE)
    t2 = _time.time()
    LAST_WALL.extend([round(t1 - t0, 3), round(t2 - t1, 3)])
    if PROFILE:
        if res.exec_time_ns is not None:
            LAST_EXEC_NS.append(res.exec_time_ns)
        if res.instructions_and_trace is not None:
            LAST_TRACES.append(res.instructions_and_trace[1])

    outs = []
    for core in range(8):
        outs.append(res.results[core]["tout"].reshape(2, 4, 4))
    return np.concatenate(outs, axis=0).astype(np.float32)
